# revision 1
# baseline (speedup 1.0000x reference)
"""ViT-Base encoder (12 layers, B=32, S=197, D=768, H=12, I=3072) on 8 trn2
NeuronCores, data-parallel over the batch (4 images per core).

Layout: activations are kept feature-major [D, T] in SBUF (features on
partitions, tokens on the free dim), so every projection chains on the
TensorEngine without transposes.  v is produced directly in transposed
layout [T, H*64]; softmax denominators come from ones-matmuls that land
pre-broadcast in PSUM rows 64-127 of each head-pair tile.  LayerNorm
stats are computed with ones-matmuls on a bf16 shadow (partition
reduction on PE); gamma/beta and all linear biases are folded into the
weights host-side.  Matmul-heavy paths run bf16; the residual stream,
LN stats and softmax denominators stay fp32.
"""

import sys

sys.path.insert(0, "/opt/trn_rl_repo")

import contextlib

import numpy as np
import ml_dtypes

import concourse.bass as bass
import concourse.mybir as mybir
import concourse.tile as tile
from concourse.vector_clock import ScopedClock
from concourse.bass_utils import run_bass_kernel_spmd

L, D, I, H, DH = 12, 768, 3072, 12, 64
B, S = 32, 197
NCORES = 8
BPC = B // NCORES  # batches per core
T = BPC * S  # 788 tokens per core
SCALE = float(1.0 / np.sqrt(DH))
EPS = 1e-5

F32 = mybir.dt.float32
BF16 = mybir.dt.bfloat16
AF = mybir.ActivationFunctionType
ALU = mybir.AluOpType

KD = D // 128  # 6 contraction chunks over D
KI = I // 128  # 24 contraction chunks over I
MD = D // 128  # 6 output tiles over D
MI = I // 128  # 24 output tiles over I

NCH = [(0, 512), (512, T - 512)]  # token chunks for dense matmuls
VW = H * 128  # 1536: per head [64 v-cols | 64 ones-cols]
VCH = [(0, 512), (512, 512), (1024, 512)]  # chunks of the v output width
TCH = [(0, 128), (128, S - 128)]  # within-batch token chunks (128+69)


class SplitDrainTileContext(tile.TileContext):
    """TileContext whose kernel-tail drain splits its sem waits across
    multiple SP instructions (this walrus rejects >1 wait on a Drain)."""

    def _drain_and_barrier(self, tick_clock, wait_clock):
        nc = self.nc
        drain_inst = nc.sync.drain()
        wait_clock.add_sem_waits(
            drain_inst.ins, ScopedClock({None: tick_clock.global_clock})
        )
        si = drain_inst.ins.sync_info
        waits = list(si.on_wait) if si is not None else []
        if len(waits) > 1:
            drain_inst.ins.sync_info = mybir.SyncInfo(
                on_wait=[waits[0]], on_update=list(si.on_update)
            )
            by_name = {}
            for h in self.sems.allocated().values():
                by_name[getattr(h, "name", None)] = h
            for w in waits[1:]:
                h = by_name.get(w.ant_name)
                assert h is not None, f"no handle for sem {w.ant_name}"
                nc.sync.wait_ge(h, w.wait_value)

        nc.all_engine_barrier()
        assert self.sems is not None
        popped = nc._tile_sem_poison_stack.pop()
        assert popped is self._sem_poison
        nc.clear_and_free_semaphores(list(self.sems.allocated().values()))
        nc.all_engine_barrier()


def _dedup_ldweights(nc):
    """Remove Ldweights whose weights are already resident in the PE array
    (identical signature to the previous Ldweights, nothing invalidated the
    array in between).  Carried sem waits/updates move to the next PE
    instruction; _split_multiwaits hoists any overflow afterwards."""
    removed = 0
    for fn in nc.m.functions:
        for bb in fn.blocks:
            lst = bb.instructions
            last_sig = None
            keep = []
            pending_waits = []
            pending_updates = []
            for inst in lst:
                eng = inst.engine
                if inst.opcode == "Ldweights":
                    sig = (
                        str(inst.ins[0]),
                        str(getattr(inst, "is_transpose", None)),
                        str(getattr(inst, "perf_mode", None)),
                        str(getattr(inst, "tile_position", None)),
                    )
                    if sig == last_sig:
                        si = inst.sync_info
                        if si is not None:
                            pending_waits.extend(si.on_wait)
                            pending_updates.extend(si.on_update)
                        removed += 1
                        continue
                    last_sig = sig
                elif inst.opcode == "Matmult" and str(
                    getattr(inst, "is_transpose", None)
                ) not in ("None", "False"):
                    last_sig = None  # transpose-mode clobbers the array
                if (pending_waits or pending_updates) and eng == mybir.EngineType.PE:
                    si = inst.sync_info
                    ow = list(si.on_wait) if si else []
                    ou = list(si.on_update) if si else []
                    inst.sync_info = mybir.SyncInfo(
                        on_wait=ow + pending_waits, on_update=ou + pending_updates
                    )
                    pending_waits, pending_updates = [], []
                keep.append(inst)
            assert not pending_waits and not pending_updates
            lst[:] = keep
    return removed


def _split_multiwaits(nc):
    """This walrus accepts at most 1 sem wait per instruction (2 on an
    EventSemaphore).  Tile freely packs several; hoist the excess into
    standalone EventSemaphore instructions inserted just before."""
    n = 0
    for fn in nc.m.functions:
        for bb in fn.blocks:
            lst = bb.instructions
            i = 0
            while i < len(lst):
                inst = lst[i]
                si = getattr(inst, "sync_info", None)
                if si is not None and si.on_wait:
                    cap = 2 if inst.opcode == "EventSemaphore" else 1
                    waits = list(si.on_wait)
                    if len(waits) > cap:
                        keep, extra = waits[:cap], waits[cap:]
                        new_insts = []
                        for j in range(0, len(extra), 2):
                            ev = mybir.InstEventSemaphore(
                                name=f"wsplit_{n}", ins=[], outs=[]
                            )
                            n += 1
                            ev.engine = inst.engine
                            ev.sync_info = mybir.SyncInfo(
                                on_wait=list(extra[j : j + 2]), on_update=[]
                            )
                            new_insts.append(ev)
                        inst.sync_info = mybir.SyncInfo(
                            on_wait=keep, on_update=list(si.on_update)
                        )
                        lst[i:i] = new_insts
                        i += len(new_insts)
                i += 1
    return n


def build(nlayers=L):
    nc = bass.Bass()

    # Dense stationary weights, pre-blocked host-side as
    # [L, NBLK, KD, 128, 128]: blocks 0-11 = q|k columns, 12-17 = Wo,
    # 18-41 = W1.  W2 is bf16-blocked [L, 6, KI, 128, 128].
    xT = nc.dram_tensor("xT", [D, T], F32, kind="ExternalInput")
    Wd_d = nc.dram_tensor("Wd", [nlayers, 42, KD, 128, 128], BF16, kind="ExternalInput")
    W2_d = nc.dram_tensor("W2", [nlayers, MD, KI, 128, 128], BF16, kind="ExternalInput")
    Wva_d = nc.dram_tensor("Wva", [nlayers, D + 1, VW], BF16, kind="ExternalInput")
    bqk_d = nc.dram_tensor("bqk", [nlayers, 2 * D], F32, kind="ExternalInput")
    bo_d = nc.dram_tensor("bo", [nlayers, D], F32, kind="ExternalInput")
    b1_d = nc.dram_tensor("b1", [nlayers, I], F32, kind="ExternalInput")
    b2_d = nc.dram_tensor("b2", [nlayers, D], F32, kind="ExternalInput")
    out_d = nc.dram_tensor("out", [D, T], F32, kind="ExternalOutput")

    with SplitDrainTileContext(nc) as tc, contextlib.ExitStack() as ctx, \
         nc.allow_low_precision(reason="bf16 activations; residual/LN stats stay fp32"):
        persist = ctx.enter_context(tc.tile_pool(name="persist", bufs=1))
        x_sb = persist.tile([128, MD, T], F32, tag="x")
        ones_row = persist.tile([1, 128], BF16, tag="ones_row")
        ones_col_b = persist.tile([128, 1], BF16, tag="ones_col_b")
        eps_t = persist.tile([1, 1], F32, tag="eps")
        nc.vector.memset(ones_row, 1.0)
        nc.vector.memset(ones_col_b, 1.0)
        nc.vector.memset(eps_t, EPS)

        for k in range(KD):
            nc.sync.dma_start(out=x_sb[:, k, :], in_=xT[128 * k : 128 * (k + 1), :])

        stat_pool = ctx.enter_context(tc.tile_pool(name="stats", bufs=1))
        xncat_pool = ctx.enter_context(tc.tile_pool(name="xncat", bufs=1))
        big_pool = ctx.enter_context(tc.tile_pool(name="big", bufs=1))
        vt_pool = ctx.enter_context(tc.tile_pool(name="vt", bufs=1))
        bias_pool = ctx.enter_context(tc.tile_pool(name="bias", bufs=2))
        wst_pool = ctx.enter_context(tc.tile_pool(name="wst", bufs=8))
        w2st_pool = ctx.enter_context(tc.tile_pool(name="w2st", bufs=4))
        wv_pool = ctx.enter_context(tc.tile_pool(name="wv", bufs=1))
        exp_pool = ctx.enter_context(tc.tile_pool(name="expt", bufs=6))
        dn_pool = ctx.enter_context(tc.tile_pool(name="dn", bufs=4))
        sq_pool = ctx.enter_context(tc.tile_pool(name="sq", bufs=13))
        lnt_pool = ctx.enter_context(tc.tile_pool(name="lnt", bufs=3))
        xb_pool = ctx.enter_context(tc.tile_pool(name="xb", bufs=1))

        class LNPipe:
            """LayerNorm over features (partitions), split into per-token-chunk
            stages so stats latency hides under neighbouring matmul phases.
            PSUM is only held transiently (2 banks in sums, 2 in finish)."""

            def __init__(self, name, src, dst):
                self.name, self.src, self.dst = name, src, dst
                self.mu = stat_pool.tile([1, T], F32, tag="mu", name=name + "_mu")
                self.va = stat_pool.tile([1, T], F32, tag="va", name=name + "_va")
                self.rs = stat_pool.tile([1, T], F32, tag="rs", name=name + "_rs")
                self.mu_b = stat_pool.tile([1, T], BF16, tag="mu_b", name=name + "_mub")
                self.rs_b = stat_pool.tile([1, T], BF16, tag="rs_b", name=name + "_rsb")
                self.xb = xb_pool.tile([128, KD, T], BF16, tag="xb", name=name + "_xb")
                self.sq_tiles = {}

            def prep(self, ci, k):
                """Shadow-copy + square one feature tile (emit as soon as
                x[:, k, chunk] is final so it overlaps the producing phase)."""
                off, sz = NCH[ci]
                cs = slice(off, off + sz)
                nc.gpsimd.tensor_copy(self.xb[:, k, cs], self.src[:, k, cs])
                sq = sq_pool.tile(
                    [128, 512], BF16, tag="sq", name=f"{self.name}_sq_{ci}_{k}"
                )
                nc.scalar.activation(sq[:, :sz], self.xb[:, k, cs], AF.Square)
                self.sq_tiles[(ci, k)] = sq

            def sums(self, ci):
                off, sz = NCH[ci]
                cs = slice(off, off + sz)
                for k in range(KD):
                    if (ci, k) not in self.sq_tiles:
                        self.prep(ci, k)
                with tc.tile_pool(
                    name=f"{self.name}_sps{ci}", bufs=1, space="PSUM"
                ) as sps:
                    sum_ps = sps.tile([1, 512], F32, tag="sum", name=f"{self.name}_sum{ci}")
                    ssq_ps = sps.tile([1, 512], F32, tag="ssq", name=f"{self.name}_ssq{ci}")
                    for k in range(KD):
                        nc.tensor.matmul(
                            sum_ps[:, :sz],
                            ones_col_b,
                            self.xb[:, k, cs],
                            start=(k == 0),
                            stop=(k == KD - 1),
                        )
                    for k in range(KD):
                        nc.tensor.matmul(
                            ssq_ps[:, :sz],
                            ones_col_b,
                            self.sq_tiles[(ci, k)][:, :sz],
                            start=(k == 0),
                            stop=(k == KD - 1),
                        )
                    nc.scalar.mul(self.mu[:, cs], sum_ps[:, :sz], 1.0 / D)
                    nc.scalar.mul(self.va[:, cs], ssq_ps[:, :sz], 1.0 / D)

            def finish(self, ci):
                off, sz = NCH[ci]
                cs = slice(off, off + sz)
                nc.vector.scalar_tensor_tensor(
                    self.rs[:, cs], self.mu[:, cs], -1.0, self.mu[:, cs],
                    ALU.mult, ALU.mult,
                )
                nc.vector.tensor_add(self.va[:, cs], self.va[:, cs], self.rs[:, cs])
                nc.scalar.activation(
                    self.rs[:, cs], self.va[:, cs], AF.Sqrt, bias=eps_t, scale=1.0
                )
                nc.vector.reciprocal(self.rs[:, cs], self.rs[:, cs])
                nc.vector.tensor_copy(self.mu_b[:, cs], self.mu[:, cs])
                nc.vector.tensor_copy(self.rs_b[:, cs], self.rs[:, cs])
                with tc.tile_pool(
                    name=f"{self.name}_bps{ci}", bufs=1, space="PSUM"
                ) as bps:
                    bmu = bps.tile(
                        [128, 512], F32, tag="bmu", name=f"{self.name}_bmu{ci}"
                    )
                    brs = bps.tile(
                        [128, 512], F32, tag="brs", name=f"{self.name}_brs{ci}"
                    )
                    nc.tensor.matmul(bmu[:, :sz], ones_row, self.mu_b[:, cs])
                    nc.tensor.matmul(brs[:, :sz], ones_row, self.rs_b[:, cs])
                    for k in range(KD):
                        lnt = lnt_pool.tile(
                            [128, 512], F32, tag="lnt", name=f"{self.name}_lnt_{ci}_{k}"
                        )
                        nc.vector.tensor_sub(
                            lnt[:, :sz], self.src[:, k, cs], bmu[:, :sz]
                        )
                        nc.vector.tensor_mul(
                            self.dst[:, k, cs], lnt[:, :sz], brs[:, :sz]
                        )

            def close(self):
                pass

        def dense_block(l, blk):
            """Stream one [768,128] stationary block (all KD chunks)."""
            wt = wst_pool.tile([128, KD, 128], BF16, tag="wst", name=f"wt_{l}_{blk}")
            nc.sync.dma_start(out=wt, in_=Wd_d[l, blk].rearrange("k p c -> p k c"))
            return wt

        ln1 = ln2 = None
        for l in range(nlayers):
            wv = wv_pool.tile([128, KD, VW], BF16, tag="wv", name=f"wv_{l}")
            for k in range(KD):
                nc.sync.dma_start(
                    out=wv[:, k, :], in_=Wva_d[l, 128 * k : 128 * (k + 1), :]
                )
            wv_aug = wv_pool.tile([1, VW], BF16, tag="wv_aug", name=f"wva_{l}")
            nc.sync.dma_start(out=wv_aug, in_=Wva_d[l, D : D + 1, :])
            bqk_sb = bias_pool.tile([128, 2 * MD], F32, tag="bqk", name=f"bqk_{l}")
            nc.sync.dma_start(out=bqk_sb, in_=bqk_d[l].rearrange("(m p) -> p m", p=128))

            # ---------------- LN1 ----------------
            xn = xncat_pool.tile([128, KD, T], BF16, tag="xncat", name=f"xn_{l}")
            if ln1 is None:  # first layer: sums not yet emitted by a W2 phase
                ln1 = LNPipe(f"ln1_{l}", x_sb, xn)
                ln1.sums(0)
                ln1.sums(1)
            ln1.dst = xn
            ln1.finish(0)
            ln1.finish(1)

            # ---------------- q, k projections (chunk-paired) -------------
            qk_sb = big_pool.tile([128, 2 * MD, T], BF16, tag="big", name=f"qk_{l}")
            with tc.tile_pool(name=f"qkps_{l}", bufs=6, space="PSUM") as qkps:
                for m in range(2 * MD):
                    wt = dense_block(l, m)
                    ps = [
                        qkps.tile([128, 512], F32, tag="ps", name=f"qkps_{l}_{m}_{ci}")
                        for ci in range(2)
                    ]
                    for k in range(KD):
                        for ci, (off, sz) in enumerate(NCH):
                            nc.tensor.matmul(
                                ps[ci][:, :sz],
                                wt[:, k, :],
                                xn[:, k, off : off + sz],
                                start=(k == 0),
                                stop=(k == KD - 1),
                            )
                    for ci, (off, sz) in enumerate(NCH):
                        nc.scalar.activation(
                            qk_sb[:, m, off : off + sz],
                            ps[ci][:, :sz],
                            AF.Identity,
                            bias=bqk_sb[:, m : m + 1],
                        )
            q_sb = qk_sb[:, 0:MD, :]
            k_sb = qk_sb[:, MD : 2 * MD, :]
            ln1.close()
            ln1 = None

            # -------- vT (transposed v + bias via K=1 ones row) -----------
            # Layout per head: [64 v-cols | 64 ones-cols]; ones are memset so
            # one M=128 matmul later yields numerator (rows 0-63) AND the
            # replicated softmax denominator (rows 64-127) in one shot.
            vt_sb = vt_pool.tile([128, 2 * BPC, VW], BF16, tag="vt", name=f"vt_{l}")
            for i in range(2 * BPC):
                ones_view = vt_sb[:, i, :].rearrange("p (h x) -> p h x", x=128)
                nc.gpsimd.memset(ones_view[:, :, 64:128], 1.0)
            with tc.tile_pool(name=f"vtps_{l}", bufs=4, space="PSUM") as vtps:
                for b in range(BPC):
                    for c, (toff, tsz) in enumerate(TCH):
                        cols = S * b + toff
                        ps = [
                            vtps.tile(
                                [128, 512], F32, tag="ps", name=f"vtps_{l}_{b}_{c}_{n}"
                            )
                            for n in range(3)
                        ]
                        for k in range(KD):
                            for n, (off, sz) in enumerate(VCH):
                                nc.tensor.matmul(
                                    ps[n][:tsz, :sz],
                                    xn[:, k, cols : cols + tsz],
                                    wv[:, k, off : off + sz],
                                    start=(k == 0),
                                    stop=False,
                                )
                        for n, (off, sz) in enumerate(VCH):
                            nc.tensor.matmul(
                                ps[n][:tsz, :sz],
                                ones_row[:, :tsz],
                                wv_aug[:, off : off + sz],
                                start=False,
                                stop=True,
                            )
                        dstv = vt_sb[:tsz, 2 * b + c, :].rearrange(
                            "p (h x) -> p h x", x=128
                        )
                        for n, (off, sz) in enumerate(VCH):
                            nc.vector.tensor_copy(
                                dstv[:, 4 * n : 4 * n + 4, 0:64],
                                ps[n][:tsz, :].rearrange(
                                    "p (h x) -> p h x", x=128
                                )[:, :, 0:64],
                            )

            # ---------------- attention ----------------
            # Per-head PSUM tiles (numerator rows 0-63, replicated softmax
            # denominator rows 64-127), 4-head score lookahead so the PE
            # stream stays dense despite the scores->exp->attn chain.
            cat_sb = xncat_pool.tile([128, MD, T], BF16, tag="xncat", name=f"cat_{l}")
            with tc.tile_pool(name=f"scps_{l}", bufs=4, space="PSUM") as scps, \
                 tc.tile_pool(name=f"bcps_{l}", bufs=1, space="PSUM") as bcps, \
                 tc.tile_pool(name=f"atps_{l}", bufs=3, space="PSUM") as atps:
                for b in range(BPC):
                    head_ps = {}
                    exp_tiles = {}

                    def emit_scores(h, b=b, exp_tiles=exp_tiles):
                        j, half = h // 2, h % 2
                        rows = slice(64 * half, 64 * half + 64)
                        sps_t = scps.tile(
                            [128, 2 * S], F32, tag="ps", name=f"sc_{l}_{b}_{h}"
                        )
                        for c, (toff, tsz) in enumerate(TCH):
                            cols = S * b + toff
                            nc.tensor.matmul(
                                sps_t[:tsz, S * c : S * c + S],
                                k_sb[rows, j, cols : cols + tsz],
                                q_sb[rows, j, S * b : S * (b + 1)],
                                skip_group_check=True,
                            )
                            et = exp_pool.tile(
                                [128, S], BF16, tag="expT", name=f"et_{l}_{b}_{h}_{c}"
                            )
                            nc.scalar.activation(
                                et[:tsz, :],
                                sps_t[:tsz, S * c : S * c + S],
                                AF.Exp,
                                scale=SCALE,
                            )
                            exp_tiles[(h, c)] = et

                    def emit_attn(h, b=b, exp_tiles=exp_tiles, head_ps=head_ps):
                        ph = atps.tile(
                            [128, S], F32, tag="head", name=f"hps_{l}_{b}_{h}"
                        )
                        head_ps[h] = ph
                        for c, (toff, tsz) in enumerate(TCH):
                            nc.tensor.matmul(
                                ph,
                                vt_sb[:tsz, 2 * b + c, 128 * h : 128 * h + 128],
                                exp_tiles[(h, c)][:tsz, :],
                                start=(c == 0),
                                stop=(c == 1),
                            )

                    def emit_norm(h, b=b, head_ps=head_ps):
                        # reciprocal of ONE denominator row, broadcast across
                        # 64 partitions with a K=1 ones-matmul, then scale the
                        # numerators out of PSUM.
                        j, half = h // 2, h % 2
                        ph = head_ps.pop(h)
                        rr = dn_pool.tile(
                            [1, S], BF16, tag="recrow", name=f"rr_{l}_{b}_{h}"
                        )
                        nc.vector.reciprocal(rr, ph[64:65, :])
                        bcp = bcps.tile(
                            [64, S], F32, tag="bcp", name=f"bcp_{l}_{b}_{h}"
                        )
                        nc.tensor.matmul(bcp, ones_row[:, 0:64], rr)
                        bc = dn_pool.tile(
                            [64, S], F32, tag="bc", name=f"bc_{l}_{b}_{h}"
                        )
                        nc.vector.tensor_copy(bc, bcp)
                        nc.vector.tensor_mul(
                            cat_sb[64 * half : 64 * half + 64, j, S * b : S * (b + 1)],
                            ph[0:64, :],
                            bc,
                        )

                    for h in range(4):
                        emit_scores(h)
                    for h in range(H):
                        if h + 4 < H:
                            emit_scores(h + 4)
                        emit_attn(h)
                        emit_norm(h)

            # ------- Wo projection + residual (chunk-paired) --------------
            bo_sb = bias_pool.tile([128, MD], F32, tag="bo", name=f"bo_{l}")
            nc.sync.dma_start(out=bo_sb, in_=bo_d[l].rearrange("(m p) -> p m", p=128))
            ln2 = LNPipe(f"ln2_{l}", x_sb, None)
            with tc.tile_pool(name=f"wops_{l}", bufs=6, space="PSUM") as wops:
                for m in range(MD):
                    wt = dense_block(l, 12 + m)
                    ps = [
                        wops.tile([128, 512], F32, tag="ps", name=f"wops_{l}_{m}_{ci}")
                        for ci in range(2)
                    ]
                    for k in range(KD):
                        for ci, (off, sz) in enumerate(NCH):
                            nc.tensor.matmul(
                                ps[ci][:, :sz],
                                wt[:, k, :],
                                cat_sb[:, k, off : off + sz],
                                start=(k == 0),
                                stop=(k == KD - 1),
                            )
                    for ci, (off, sz) in enumerate(NCH):
                        nc.vector.scalar_tensor_tensor(
                            x_sb[:, m, off : off + sz],
                            ps[ci][:, :sz],
                            bo_sb[:, m : m + 1],
                            x_sb[:, m, off : off + sz],
                            ALU.add,
                            ALU.add,
                        )
                        ln2.prep(ci, m)
            ln2.sums(0)
            ln2.sums(1)

            # ---------------- LN2 ----------------
            xn2 = xncat_pool.tile([128, KD, T], BF16, tag="xncat", name=f"xn2_{l}")
            ln2.dst = xn2
            ln2.finish(0)
            ln2.finish(1)

            # ---------------- MLP (chunk-paired) ----------------
            b1_sb = bias_pool.tile([128, MI], F32, tag="b1", name=f"b1_{l}")
            nc.sync.dma_start(out=b1_sb, in_=b1_d[l].rearrange("(m p) -> p m", p=128))
            b2_sb = bias_pool.tile([128, MD], F32, tag="b2", name=f"b2_{l}")
            nc.sync.dma_start(out=b2_sb, in_=b2_d[l].rearrange("(m p) -> p m", p=128))
            h_sb = big_pool.tile([128, KI, T], BF16, tag="big", name=f"h_{l}")
            with tc.tile_pool(name=f"w1ps_{l}", bufs=6, space="PSUM") as w1ps:
                for m in range(MI):
                    wt = dense_block(l, 18 + m)
                    ps = [
                        w1ps.tile([128, 512], F32, tag="ps", name=f"w1ps_{l}_{m}_{ci}")
                        for ci in range(2)
                    ]
                    for k in range(KD):
                        for ci, (off, sz) in enumerate(NCH):
                            nc.tensor.matmul(
                                ps[ci][:, :sz],
                                wt[:, k, :],
                                xn2[:, k, off : off + sz],
                                start=(k == 0),
                                stop=(k == KD - 1),
                            )
                    for ci, (off, sz) in enumerate(NCH):
                        nc.scalar.activation(
                            h_sb[:, m, off : off + sz],
                            ps[ci][:, :sz],
                            AF.Gelu,
                            bias=b1_sb[:, m : m + 1],
                        )
            ln2.close()
            ln2 = None
            ln1 = LNPipe(f"ln1n_{l}", x_sb, None)
            with tc.tile_pool(name=f"w2ps_{l}", bufs=6, space="PSUM") as w2ps:
                for m in range(MD):
                    w2t = w2st_pool.tile(
                        [128, KI, 128], BF16, tag="w2st", name=f"w2t_{l}_{m}"
                    )
                    nc.sync.dma_start(
                        out=w2t, in_=W2_d[l, m].rearrange("k p c -> p k c")
                    )
                    ps = [
                        w2ps.tile([128, 512], F32, tag="ps", name=f"w2ps_{l}_{m}_{ci}")
                        for ci in range(2)
                    ]
                    for k in range(KI):
                        for ci, (off, sz) in enumerate(NCH):
                            nc.tensor.matmul(
                                ps[ci][:, :sz],
                                w2t[:, k, :],
                                h_sb[:, k, off : off + sz],
                                start=(k == 0),
                                stop=(k == KI - 1),
                            )
                    for ci, (off, sz) in enumerate(NCH):
                        nc.vector.scalar_tensor_tensor(
                            x_sb[:, m, off : off + sz],
                            ps[ci][:, :sz],
                            b2_sb[:, m : m + 1],
                            x_sb[:, m, off : off + sz],
                            ALU.add,
                            ALU.add,
                        )
                        if l + 1 < nlayers:
                            ln1.prep(ci, m)
            if l + 1 < nlayers:
                ln1.sums(0)
                ln1.sums(1)
            else:
                ln1.close()
                ln1 = None

        for k in range(KD):
            nc.sync.dma_start(out=out_d[128 * k : 128 * (k + 1), :], in_=x_sb[:, k, :])

    ndedup = _dedup_ldweights(nc)
    nsplit = _split_multiwaits(nc)
    print(f"dedup {ndedup} ldweights; split {nsplit} multi-wait instructions")
    return nc


def prep_weights(inputs, nlayers=L):
    """Fold gamma/beta/biases into effective weights, host side (numpy)."""
    f32 = np.float32
    Wq = np.asarray(inputs["Wq"], f32)
    bq = np.asarray(inputs["bq"], f32)
    Wk = np.asarray(inputs["Wk"], f32)
    bk = np.asarray(inputs["bk"], f32)
    Wv = np.asarray(inputs["Wv"], f32)
    bv = np.asarray(inputs["bv"], f32)
    Wo = np.asarray(inputs["Wo"], f32)
    bo = np.asarray(inputs["bo"], f32)
    W1 = np.asarray(inputs["W1"], f32)
    b1 = np.asarray(inputs["b1"], f32)
    W2 = np.asarray(inputs["W2"], f32)
    b2 = np.asarray(inputs["b2"], f32)
    g1 = np.asarray(inputs["g1"], f32)
    be1 = np.asarray(inputs["be1"], f32)
    g2 = np.asarray(inputs["g2"], f32)
    be2 = np.asarray(inputs["be2"], f32)

    Wqk = np.zeros((nlayers, D, 2 * D), f32)
    bqk = np.zeros((nlayers, 2 * D), f32)
    Wva = np.zeros((nlayers, D + 1, VW), f32)
    W1e = np.zeros((nlayers, D, I), f32)
    b1e = np.zeros((nlayers, I), f32)
    for l in range(nlayers):
        for h in range(H):
            Wqk[l, :, h * DH : (h + 1) * DH] = Wq[l, h] * g1[l][:, None]
            Wqk[l, :, D + h * DH : D + (h + 1) * DH] = Wk[l, h] * g1[l][:, None]
            bqk[l, h * DH : (h + 1) * DH] = bq[l, h] + Wq[l, h].T @ be1[l]
            bqk[l, D + h * DH : D + (h + 1) * DH] = bk[l, h] + Wk[l, h].T @ be1[l]
            Wva[l, :D, 128 * h : 128 * h + DH] = Wv[l, h] * g1[l][:, None]
            Wva[l, D, 128 * h : 128 * h + DH] = bv[l, h] + Wv[l, h].T @ be1[l]
        W1e[l] = W1[l] * g2[l][:, None]
        b1e[l] = b1[l] + W1[l].T @ be2[l]

    # blocked dense stationary tensor [L, 42, KD, 128, 128]
    Wd = np.zeros((nlayers, 42, KD, 128, 128), ml_dtypes.bfloat16)
    for l in range(nlayers):
        for m in range(12):
            Wd[l, m] = Wqk[l][:, 128 * m : 128 * (m + 1)].reshape(KD, 128, 128)
        for m in range(6):
            Wd[l, 12 + m] = Wo[l][:, 128 * m : 128 * (m + 1)].reshape(KD, 128, 128)
        for m in range(24):
            Wd[l, 18 + m] = W1e[l][:, 128 * m : 128 * (m + 1)].reshape(KD, 128, 128)
    W2b = np.zeros((nlayers, MD, KI, 128, 128), ml_dtypes.bfloat16)
    for l in range(nlayers):
        for m in range(MD):
            W2b[l, m] = (
                W2[l][:, 128 * m : 128 * (m + 1)]
                .reshape(KI, 128, 128)
                .astype(ml_dtypes.bfloat16)
            )

    return {
        "Wd": Wd,
        "W2": W2b,
        "Wva": Wva.astype(ml_dtypes.bfloat16),
        "bqk": bqk,
        "bo": np.ascontiguousarray(bo[:nlayers]),
        "b1": b1e,
        "b2": np.ascontiguousarray(b2[:nlayers]),
    }


_cache = {}


def run_cores(inputs, nlayers=L, trace=False):
    X = np.asarray(inputs["X"], np.float32)
    wmap = prep_weights(inputs, nlayers)

    key = ("nc", nlayers)
    if key not in _cache:
        _cache[key] = build(nlayers)
    nc = _cache[key]

    in_maps = []
    for c in range(NCORES):
        xc = X[BPC * c : BPC * (c + 1)].reshape(T, D).T  # [D, T]
        m = {"xT": np.ascontiguousarray(xc)}
        m.update(wmap)
        in_maps.append(m)

    res = run_bass_kernel_spmd(nc, in_maps, core_ids=list(range(NCORES)), trace=trace)
    out = np.zeros((B, S, D), np.float32)
    for c in range(NCORES):
        out[BPC * c : BPC * (c + 1)] = res.results[c]["out"].T.reshape(BPC, S, D)
    return out, res


def kernel(**inputs):
    out, _ = run_cores(inputs)
    return out



# revision 7
# speedup vs baseline: 1.2099x; 1.2099x over previous
"""ViT-Base encoder (12 layers, B=32, S=197, D=768, H=12, I=3072) on 8 trn2
NeuronCores, data-parallel over the batch (4 images per core).

v2: the attention block (q/k/v projections, Wo) and the LayerNorm stat
reductions run as fp8e4m3 DoubleRow matmuls (2 contraction rows per PE
cell, 2x bf16 throughput); the MLP stays bf16 (fp8 there costs ~6e-2
rel err).  Softmax normalization exploits the ones-columns trick: the
attention matmul leaves the denominator replicated on PSUM rows 64-127,
so a single [64,S] bf16 reciprocal + one multiply normalizes a head
(no PE broadcast, no per-head staging copies).  Activations feeding fp8
matmuls (xn, cat) are stored fp8; the residual stream and LN stats stay
fp32.
"""

import sys

sys.path.insert(0, "/opt/trn_rl_repo")

import contextlib

import numpy as np
import ml_dtypes

import concourse.bass as bass
import concourse.mybir as mybir
import concourse.tile as tile
from concourse.vector_clock import ScopedClock
from concourse.bass_utils import run_bass_kernel_spmd

L, D, I, H, DH = 12, 768, 3072, 12, 64
B, S = 32, 197
NCORES = 8
BPC = B // NCORES  # batches per core
T = BPC * S  # 788 tokens per core
SCALE = float(1.0 / np.sqrt(DH))
EPS = 1e-5

F32 = mybir.dt.float32
BF16 = mybir.dt.bfloat16
FP8 = mybir.dt.float8e4
AF = mybir.ActivationFunctionType
ALU = mybir.AluOpType
PM = mybir.MatmulPerfMode

KD = D // 128  # 6 contraction chunks over D
KI = I // 128  # 24 contraction chunks over I
MD = D // 128  # 6 output tiles over D
MI = I // 128  # 24 output tiles over I
KP = KD // 2  # 3 fp8 DoubleRow contraction pairs over D

NCH = [(0, 394), (394, 394)]  # PSUM-half chunks for dense matmul phases
LCH = [(0, 512), (512, T - 512)]  # chunks for LN/elementwise work
TCH = [(0, 128), (128, S - 128)]  # within-batch token chunks (128+69)
VW = H * 128  # vt tile: per head [64 v-cols | 64 ones-cols]


class SplitDrainTileContext(tile.TileContext):
    """TileContext whose kernel-tail drain splits its sem waits across
    multiple SP instructions (this walrus rejects >1 wait on a Drain)."""

    def _drain_and_barrier(self, tick_clock, wait_clock):
        nc = self.nc
        drain_inst = nc.sync.drain()
        wait_clock.add_sem_waits(
            drain_inst.ins, ScopedClock({None: tick_clock.global_clock})
        )
        si = drain_inst.ins.sync_info
        waits = list(si.on_wait) if si is not None else []
        if len(waits) > 1:
            drain_inst.ins.sync_info = mybir.SyncInfo(
                on_wait=[waits[0]], on_update=list(si.on_update)
            )
            by_name = {}
            for h in self.sems.allocated().values():
                by_name[getattr(h, "name", None)] = h
            for w in waits[1:]:
                h = by_name.get(w.ant_name)
                assert h is not None, f"no handle for sem {w.ant_name}"
                nc.sync.wait_ge(h, w.wait_value)

        nc.all_engine_barrier()
        assert self.sems is not None
        popped = nc._tile_sem_poison_stack.pop()
        assert popped is self._sem_poison
        nc.clear_and_free_semaphores(list(self.sems.allocated().values()))
        nc.all_engine_barrier()


def _dedup_ldweights(nc):
    """Remove Ldweights whose weights are already resident in the PE array
    (identical signature to the previous Ldweights, nothing invalidated the
    array in between).  Carried sem waits/updates move to the next PE
    instruction; _split_multiwaits hoists any overflow afterwards."""
    removed = 0
    for fn in nc.m.functions:
        for bb in fn.blocks:
            lst = bb.instructions
            last_sig = None
            keep = []
            pending_waits = []
            pending_updates = []
            for inst in lst:
                eng = inst.engine
                if inst.opcode == "Ldweights":
                    sig = (
                        str(inst.ins[0]),
                        str(getattr(inst, "is_transpose", None)),
                        str(getattr(inst, "perf_mode", None)),
                        str(getattr(inst, "tile_position", None)),
                    )
                    if sig == last_sig:
                        si = inst.sync_info
                        if si is not None:
                            pending_waits.extend(si.on_wait)
                            pending_updates.extend(si.on_update)
                        removed += 1
                        continue
                    last_sig = sig
                elif inst.opcode == "Matmult" and str(
                    getattr(inst, "is_transpose", None)
                ) not in ("None", "False"):
                    last_sig = None  # transpose-mode clobbers the array
                if (pending_waits or pending_updates) and eng == mybir.EngineType.PE:
                    si = inst.sync_info
                    ow = list(si.on_wait) if si else []
                    ou = list(si.on_update) if si else []
                    inst.sync_info = mybir.SyncInfo(
                        on_wait=ow + pending_waits, on_update=ou + pending_updates
                    )
                    pending_waits, pending_updates = [], []
                keep.append(inst)
            assert not pending_waits and not pending_updates
            lst[:] = keep
    return removed


def _split_multiwaits(nc):
    """This walrus accepts at most 1 sem wait per instruction (2 on an
    EventSemaphore).  Tile freely packs several; hoist the excess into
    standalone EventSemaphore instructions inserted just before."""
    n = 0
    for fn in nc.m.functions:
        for bb in fn.blocks:
            lst = bb.instructions
            i = 0
            while i < len(lst):
                inst = lst[i]
                si = getattr(inst, "sync_info", None)
                if si is not None and si.on_wait:
                    cap = 2 if inst.opcode == "EventSemaphore" else 1
                    waits = list(si.on_wait)
                    if len(waits) > cap:
                        keep, extra = waits[:cap], waits[cap:]
                        new_insts = []
                        for j in range(0, len(extra), 2):
                            ev = mybir.InstEventSemaphore(
                                name=f"wsplit_{n}", ins=[], outs=[]
                            )
                            n += 1
                            ev.engine = inst.engine
                            ev.sync_info = mybir.SyncInfo(
                                on_wait=list(extra[j : j + 2]), on_update=[]
                            )
                            new_insts.append(ev)
                        inst.sync_info = mybir.SyncInfo(
                            on_wait=keep, on_update=list(si.on_update)
                        )
                        lst[i:i] = new_insts
                        i += len(new_insts)
                i += 1
    return n


def build(nlayers=L):
    nc = bass.Bass()

    xT = nc.dram_tensor("xT", [D, T], F32, kind="ExternalInput")
    Wqk_d = nc.dram_tensor("Wqk", [nlayers, 12, KD, 128, 128], FP8, kind="ExternalInput")
    Wo_d = nc.dram_tensor("Wo", [nlayers, MD, KD, 128, 128], FP8, kind="ExternalInput")
    W1_d = nc.dram_tensor("W1", [nlayers, MI, KD, 128, 128], BF16, kind="ExternalInput")
    W2_d = nc.dram_tensor("W2", [nlayers, MD, KI, 128, 128], BF16, kind="ExternalInput")
    Wv_d = nc.dram_tensor("Wv", [nlayers, D, 768], FP8, kind="ExternalInput")
    Wvaug_d = nc.dram_tensor("Wvaug", [nlayers, 1, 768], BF16, kind="ExternalInput")
    bqk_d = nc.dram_tensor("bqk", [nlayers, 2 * D], F32, kind="ExternalInput")
    bo_d = nc.dram_tensor("bo", [nlayers, D], F32, kind="ExternalInput")
    b1_d = nc.dram_tensor("b1", [nlayers, I], F32, kind="ExternalInput")
    b2_d = nc.dram_tensor("b2", [nlayers, D], F32, kind="ExternalInput")
    out_d = nc.dram_tensor("out", [D, T], F32, kind="ExternalOutput")

    with SplitDrainTileContext(nc) as tc, contextlib.ExitStack() as ctx, \
         nc.allow_low_precision(reason="fp8 attention, bf16 MLP; residual/stats fp32"):
        persist = ctx.enter_context(tc.tile_pool(name="persist", bufs=1))
        x_sb = persist.tile([128, MD, T], F32, tag="x")
        ones_row = persist.tile([1, 128], BF16, tag="ones_row")
        eps_t = persist.tile([1, 1], F32, tag="eps")
        # DoubleRow stationaries for the LN partition sums: [K=128, pair, col]
        # col 0 sums the tile, col 1 sums the squares tile.
        ones2s = persist.tile([128, 2, 64], FP8, tag="ones2s")
        ones2q = persist.tile([128, 2, 64], FP8, tag="ones2q")
        # vt tile persists so its ones-columns are memset exactly once.
        vt_sb = persist.tile([128, 2 * BPC, VW], BF16, tag="vt")
        nc.vector.memset(ones_row, 1.0)
        nc.vector.memset(eps_t, EPS)
        nc.vector.memset(ones2s, 0.0)
        nc.vector.memset(ones2q, 0.0)
        nc.vector.memset(ones2s[:, :, 0:1], 1.0)
        nc.vector.memset(ones2q[:, :, 32:33], 1.0)
        for i in range(2 * BPC):
            ones_view = vt_sb[:, i, :].rearrange("p (h x) -> p h x", x=128)
            nc.gpsimd.memset(ones_view[:, :, 64:128], 1.0)

        for k in range(KD):
            nc.sync.dma_start(out=x_sb[:, k, :], in_=xT[128 * k : 128 * (k + 1), :])

        stat_pool = ctx.enter_context(tc.tile_pool(name="stats", bufs=1))
        xn8_pool = ctx.enter_context(tc.tile_pool(name="xn8", bufs=1))
        xn2_pool = ctx.enter_context(tc.tile_pool(name="xn2", bufs=1))
        qk_pool = ctx.enter_context(tc.tile_pool(name="qk", bufs=1))
        h_pool = ctx.enter_context(tc.tile_pool(name="h", bufs=1))
        bias_pool = ctx.enter_context(tc.tile_pool(name="bias", bufs=2))
        wst_pool = ctx.enter_context(tc.tile_pool(name="wst", bufs=8))
        w1st_pool = ctx.enter_context(tc.tile_pool(name="w1st", bufs=8))
        w2st_pool = ctx.enter_context(tc.tile_pool(name="w2st", bufs=4))
        wv_pool = ctx.enter_context(tc.tile_pool(name="wv", bufs=1))
        et_pool = ctx.enter_context(tc.tile_pool(name="expt", bufs=6))
        rec_pool = ctx.enter_context(tc.tile_pool(name="rec", bufs=3))
        xb_pool = ctx.enter_context(tc.tile_pool(name="xb", bufs=1))
        sq_pool = ctx.enter_context(tc.tile_pool(name="sq", bufs=1))
        lnt_pool = ctx.enter_context(tc.tile_pool(name="lnt", bufs=3))

        class LNPipe:
            """LayerNorm over features (partitions).  Stats come from an fp8
            shadow of x via DoubleRow ones-matmuls (sum into PSUM row 0,
            sum-of-squares into row 1); normalization multiplies the fp32
            residual by PE-broadcast stats."""

            def __init__(self, name, src, dst, dst_dtype):
                self.name, self.src, self.dst = name, src, dst
                self.dst_dtype = dst_dtype
                self.mu = stat_pool.tile([1, T], F32, tag="mu", name=name + "_mu")
                self.rs = stat_pool.tile([1, T], F32, tag="rs", name=name + "_rs")
                self.mu_b = stat_pool.tile([1, T], BF16, tag="mu_b", name=name + "_mub")
                self.rs_b = stat_pool.tile([1, T], BF16, tag="rs_b", name=name + "_rsb")
                self.xb = xb_pool.tile([128, KD, T], FP8, tag="xb", name=name + "_xb")
                self.sq = sq_pool.tile([128, KD, T], FP8, tag="sq", name=name + "_sq")
                self.prepped = set()

            def prep(self, ci, k):
                """fp8 shadow + squares for x[:, k, LCH[ci]] (emit as soon as
                that region is final so it overlaps the producing phase)."""
                off, sz = LCH[ci]
                cs = slice(off, off + sz)
                nc.gpsimd.tensor_copy(self.xb[:, k, cs], self.src[:, k, cs])
                nc.scalar.activation(self.sq[:, k, cs], self.xb[:, k, cs], AF.Square)
                self.prepped.add((ci, k))

            def sums(self, ci):
                off, sz = LCH[ci]
                cs = slice(off, off + sz)
                for k in range(KD):
                    if (ci, k) not in self.prepped:
                        self.prep(ci, k)
                with tc.tile_pool(
                    name=f"{self.name}_sps{ci}", bufs=1, space="PSUM"
                ) as sps:
                    sp = sps.tile([64, 512], F32, tag="sum", name=f"{self.name}_sum{ci}")
                    for p in range(KP):
                        ks = slice(2 * p, 2 * p + 2)
                        nc.tensor.matmul(
                            sp[:, :sz], ones2s, self.xb[:, ks, cs],
                            start=(p == 0), stop=False, perf_mode=PM.DoubleRow,
                            skip_group_check=True,
                        )
                        nc.tensor.matmul(
                            sp[:, :sz], ones2q, self.sq[:, ks, cs],
                            start=False, stop=(p == KP - 1), perf_mode=PM.DoubleRow,
                            skip_group_check=True,
                        )
                    nc.scalar.mul(self.mu[0:1, cs], sp[0:1, :sz], 1.0 / D)
                    # var = E[x^2] - mu^2, with E[x^2] read straight from PSUM
                    nc.vector.scalar_tensor_tensor(
                        self.rs[:, cs], self.mu[0:1, cs], -1.0, self.mu[0:1, cs],
                        ALU.mult, ALU.mult,
                    )
                    nc.vector.scalar_tensor_tensor(
                        self.rs[:, cs], sp[32:33, :sz], 1.0 / D, self.rs[:, cs],
                        ALU.mult, ALU.add,
                    )

            def finish(self, ci):
                off, sz = LCH[ci]
                cs = slice(off, off + sz)
                nc.scalar.activation(
                    self.rs[:, cs], self.rs[:, cs], AF.Sqrt, bias=eps_t, scale=1.0
                )
                nc.vector.reciprocal(self.rs[:, cs], self.rs[:, cs])
                nc.vector.tensor_copy(self.mu_b[:, cs], self.mu[0:1, cs])
                nc.vector.tensor_copy(self.rs_b[:, cs], self.rs[:, cs])
                with tc.tile_pool(
                    name=f"{self.name}_bps{ci}", bufs=1, space="PSUM"
                ) as bps:
                    bmu = bps.tile([128, 512], F32, tag="bmu", name=f"{self.name}_bmu{ci}")
                    brs = bps.tile([128, 512], F32, tag="brs", name=f"{self.name}_brs{ci}")
                    nc.tensor.matmul(bmu[:, :sz], ones_row, self.mu_b[:, cs])
                    nc.tensor.matmul(brs[:, :sz], ones_row, self.rs_b[:, cs])
                    for k in range(KD):
                        lnt = lnt_pool.tile(
                            [128, 512], F32, tag="lnt", name=f"{self.name}_lnt_{ci}_{k}"
                        )
                        nc.vector.tensor_sub(
                            lnt[:, :sz], self.src[:, k, cs], bmu[:, :sz]
                        )
                        nc.vector.tensor_mul(
                            self.dst[:, k, cs], lnt[:, :sz], brs[:, :sz]
                        )

        ln1 = ln2 = None
        for l in range(nlayers):
            wv = wv_pool.tile([128, KD, 768], FP8, tag="wv", name=f"wv_{l}")
            for k in range(KD):
                nc.sync.dma_start(
                    out=wv[:, k, :], in_=Wv_d[l, 128 * k : 128 * (k + 1), :]
                )
            wv_aug = wv_pool.tile([1, 768], BF16, tag="wv_aug", name=f"wva_{l}")
            nc.sync.dma_start(out=wv_aug, in_=Wvaug_d[l, :, :])
            bqk_sb = bias_pool.tile([128, 2 * MD], F32, tag="bqk", name=f"bqk_{l}")
            nc.sync.dma_start(out=bqk_sb, in_=bqk_d[l].rearrange("(m p) -> p m", p=128))

            # ---------------- LN1 -> xn (fp8) ----------------
            xn = xn8_pool.tile([128, KD, T], FP8, tag="xn", name=f"xn_{l}")
            if ln1 is None:  # first layer: sums not yet emitted by a W2 phase
                ln1 = LNPipe(f"ln1_{l}", x_sb, xn, FP8)
                ln1.sums(0)
                ln1.sums(1)
            ln1.dst = xn
            ln1.finish(0)
            ln1.finish(1)

            # ------------- q, k projections (fp8 DoubleRow) -------------
            qk_sb = qk_pool.tile([128, 2 * MD, T], BF16, tag="qk", name=f"qk_{l}")
            with tc.tile_pool(name=f"qkps_{l}", bufs=4, space="PSUM") as qkps:
                for m in range(2 * MD):
                    wt = wst_pool.tile(
                        [128, KD, 128], FP8, tag="wst", name=f"wt_{l}_{m}"
                    )
                    nc.sync.dma_start(
                        out=wt, in_=Wqk_d[l, m].rearrange("k p c -> p k c")
                    )
                    ps = [
                        qkps.tile([128, 394], F32, tag="ps", name=f"qkps_{l}_{m}_{ci}")
                        for ci in range(2)
                    ]
                    for p in range(KP):
                        ks = slice(2 * p, 2 * p + 2)
                        for ci, (off, sz) in enumerate(NCH):
                            nc.tensor.matmul(
                                ps[ci][:, :sz],
                                wt[:, ks, :],
                                xn[:, ks, off : off + sz],
                                start=(p == 0),
                                stop=(p == KP - 1),
                                perf_mode=PM.DoubleRow,
                            )
                    for ci, (off, sz) in enumerate(NCH):
                        nc.scalar.activation(
                            qk_sb[:, m, off : off + sz],
                            ps[ci][:, :sz],
                            AF.Identity,
                            bias=bqk_sb[:, m : m + 1],
                        )
            q_sb = qk_sb[:, 0:MD, :]
            k_sb = qk_sb[:, MD : 2 * MD, :]
            ln1 = None

            # -------- vT (fp8 DoubleRow; xn stationary, wv moving) --------
            # vt layout per head: [64 v-cols | 64 ones-cols]; the ones are
            # persistent so the attention matmul yields the numerator (rows
            # 0-63) AND the replicated softmax denominator (rows 64-127).
            with tc.tile_pool(name=f"vtps_{l}", bufs=4, space="PSUM") as vtps:
                for b in range(BPC):
                    for c, (toff, tsz) in enumerate(TCH):
                        cols = S * b + toff
                        ps = [
                            vtps.tile(
                                [128, 384], F32, tag="ps", name=f"vtps_{l}_{b}_{c}_{n}"
                            )
                            for n in range(2)
                        ]
                        for k in range(KD):
                            for n in range(2):
                                nc.tensor.matmul(
                                    ps[n][:tsz, :],
                                    xn[:, k, cols : cols + tsz],
                                    wv[:, k, 384 * n : 384 * (n + 1)],
                                    start=(k == 0),
                                    stop=False,
                                    skip_group_check=True,
                                )
                        for n in range(2):
                            nc.tensor.matmul(
                                ps[n][:tsz, :],
                                ones_row[:, :tsz],
                                wv_aug[:, 384 * n : 384 * (n + 1)],
                                start=False,
                                stop=True,
                                skip_group_check=True,
                            )
                        dstv = vt_sb[:tsz, 2 * b + c, :].rearrange(
                            "p (h x) -> p h x", x=128
                        )
                        for n in range(2):
                            nc.vector.tensor_copy(
                                dstv[:, 6 * n : 6 * n + 6, 0:64],
                                ps[n][:tsz, :].rearrange("p (h x) -> p h x", x=64),
                            )

            # ---------------- attention ----------------
            cat_sb = xn8_pool.tile([128, KD, T], FP8, tag="cat", name=f"cat_{l}")
            with tc.tile_pool(name=f"scps_{l}", bufs=3, space="PSUM") as scps, \
                 tc.tile_pool(name=f"atps_{l}", bufs=3, space="PSUM") as atps:
                for b in range(BPC):
                    exp_tiles = {}

                    def emit_scores(h, b=b, exp_tiles=exp_tiles):
                        j, half = h // 2, h % 2
                        rows = slice(64 * half, 64 * half + 64)
                        sps_t = scps.tile(
                            [128, 2 * S], F32, tag="ps", name=f"sc_{l}_{b}_{h}"
                        )
                        for c, (toff, tsz) in enumerate(TCH):
                            cols = S * b + toff
                            nc.tensor.matmul(
                                sps_t[:tsz, S * c : S * c + S],
                                k_sb[rows, j, cols : cols + tsz],
                                q_sb[rows, j, S * b : S * (b + 1)],
                                start=(c == 0),
                                stop=True,
                                skip_group_check=True,
                            )
                        et = et_pool.tile(
                            [128, 2 * S], BF16, tag="expT", name=f"et_{l}_{b}_{h}"
                        )
                        nc.scalar.activation(et, sps_t, AF.Exp, scale=SCALE)
                        exp_tiles[h] = et

                    def emit_attn(h, b=b, exp_tiles=exp_tiles):
                        j, half = h // 2, h % 2
                        et = exp_tiles.pop(h)
                        ph = atps.tile(
                            [128, S], F32, tag="head", name=f"hps_{l}_{b}_{h}"
                        )
                        for c, (toff, tsz) in enumerate(TCH):
                            nc.tensor.matmul(
                                ph,
                                vt_sb[:tsz, 2 * b + c, 128 * h : 128 * h + 128],
                                et[0:tsz, S * c : S * c + S],
                                start=(c == 0),
                                stop=(c == 1),
                            )
                        # denominator arrives replicated on rows 64-127: one
                        # wide bf16 reciprocal + one multiply normalizes.
                        rec = rec_pool.tile(
                            [64, S], BF16, tag="rec", name=f"rec_{l}_{b}_{h}"
                        )
                        nc.vector.reciprocal(rec, ph[64:128, :])
                        nc.vector.tensor_mul(
                            cat_sb[64 * half : 64 * half + 64, j, S * b : S * (b + 1)],
                            ph[0:64, :],
                            rec,
                        )

                    for h in range(4):
                        emit_scores(h)
                    for h in range(H):
                        if h + 4 < H:
                            emit_scores(h + 4)
                        emit_attn(h)

            # ------- Wo projection (fp8 DR) + residual --------------
            bo_sb = bias_pool.tile([128, MD], F32, tag="bo", name=f"bo_{l}")
            nc.sync.dma_start(out=bo_sb, in_=bo_d[l].rearrange("(m p) -> p m", p=128))
            xn2 = xn2_pool.tile([128, KD, T], BF16, tag="xn2", name=f"xn2_{l}")
            ln2 = LNPipe(f"ln2_{l}", x_sb, xn2, BF16)
            with tc.tile_pool(name=f"wops_{l}", bufs=4, space="PSUM") as wops:
                for m in range(MD):
                    wt = wst_pool.tile(
                        [128, KD, 128], FP8, tag="wst", name=f"wto_{l}_{m}"
                    )
                    nc.sync.dma_start(
                        out=wt, in_=Wo_d[l, m].rearrange("k p c -> p k c")
                    )
                    ps = [
                        wops.tile([128, 394], F32, tag="ps", name=f"wops_{l}_{m}_{ci}")
                        for ci in range(2)
                    ]
                    for p in range(KP):
                        ks = slice(2 * p, 2 * p + 2)
                        for ci, (off, sz) in enumerate(NCH):
                            nc.tensor.matmul(
                                ps[ci][:, :sz],
                                wt[:, ks, :],
                                cat_sb[:, ks, off : off + sz],
                                start=(p == 0),
                                stop=(p == KP - 1),
                                perf_mode=PM.DoubleRow,
                            )
                    for ci, (off, sz) in enumerate(NCH):
                        nc.vector.scalar_tensor_tensor(
                            x_sb[:, m, off : off + sz],
                            ps[ci][:, :sz],
                            bo_sb[:, m : m + 1],
                            x_sb[:, m, off : off + sz],
                            ALU.add,
                            ALU.add,
                        )
                    ln2.prep(0, m)
                    ln2.prep(1, m)
            ln2.sums(0)
            ln2.sums(1)

            # ---------------- LN2 -> xn2 (bf16) ----------------
            ln2.finish(0)
            ln2.finish(1)
            ln2 = None

            # ---------------- MLP (bf16) ----------------
            b1_sb = bias_pool.tile([128, MI], F32, tag="b1", name=f"b1_{l}")
            nc.sync.dma_start(out=b1_sb, in_=b1_d[l].rearrange("(m p) -> p m", p=128))
            b2_sb = bias_pool.tile([128, MD], F32, tag="b2", name=f"b2_{l}")
            nc.sync.dma_start(out=b2_sb, in_=b2_d[l].rearrange("(m p) -> p m", p=128))
            h_sb = h_pool.tile([128, KI, T], BF16, tag="h", name=f"h_{l}")
            with tc.tile_pool(name=f"w1ps_{l}", bufs=4, space="PSUM") as w1ps:
                for m in range(MI):
                    wt = w1st_pool.tile(
                        [128, KD, 128], BF16, tag="w1st", name=f"w1t_{l}_{m}"
                    )
                    nc.sync.dma_start(
                        out=wt, in_=W1_d[l, m].rearrange("k p c -> p k c")
                    )
                    ps = [
                        w1ps.tile([128, 394], F32, tag="ps", name=f"w1ps_{l}_{m}_{ci}")
                        for ci in range(2)
                    ]
                    for k in range(KD):
                        for ci, (off, sz) in enumerate(NCH):
                            nc.tensor.matmul(
                                ps[ci][:, :sz],
                                wt[:, k, :],
                                xn2[:, k, off : off + sz],
                                start=(k == 0),
                                stop=(k == KD - 1),
                            )
                    for ci, (off, sz) in enumerate(NCH):
                        nc.scalar.activation(
                            h_sb[:, m, off : off + sz],
                            ps[ci][:, :sz],
                            AF.Gelu,
                            bias=b1_sb[:, m : m + 1],
                        )
            xn_next = None
            ln1 = LNPipe(f"ln1n_{l}", x_sb, None, FP8)
            with tc.tile_pool(name=f"w2ps_{l}", bufs=4, space="PSUM") as w2ps:
                for m in range(MD):
                    w2t = w2st_pool.tile(
                        [128, KI, 128], BF16, tag="w2st", name=f"w2t_{l}_{m}"
                    )
                    nc.sync.dma_start(
                        out=w2t, in_=W2_d[l, m].rearrange("k p c -> p k c")
                    )
                    ps = [
                        w2ps.tile([128, 394], F32, tag="ps", name=f"w2ps_{l}_{m}_{ci}")
                        for ci in range(2)
                    ]
                    for k in range(KI):
                        for ci, (off, sz) in enumerate(NCH):
                            nc.tensor.matmul(
                                ps[ci][:, :sz],
                                w2t[:, k, :],
                                h_sb[:, k, off : off + sz],
                                start=(k == 0),
                                stop=(k == KI - 1),
                            )
                    for ci, (off, sz) in enumerate(NCH):
                        nc.vector.scalar_tensor_tensor(
                            x_sb[:, m, off : off + sz],
                            ps[ci][:, :sz],
                            b2_sb[:, m : m + 1],
                            x_sb[:, m, off : off + sz],
                            ALU.add,
                            ALU.add,
                        )
                    if l + 1 < nlayers:
                        ln1.prep(0, m)
                        ln1.prep(1, m)
            if l + 1 < nlayers:
                ln1.sums(0)
                ln1.sums(1)
            else:
                ln1 = None

        for k in range(KD):
            nc.sync.dma_start(out=out_d[128 * k : 128 * (k + 1), :], in_=x_sb[:, k, :])

    ndedup = _dedup_ldweights(nc)
    nsplit = _split_multiwaits(nc)
    print(f"dedup {ndedup} ldweights; split {nsplit} multi-wait instructions")
    return nc


def prep_weights(inputs, nlayers=L):
    """Fold gamma/beta/biases into effective weights, host side (numpy)."""
    f32 = np.float32
    fp8 = ml_dtypes.float8_e4m3fn
    bf16 = ml_dtypes.bfloat16
    Wq = np.asarray(inputs["Wq"], f32)
    bq = np.asarray(inputs["bq"], f32)
    Wk = np.asarray(inputs["Wk"], f32)
    bk = np.asarray(inputs["bk"], f32)
    Wv = np.asarray(inputs["Wv"], f32)
    bv = np.asarray(inputs["bv"], f32)
    Wo = np.asarray(inputs["Wo"], f32)
    bo = np.asarray(inputs["bo"], f32)
    W1 = np.asarray(inputs["W1"], f32)
    b1 = np.asarray(inputs["b1"], f32)
    W2 = np.asarray(inputs["W2"], f32)
    b2 = np.asarray(inputs["b2"], f32)
    g1 = np.asarray(inputs["g1"], f32)
    be1 = np.asarray(inputs["be1"], f32)
    g2 = np.asarray(inputs["g2"], f32)
    be2 = np.asarray(inputs["be2"], f32)

    Wqk = np.zeros((nlayers, D, 2 * D), f32)
    bqk = np.zeros((nlayers, 2 * D), f32)
    Wvd = np.zeros((nlayers, D, 768), f32)
    Wvaug = np.zeros((nlayers, 1, 768), f32)
    W1e = np.zeros((nlayers, D, I), f32)
    b1e = np.zeros((nlayers, I), f32)
    for l in range(nlayers):
        for h in range(H):
            Wqk[l, :, h * DH : (h + 1) * DH] = Wq[l, h] * g1[l][:, None]
            Wqk[l, :, D + h * DH : D + (h + 1) * DH] = Wk[l, h] * g1[l][:, None]
            bqk[l, h * DH : (h + 1) * DH] = bq[l, h] + Wq[l, h].T @ be1[l]
            bqk[l, D + h * DH : D + (h + 1) * DH] = bk[l, h] + Wk[l, h].T @ be1[l]
            Wvd[l, :, 64 * h : 64 * h + DH] = Wv[l, h] * g1[l][:, None]
            Wvaug[l, 0, 64 * h : 64 * h + DH] = bv[l, h] + Wv[l, h].T @ be1[l]
        W1e[l] = W1[l] * g2[l][:, None]
        b1e[l] = b1[l] + W1[l].T @ be2[l]

    Wqk8 = np.zeros((nlayers, 12, KD, 128, 128), fp8)
    Wo8 = np.zeros((nlayers, MD, KD, 128, 128), fp8)
    W1b = np.zeros((nlayers, MI, KD, 128, 128), bf16)
    W2b = np.zeros((nlayers, MD, KI, 128, 128), bf16)
    for l in range(nlayers):
        for m in range(12):
            Wqk8[l, m] = Wqk[l][:, 128 * m : 128 * (m + 1)].reshape(KD, 128, 128)
        for m in range(MD):
            Wo8[l, m] = Wo[l][:, 128 * m : 128 * (m + 1)].reshape(KD, 128, 128)
        for m in range(MI):
            W1b[l, m] = W1e[l][:, 128 * m : 128 * (m + 1)].reshape(KD, 128, 128)
        for m in range(MD):
            W2b[l, m] = W2[l][:, 128 * m : 128 * (m + 1)].reshape(KI, 128, 128)

    return {
        "Wqk": Wqk8,
        "Wo": Wo8,
        "W1": W1b,
        "W2": W2b,
        "Wv": Wvd.astype(fp8),
        "Wvaug": Wvaug.astype(bf16),
        "bqk": bqk,
        "bo": np.ascontiguousarray(bo[:nlayers]),
        "b1": b1e,
        "b2": np.ascontiguousarray(b2[:nlayers]),
    }


_cache = {}


def run_cores(inputs, nlayers=L, trace=False):
    X = np.asarray(inputs["X"], np.float32)
    wmap = prep_weights(inputs, nlayers)

    key = ("nc", nlayers)
    if key not in _cache:
        _cache[key] = build(nlayers)
    nc = _cache[key]

    in_maps = []
    for c in range(NCORES):
        xc = X[BPC * c : BPC * (c + 1)].reshape(T, D).T  # [D, T]
        m = {"xT": np.ascontiguousarray(xc)}
        m.update(wmap)
        in_maps.append(m)

    res = run_bass_kernel_spmd(nc, in_maps, core_ids=list(range(NCORES)), trace=trace)
    out = np.zeros((B, S, D), np.float32)
    for c in range(NCORES):
        out[BPC * c : BPC * (c + 1)] = res.results[c]["out"].T.reshape(BPC, S, D)
    return out, res


def kernel(**inputs):
    out, _ = run_cores(inputs)
    return out


# revision 8
# speedup vs baseline: 1.3659x; 1.1290x over previous
"""ViT-Base encoder (12 layers, B=32, S=197, D=768, H=12, I=3072) on 8 trn2
NeuronCores, data-parallel over the batch (4 images per core).

v2: the attention block (q/k/v projections, Wo) and the LayerNorm stat
reductions run as fp8e4m3 DoubleRow matmuls (2 contraction rows per PE
cell, 2x bf16 throughput); the MLP stays bf16 (fp8 there costs ~6e-2
rel err).  Softmax normalization exploits the ones-columns trick: the
attention matmul leaves the denominator replicated on PSUM rows 64-127,
so a single [64,S] bf16 reciprocal + one multiply normalizes a head
(no PE broadcast, no per-head staging copies).  Activations feeding fp8
matmuls (xn, cat) are stored fp8; the residual stream and LN stats stay
fp32.
"""

import sys

sys.path.insert(0, "/opt/trn_rl_repo")

import contextlib

import numpy as np
import ml_dtypes

import concourse.bass as bass
import concourse.mybir as mybir
import concourse.tile as tile
from concourse.vector_clock import ScopedClock
from concourse.bass_utils import run_bass_kernel_spmd

L, D, I, H, DH = 12, 768, 3072, 12, 64
B, S = 32, 197
NCORES = 8
BPC = B // NCORES  # batches per core
T = BPC * S  # 788 tokens per core
SCALE = float(1.0 / np.sqrt(DH))
EPS = 1e-5

F32 = mybir.dt.float32
BF16 = mybir.dt.bfloat16
FP8 = mybir.dt.float8e4
AF = mybir.ActivationFunctionType
ALU = mybir.AluOpType
PM = mybir.MatmulPerfMode

KD = D // 128  # 6 contraction chunks over D
KI = I // 128  # 24 contraction chunks over I
MD = D // 128  # 6 output tiles over D
MI = I // 128  # 24 output tiles over I
KP = KD // 2  # 3 fp8 DoubleRow contraction pairs over D

NCH = [(0, 394), (394, 394)]  # PSUM-half chunks for dense matmul phases
LCH = [(0, 512), (512, T - 512)]  # chunks for LN/elementwise work
TCH = [(0, 128), (128, S - 128)]  # within-batch token chunks (128+69)
VW = H * 128  # vt tile: per head [64 v-cols | 64 ones-cols]


class SplitDrainTileContext(tile.TileContext):
    """TileContext whose kernel-tail drain splits its sem waits across
    multiple SP instructions (this walrus rejects >1 wait on a Drain)."""

    def _drain_and_barrier(self, tick_clock, wait_clock):
        nc = self.nc
        drain_inst = nc.sync.drain()
        wait_clock.add_sem_waits(
            drain_inst.ins, ScopedClock({None: tick_clock.global_clock})
        )
        si = drain_inst.ins.sync_info
        waits = list(si.on_wait) if si is not None else []
        if len(waits) > 1:
            drain_inst.ins.sync_info = mybir.SyncInfo(
                on_wait=[waits[0]], on_update=list(si.on_update)
            )
            by_name = {}
            for h in self.sems.allocated().values():
                by_name[getattr(h, "name", None)] = h
            for w in waits[1:]:
                h = by_name.get(w.ant_name)
                assert h is not None, f"no handle for sem {w.ant_name}"
                nc.sync.wait_ge(h, w.wait_value)

        nc.all_engine_barrier()
        assert self.sems is not None
        popped = nc._tile_sem_poison_stack.pop()
        assert popped is self._sem_poison
        nc.clear_and_free_semaphores(list(self.sems.allocated().values()))
        nc.all_engine_barrier()


def _dedup_ldweights(nc):
    """Remove Ldweights whose weights are already resident in the PE array
    (identical signature to the previous Ldweights, nothing invalidated the
    array in between).  Carried sem waits/updates move to the next PE
    instruction; _split_multiwaits hoists any overflow afterwards."""
    removed = 0
    for fn in nc.m.functions:
        for bb in fn.blocks:
            lst = bb.instructions
            last_sig = None
            keep = []
            pending_waits = []
            pending_updates = []
            for inst in lst:
                eng = inst.engine
                if inst.opcode == "Ldweights":
                    sig = (
                        str(inst.ins[0]),
                        str(getattr(inst, "is_transpose", None)),
                        str(getattr(inst, "perf_mode", None)),
                        str(getattr(inst, "tile_position", None)),
                    )
                    if sig == last_sig:
                        si = inst.sync_info
                        if si is not None:
                            pending_waits.extend(si.on_wait)
                            pending_updates.extend(si.on_update)
                        removed += 1
                        continue
                    last_sig = sig
                elif inst.opcode == "Matmult" and str(
                    getattr(inst, "is_transpose", None)
                ) not in ("None", "False"):
                    last_sig = None  # transpose-mode clobbers the array
                if (pending_waits or pending_updates) and eng == mybir.EngineType.PE:
                    si = inst.sync_info
                    ow = list(si.on_wait) if si else []
                    ou = list(si.on_update) if si else []
                    inst.sync_info = mybir.SyncInfo(
                        on_wait=ow + pending_waits, on_update=ou + pending_updates
                    )
                    pending_waits, pending_updates = [], []
                keep.append(inst)
            assert not pending_waits and not pending_updates
            lst[:] = keep
    return removed


def _split_multiwaits(nc):
    """This walrus accepts at most 1 sem wait per instruction (2 on an
    EventSemaphore).  Tile freely packs several; hoist the excess into
    standalone EventSemaphore instructions inserted just before."""
    n = 0
    for fn in nc.m.functions:
        for bb in fn.blocks:
            lst = bb.instructions
            i = 0
            while i < len(lst):
                inst = lst[i]
                si = getattr(inst, "sync_info", None)
                if si is not None and si.on_wait:
                    cap = 2 if inst.opcode == "EventSemaphore" else 1
                    waits = list(si.on_wait)
                    if len(waits) > cap:
                        keep, extra = waits[:cap], waits[cap:]
                        new_insts = []
                        for j in range(0, len(extra), 2):
                            ev = mybir.InstEventSemaphore(
                                name=f"wsplit_{n}", ins=[], outs=[]
                            )
                            n += 1
                            ev.engine = inst.engine
                            ev.sync_info = mybir.SyncInfo(
                                on_wait=list(extra[j : j + 2]), on_update=[]
                            )
                            new_insts.append(ev)
                        inst.sync_info = mybir.SyncInfo(
                            on_wait=keep, on_update=list(si.on_update)
                        )
                        lst[i:i] = new_insts
                        i += len(new_insts)
                i += 1
    return n


def build(nlayers=L):
    nc = bass.Bass()

    xT = nc.dram_tensor("xT", [D, T], F32, kind="ExternalInput")
    Wqk_d = nc.dram_tensor("Wqk", [nlayers, 12, KD, 128, 128], FP8, kind="ExternalInput")
    Wo_d = nc.dram_tensor("Wo", [nlayers, MD, KD, 128, 128], FP8, kind="ExternalInput")
    W1_d = nc.dram_tensor("W1", [nlayers, MI, KD, 128, 128], BF16, kind="ExternalInput")
    W2_d = nc.dram_tensor("W2", [nlayers, MD, KI, 128, 128], BF16, kind="ExternalInput")
    Wv_d = nc.dram_tensor("Wv", [nlayers, D, 768], FP8, kind="ExternalInput")
    Wvaug_d = nc.dram_tensor("Wvaug", [nlayers, 1, 768], BF16, kind="ExternalInput")
    bqk_d = nc.dram_tensor("bqk", [nlayers, 2 * D], F32, kind="ExternalInput")
    bo_d = nc.dram_tensor("bo", [nlayers, D], F32, kind="ExternalInput")
    b1_d = nc.dram_tensor("b1", [nlayers, I], F32, kind="ExternalInput")
    b2_d = nc.dram_tensor("b2", [nlayers, D], F32, kind="ExternalInput")
    ident_d = nc.dram_tensor("ident", [128, 128], BF16, kind="ExternalInput")
    out_d = nc.dram_tensor("out", [D, T], F32, kind="ExternalOutput")

    with SplitDrainTileContext(nc) as tc, contextlib.ExitStack() as ctx, \
         nc.allow_low_precision(reason="fp8 attention, bf16 MLP; residual/stats fp32"):
        persist = ctx.enter_context(tc.tile_pool(name="persist", bufs=1))
        x_sb = persist.tile([128, MD, T], F32, tag="x")
        ones_row = persist.tile([1, 128], BF16, tag="ones_row")
        eps_t = persist.tile([1, 1], F32, tag="eps")
        # DoubleRow stationaries for the LN partition sums: [K=128, pair, col]
        # col 0 sums the tile, col 1 sums the squares tile.
        ones2s = persist.tile([128, 2, 64], FP8, tag="ones2s")
        ones2q = persist.tile([128, 2, 64], FP8, tag="ones2q")
        # vt tile persists so its ones-columns are memset exactly once.
        # Per head: [64 v-cols | 1 ones-col] -> token-major attention output
        # [s, 65] whose col 64 is the softmax denominator (per-partition!).
        vt_sb = persist.tile([128, 2 * BPC, H * 65], BF16, tag="vt")
        ident_b = persist.tile([128, 128], BF16, tag="ident")
        nc.sync.dma_start(out=ident_b, in_=ident_d[:, :])
        nc.vector.memset(ones_row, 1.0)
        nc.vector.memset(eps_t, EPS)
        nc.vector.memset(ones2s, 0.0)
        nc.vector.memset(ones2q, 0.0)
        nc.vector.memset(ones2s[:, :, 0:1], 1.0)
        nc.vector.memset(ones2q[:, :, 32:33], 1.0)
        for i in range(2 * BPC):
            ones_view = vt_sb[:, i, :].rearrange("p (h x) -> p h x", x=65)
            nc.gpsimd.memset(ones_view[:, :, 64:65], 1.0)

        for k in range(KD):
            nc.sync.dma_start(out=x_sb[:, k, :], in_=xT[128 * k : 128 * (k + 1), :])

        stat_pool = ctx.enter_context(tc.tile_pool(name="stats", bufs=1))
        xn8_pool = ctx.enter_context(tc.tile_pool(name="xn8", bufs=1))
        xn2_pool = ctx.enter_context(tc.tile_pool(name="xn2", bufs=1))
        qk_pool = ctx.enter_context(tc.tile_pool(name="qk", bufs=1))
        h_pool = ctx.enter_context(tc.tile_pool(name="h", bufs=1))
        bias_pool = ctx.enter_context(tc.tile_pool(name="bias", bufs=2))
        wst_pool = ctx.enter_context(tc.tile_pool(name="wst", bufs=8))
        w1st_pool = ctx.enter_context(tc.tile_pool(name="w1st", bufs=8))
        w2st_pool = ctx.enter_context(tc.tile_pool(name="w2st", bufs=4))
        wv_pool = ctx.enter_context(tc.tile_pool(name="wv", bufs=1))
        et_pool = ctx.enter_context(tc.tile_pool(name="expt", bufs=6))
        rec_pool = ctx.enter_context(tc.tile_pool(name="rec", bufs=3))
        xb_pool = ctx.enter_context(tc.tile_pool(name="xb", bufs=1))
        sq_pool = ctx.enter_context(tc.tile_pool(name="sq", bufs=1))
        lnt_pool = ctx.enter_context(tc.tile_pool(name="lnt", bufs=3))

        class LNPipe:
            """LayerNorm over features (partitions).  Stats come from an fp8
            shadow of x via DoubleRow ones-matmuls (sum into PSUM row 0,
            sum-of-squares into row 1); normalization multiplies the fp32
            residual by PE-broadcast stats."""

            def __init__(self, name, src, dst, dst_dtype):
                self.name, self.src, self.dst = name, src, dst
                self.dst_dtype = dst_dtype
                self.mu = stat_pool.tile([1, T], F32, tag="mu", name=name + "_mu")
                self.rs = stat_pool.tile([1, T], F32, tag="rs", name=name + "_rs")
                self.mu_b = stat_pool.tile([1, T], BF16, tag="mu_b", name=name + "_mub")
                self.rs_b = stat_pool.tile([1, T], BF16, tag="rs_b", name=name + "_rsb")
                self.xb = xb_pool.tile([128, KD, T], FP8, tag="xb", name=name + "_xb")
                self.sq = sq_pool.tile([128, KD, T], FP8, tag="sq", name=name + "_sq")
                self.prepped = set()

            def prep(self, ci, k):
                """fp8 shadow + squares for x[:, k, LCH[ci]] (emit as soon as
                that region is final so it overlaps the producing phase)."""
                off, sz = LCH[ci]
                cs = slice(off, off + sz)
                nc.gpsimd.tensor_copy(self.xb[:, k, cs], self.src[:, k, cs])
                nc.scalar.activation(self.sq[:, k, cs], self.xb[:, k, cs], AF.Square)
                self.prepped.add((ci, k))

            def sums(self, ci):
                off, sz = LCH[ci]
                cs = slice(off, off + sz)
                for k in range(KD):
                    if (ci, k) not in self.prepped:
                        self.prep(ci, k)
                with tc.tile_pool(
                    name=f"{self.name}_sps{ci}", bufs=1, space="PSUM"
                ) as sps:
                    sp = sps.tile([64, 512], F32, tag="sum", name=f"{self.name}_sum{ci}")
                    for p in range(KP):
                        ks = slice(2 * p, 2 * p + 2)
                        nc.tensor.matmul(
                            sp[:, :sz], ones2s, self.xb[:, ks, cs],
                            start=(p == 0), stop=False, perf_mode=PM.DoubleRow,
                            skip_group_check=True,
                        )
                        nc.tensor.matmul(
                            sp[:, :sz], ones2q, self.sq[:, ks, cs],
                            start=False, stop=(p == KP - 1), perf_mode=PM.DoubleRow,
                            skip_group_check=True,
                        )
                    nc.scalar.mul(self.mu[0:1, cs], sp[0:1, :sz], 1.0 / D)
                    # var = E[x^2] - mu^2, with E[x^2] read straight from PSUM
                    nc.vector.scalar_tensor_tensor(
                        self.rs[:, cs], self.mu[0:1, cs], -1.0, self.mu[0:1, cs],
                        ALU.mult, ALU.mult,
                    )
                    nc.vector.scalar_tensor_tensor(
                        self.rs[:, cs], sp[32:33, :sz], 1.0 / D, self.rs[:, cs],
                        ALU.mult, ALU.add,
                    )

            def finish(self, ci):
                off, sz = LCH[ci]
                cs = slice(off, off + sz)
                nc.scalar.activation(
                    self.rs[:, cs], self.rs[:, cs], AF.Sqrt, bias=eps_t, scale=1.0
                )
                nc.vector.reciprocal(self.rs[:, cs], self.rs[:, cs])
                nc.vector.tensor_copy(self.mu_b[:, cs], self.mu[0:1, cs])
                nc.vector.tensor_copy(self.rs_b[:, cs], self.rs[:, cs])
                with tc.tile_pool(
                    name=f"{self.name}_bps{ci}", bufs=1, space="PSUM"
                ) as bps:
                    bmu = bps.tile([128, 512], F32, tag="bmu", name=f"{self.name}_bmu{ci}")
                    brs = bps.tile([128, 512], F32, tag="brs", name=f"{self.name}_brs{ci}")
                    nc.tensor.matmul(bmu[:, :sz], ones_row, self.mu_b[:, cs])
                    nc.tensor.matmul(brs[:, :sz], ones_row, self.rs_b[:, cs])
                    for k in range(KD):
                        lnt = lnt_pool.tile(
                            [128, 512], F32, tag="lnt", name=f"{self.name}_lnt_{ci}_{k}"
                        )
                        nc.vector.tensor_sub(
                            lnt[:, :sz], self.src[:, k, cs], bmu[:, :sz]
                        )
                        nc.vector.tensor_mul(
                            self.dst[:, k, cs], lnt[:, :sz], brs[:, :sz]
                        )

        ln1 = ln2 = None
        for l in range(nlayers):
            wv = wv_pool.tile([128, KD, 768], FP8, tag="wv", name=f"wv_{l}")
            for k in range(KD):
                nc.sync.dma_start(
                    out=wv[:, k, :], in_=Wv_d[l, 128 * k : 128 * (k + 1), :]
                )
            wv_aug = wv_pool.tile([1, 768], BF16, tag="wv_aug", name=f"wva_{l}")
            nc.sync.dma_start(out=wv_aug, in_=Wvaug_d[l, :, :])
            bqk_sb = bias_pool.tile([128, 2 * MD], F32, tag="bqk", name=f"bqk_{l}")
            nc.sync.dma_start(out=bqk_sb, in_=bqk_d[l].rearrange("(m p) -> p m", p=128))

            # ---------------- LN1 -> xn (fp8) ----------------
            xn = xn8_pool.tile([128, KD, T], FP8, tag="xn", name=f"xn_{l}")
            if ln1 is None:  # first layer: sums not yet emitted by a W2 phase
                ln1 = LNPipe(f"ln1_{l}", x_sb, xn, FP8)
                ln1.sums(0)
                ln1.sums(1)
            ln1.dst = xn
            ln1.finish(0)
            ln1.finish(1)

            # ------------- q, k projections (fp8 DoubleRow) -------------
            qk_sb = qk_pool.tile([128, 2 * MD, T], BF16, tag="qk", name=f"qk_{l}")
            with tc.tile_pool(name=f"qkps_{l}", bufs=4, space="PSUM") as qkps:
                for m in range(2 * MD):
                    wt = wst_pool.tile(
                        [128, KD, 128], FP8, tag="wst", name=f"wt_{l}_{m}"
                    )
                    nc.sync.dma_start(
                        out=wt, in_=Wqk_d[l, m].rearrange("k p c -> p k c")
                    )
                    ps = [
                        qkps.tile([128, 394], F32, tag="ps", name=f"qkps_{l}_{m}_{ci}")
                        for ci in range(2)
                    ]
                    for p in range(KP):
                        ks = slice(2 * p, 2 * p + 2)
                        for ci, (off, sz) in enumerate(NCH):
                            nc.tensor.matmul(
                                ps[ci][:, :sz],
                                wt[:, ks, :],
                                xn[:, ks, off : off + sz],
                                start=(p == 0),
                                stop=(p == KP - 1),
                                perf_mode=PM.DoubleRow,
                            )
                    for ci, (off, sz) in enumerate(NCH):
                        nc.scalar.activation(
                            qk_sb[:, m, off : off + sz],
                            ps[ci][:, :sz],
                            AF.Identity,
                            bias=bqk_sb[:, m : m + 1],
                        )
            q_sb = qk_sb[:, 0:MD, :]
            k_sb = qk_sb[:, MD : 2 * MD, :]
            ln1 = None

            # -------- vT (fp8 DoubleRow; xn stationary, wv moving) --------
            # vt layout per head: [64 v-cols | 64 ones-cols]; the ones are
            # persistent so the attention matmul yields the numerator (rows
            # 0-63) AND the replicated softmax denominator (rows 64-127).
            with tc.tile_pool(name=f"vtps_{l}", bufs=4, space="PSUM") as vtps:
                for b in range(BPC):
                    for c, (toff, tsz) in enumerate(TCH):
                        cols = S * b + toff
                        ps = [
                            vtps.tile(
                                [128, 384], F32, tag="ps", name=f"vtps_{l}_{b}_{c}_{n}"
                            )
                            for n in range(2)
                        ]
                        for k in range(KD):
                            for n in range(2):
                                nc.tensor.matmul(
                                    ps[n][:tsz, :],
                                    xn[:, k, cols : cols + tsz],
                                    wv[:, k, 384 * n : 384 * (n + 1)],
                                    start=(k == 0),
                                    stop=False,
                                    skip_group_check=True,
                                )
                        for n in range(2):
                            nc.tensor.matmul(
                                ps[n][:tsz, :],
                                ones_row[:, :tsz],
                                wv_aug[:, 384 * n : 384 * (n + 1)],
                                start=False,
                                stop=True,
                                skip_group_check=True,
                            )
                        dstv = vt_sb[:tsz, 2 * b + c, :].rearrange(
                            "p (h x) -> p h x", x=65
                        )
                        for n in range(2):
                            nc.vector.tensor_copy(
                                dstv[:, 6 * n : 6 * n + 6, 0:64],
                                ps[n][:tsz, :].rearrange("p (h x) -> p h x", x=64),
                            )

            # ---------------- attention (token-major) ----------------
            # attn output per (batch, s-chunk): [s, 12*65] split across two
            # PSUM banks of 6 heads; col 64 of each head-block is the softmax
            # denominator, landing on the token partition so one strided
            # reciprocal + per-partition tensor_scalar normalizes 6 heads.
            # The bf16 normalized tile is transposed back to feature-major
            # fp8 via PE identity-transposes.
            cat_sb = xn8_pool.tile([128, KD, T], FP8, tag="cat", name=f"cat_{l}")
            SCH = [(0, 128), (128, S - 128)]  # s-chunks within a batch
            with tc.tile_pool(name=f"scps_{l}", bufs=2, space="PSUM") as scps, \
                 tc.tile_pool(name=f"tmps_{l}", bufs=4, space="PSUM") as tmps, \
                 tc.tile_pool(name=f"tpps_{l}", bufs=2, space="PSUM") as tpps, \
                 tc.tile_pool(name=f"ctm_{l}", bufs=4) as ctm_pool, \
                 tc.tile_pool(name=f"rcp_{l}", bufs=4) as rcp_pool:

                def emit_norm(b, tm_tiles, cat_tm):
                    for sg, (soff, ssz) in enumerate(SCH):
                        for g in range(2):
                            tmt = tm_tiles[(sg, g)]
                            rcp = rcp_pool.tile(
                                [128, 6], F32, tag="rcp", name=f"rcp_{l}_{b}_{sg}_{g}"
                            )
                            den = tmt[0:ssz, :].rearrange(
                                "p (h x) -> p h x", x=65
                            )[:, :, 64]
                            nc.vector.tensor_copy(rcp[0:ssz, :], den)
                            nc.vector.reciprocal(rcp[0:ssz, :], rcp[0:ssz, :])
                            for j in range(6):
                                nc.vector.tensor_scalar(
                                    out=cat_tm[sg][0:ssz, 64 * (6 * g + j) : 64 * (6 * g + j) + 64],
                                    in0=tmt[0:ssz, 65 * j : 65 * j + 64],
                                    scalar1=rcp[0:ssz, j : j + 1],
                                    scalar2=None,
                                    op0=ALU.mult,
                                )

                def emit_transpose(b, cat_tm):
                    for sg, (soff, ssz) in enumerate(SCH):
                        for f in range(MD):
                            tp = tpps.tile(
                                [128, 128], BF16, tag="tp", name=f"tp_{l}_{b}_{sg}_{f}"
                            )
                            nc.tensor.matmul(
                                tp[:, 0:ssz],
                                cat_tm[sg][0:ssz, 128 * f : 128 * f + 128],
                                ident_b[0:ssz, 0:ssz],
                                is_transpose=True,
                            )
                            dst = cat_sb[:, f, S * b + soff : S * b + soff + ssz]
                            if f % 2 == 0:
                                nc.vector.tensor_copy(dst, tp[:, 0:ssz])
                            else:
                                nc.scalar.copy(dst, tp[:, 0:ssz])

                prev = None
                for b in range(BPC):
                    exp_tiles = {}
                    tm_tiles = {}
                    cat_tm = [
                        ctm_pool.tile(
                            [128, 768], BF16, tag="ctm", name=f"ctm_{l}_{b}_{sg}"
                        )
                        for sg in range(2)
                    ]

                    def emit_scores(h, b=b, exp_tiles=exp_tiles):
                        j, half = h // 2, h % 2
                        rows = slice(64 * half, 64 * half + 64)
                        sps_t = scps.tile(
                            [128, 2 * S], F32, tag="ps", name=f"sc_{l}_{b}_{h}"
                        )
                        for c, (toff, tsz) in enumerate(TCH):
                            cols = S * b + toff
                            nc.tensor.matmul(
                                sps_t[:tsz, S * c : S * c + S],
                                k_sb[rows, j, cols : cols + tsz],
                                q_sb[rows, j, S * b : S * (b + 1)],
                                start=(c == 0),
                                stop=True,
                                skip_group_check=True,
                            )
                        et = et_pool.tile(
                            [128, 2 * S], BF16, tag="expT", name=f"et_{l}_{b}_{h}"
                        )
                        nc.scalar.activation(et, sps_t, AF.Exp, scale=SCALE)
                        exp_tiles[h] = et

                    def emit_attn(h, b=b, exp_tiles=exp_tiles, tm_tiles=tm_tiles):
                        g, j = h // 6, h % 6
                        et = exp_tiles.pop(h)
                        for sg, (soff, ssz) in enumerate(SCH):
                            if (sg, g) not in tm_tiles:
                                tm_tiles[(sg, g)] = tmps.tile(
                                    [128, 390], F32, tag="tm",
                                    name=f"tm_{l}_{b}_{sg}_{g}",
                                )
                            tmt = tm_tiles[(sg, g)]
                            for c, (toff, tsz) in enumerate(TCH):
                                nc.tensor.matmul(
                                    tmt[0:ssz, 65 * j : 65 * j + 65],
                                    et[0:tsz, S * c + soff : S * c + soff + ssz],
                                    vt_sb[0:tsz, 2 * b + c, 65 * h : 65 * h + 65],
                                    start=(j == 0 and c == 0),
                                    stop=(c == 1),
                                    skip_group_check=True,
                                )

                    for h in range(2):
                        emit_scores(h)
                    for h in range(H):
                        if h + 2 < H:
                            emit_scores(h + 2)
                        emit_attn(h)
                    emit_norm(b, tm_tiles, cat_tm)
                    if prev is not None:
                        emit_transpose(*prev)
                    prev = (b, cat_tm)
                emit_transpose(*prev)

            # ------- Wo projection (fp8 DR) + residual --------------
            bo_sb = bias_pool.tile([128, MD], F32, tag="bo", name=f"bo_{l}")
            nc.sync.dma_start(out=bo_sb, in_=bo_d[l].rearrange("(m p) -> p m", p=128))
            xn2 = xn2_pool.tile([128, KD, T], BF16, tag="xn2", name=f"xn2_{l}")
            ln2 = LNPipe(f"ln2_{l}", x_sb, xn2, BF16)
            with tc.tile_pool(name=f"wops_{l}", bufs=4, space="PSUM") as wops:
                for m in range(MD):
                    wt = wst_pool.tile(
                        [128, KD, 128], FP8, tag="wst", name=f"wto_{l}_{m}"
                    )
                    nc.sync.dma_start(
                        out=wt, in_=Wo_d[l, m].rearrange("k p c -> p k c")
                    )
                    ps = [
                        wops.tile([128, 394], F32, tag="ps", name=f"wops_{l}_{m}_{ci}")
                        for ci in range(2)
                    ]
                    for p in range(KP):
                        ks = slice(2 * p, 2 * p + 2)
                        for ci, (off, sz) in enumerate(NCH):
                            nc.tensor.matmul(
                                ps[ci][:, :sz],
                                wt[:, ks, :],
                                cat_sb[:, ks, off : off + sz],
                                start=(p == 0),
                                stop=(p == KP - 1),
                                perf_mode=PM.DoubleRow,
                            )
                    for ci, (off, sz) in enumerate(NCH):
                        nc.vector.scalar_tensor_tensor(
                            x_sb[:, m, off : off + sz],
                            ps[ci][:, :sz],
                            bo_sb[:, m : m + 1],
                            x_sb[:, m, off : off + sz],
                            ALU.add,
                            ALU.add,
                        )
                    ln2.prep(0, m)
                    ln2.prep(1, m)
            ln2.sums(0)
            ln2.sums(1)

            # ---------------- LN2 -> xn2 (bf16) ----------------
            ln2.finish(0)
            ln2.finish(1)
            ln2 = None

            # ---------------- MLP (bf16) ----------------
            b1_sb = bias_pool.tile([128, MI], F32, tag="b1", name=f"b1_{l}")
            nc.sync.dma_start(out=b1_sb, in_=b1_d[l].rearrange("(m p) -> p m", p=128))
            b2_sb = bias_pool.tile([128, MD], F32, tag="b2", name=f"b2_{l}")
            nc.sync.dma_start(out=b2_sb, in_=b2_d[l].rearrange("(m p) -> p m", p=128))
            h_sb = h_pool.tile([128, KI, T], BF16, tag="h", name=f"h_{l}")
            with tc.tile_pool(name=f"w1ps_{l}", bufs=4, space="PSUM") as w1ps:
                for m in range(MI):
                    wt = w1st_pool.tile(
                        [128, KD, 128], BF16, tag="w1st", name=f"w1t_{l}_{m}"
                    )
                    nc.sync.dma_start(
                        out=wt, in_=W1_d[l, m].rearrange("k p c -> p k c")
                    )
                    ps = [
                        w1ps.tile([128, 394], F32, tag="ps", name=f"w1ps_{l}_{m}_{ci}")
                        for ci in range(2)
                    ]
                    for k in range(KD):
                        for ci, (off, sz) in enumerate(NCH):
                            nc.tensor.matmul(
                                ps[ci][:, :sz],
                                wt[:, k, :],
                                xn2[:, k, off : off + sz],
                                start=(k == 0),
                                stop=(k == KD - 1),
                            )
                    for ci, (off, sz) in enumerate(NCH):
                        nc.scalar.activation(
                            h_sb[:, m, off : off + sz],
                            ps[ci][:, :sz],
                            AF.Gelu,
                            bias=b1_sb[:, m : m + 1],
                        )
            xn_next = None
            ln1 = LNPipe(f"ln1n_{l}", x_sb, None, FP8)
            with tc.tile_pool(name=f"w2ps_{l}", bufs=4, space="PSUM") as w2ps:
                for m in range(MD):
                    w2t = w2st_pool.tile(
                        [128, KI, 128], BF16, tag="w2st", name=f"w2t_{l}_{m}"
                    )
                    nc.sync.dma_start(
                        out=w2t, in_=W2_d[l, m].rearrange("k p c -> p k c")
                    )
                    ps = [
                        w2ps.tile([128, 394], F32, tag="ps", name=f"w2ps_{l}_{m}_{ci}")
                        for ci in range(2)
                    ]
                    for k in range(KI):
                        for ci, (off, sz) in enumerate(NCH):
                            nc.tensor.matmul(
                                ps[ci][:, :sz],
                                w2t[:, k, :],
                                h_sb[:, k, off : off + sz],
                                start=(k == 0),
                                stop=(k == KI - 1),
                            )
                    for ci, (off, sz) in enumerate(NCH):
                        nc.vector.scalar_tensor_tensor(
                            x_sb[:, m, off : off + sz],
                            ps[ci][:, :sz],
                            b2_sb[:, m : m + 1],
                            x_sb[:, m, off : off + sz],
                            ALU.add,
                            ALU.add,
                        )
                    if l + 1 < nlayers:
                        ln1.prep(0, m)
                        ln1.prep(1, m)
            if l + 1 < nlayers:
                ln1.sums(0)
                ln1.sums(1)
            else:
                ln1 = None

        for k in range(KD):
            nc.sync.dma_start(out=out_d[128 * k : 128 * (k + 1), :], in_=x_sb[:, k, :])

    ndedup = _dedup_ldweights(nc)
    nsplit = _split_multiwaits(nc)
    print(f"dedup {ndedup} ldweights; split {nsplit} multi-wait instructions")
    return nc


def prep_weights(inputs, nlayers=L):
    """Fold gamma/beta/biases into effective weights, host side (numpy)."""
    f32 = np.float32
    fp8 = ml_dtypes.float8_e4m3fn
    bf16 = ml_dtypes.bfloat16
    Wq = np.asarray(inputs["Wq"], f32)
    bq = np.asarray(inputs["bq"], f32)
    Wk = np.asarray(inputs["Wk"], f32)
    bk = np.asarray(inputs["bk"], f32)
    Wv = np.asarray(inputs["Wv"], f32)
    bv = np.asarray(inputs["bv"], f32)
    Wo = np.asarray(inputs["Wo"], f32)
    bo = np.asarray(inputs["bo"], f32)
    W1 = np.asarray(inputs["W1"], f32)
    b1 = np.asarray(inputs["b1"], f32)
    W2 = np.asarray(inputs["W2"], f32)
    b2 = np.asarray(inputs["b2"], f32)
    g1 = np.asarray(inputs["g1"], f32)
    be1 = np.asarray(inputs["be1"], f32)
    g2 = np.asarray(inputs["g2"], f32)
    be2 = np.asarray(inputs["be2"], f32)

    Wqk = np.zeros((nlayers, D, 2 * D), f32)
    bqk = np.zeros((nlayers, 2 * D), f32)
    Wvd = np.zeros((nlayers, D, 768), f32)
    Wvaug = np.zeros((nlayers, 1, 768), f32)
    W1e = np.zeros((nlayers, D, I), f32)
    b1e = np.zeros((nlayers, I), f32)
    for l in range(nlayers):
        for h in range(H):
            Wqk[l, :, h * DH : (h + 1) * DH] = Wq[l, h] * g1[l][:, None]
            Wqk[l, :, D + h * DH : D + (h + 1) * DH] = Wk[l, h] * g1[l][:, None]
            bqk[l, h * DH : (h + 1) * DH] = bq[l, h] + Wq[l, h].T @ be1[l]
            bqk[l, D + h * DH : D + (h + 1) * DH] = bk[l, h] + Wk[l, h].T @ be1[l]
            Wvd[l, :, 64 * h : 64 * h + DH] = Wv[l, h] * g1[l][:, None]
            Wvaug[l, 0, 64 * h : 64 * h + DH] = bv[l, h] + Wv[l, h].T @ be1[l]
        W1e[l] = W1[l] * g2[l][:, None]
        b1e[l] = b1[l] + W1[l].T @ be2[l]

    Wqk8 = np.zeros((nlayers, 12, KD, 128, 128), fp8)
    Wo8 = np.zeros((nlayers, MD, KD, 128, 128), fp8)
    W1b = np.zeros((nlayers, MI, KD, 128, 128), bf16)
    W2b = np.zeros((nlayers, MD, KI, 128, 128), bf16)
    for l in range(nlayers):
        for m in range(12):
            Wqk8[l, m] = Wqk[l][:, 128 * m : 128 * (m + 1)].reshape(KD, 128, 128)
        for m in range(MD):
            Wo8[l, m] = Wo[l][:, 128 * m : 128 * (m + 1)].reshape(KD, 128, 128)
        for m in range(MI):
            W1b[l, m] = W1e[l][:, 128 * m : 128 * (m + 1)].reshape(KD, 128, 128)
        for m in range(MD):
            W2b[l, m] = W2[l][:, 128 * m : 128 * (m + 1)].reshape(KI, 128, 128)

    return {
        "ident": np.eye(128, dtype=bf16),
        "Wqk": Wqk8,
        "Wo": Wo8,
        "W1": W1b,
        "W2": W2b,
        "Wv": Wvd.astype(fp8),
        "Wvaug": Wvaug.astype(bf16),
        "bqk": bqk,
        "bo": np.ascontiguousarray(bo[:nlayers]),
        "b1": b1e,
        "b2": np.ascontiguousarray(b2[:nlayers]),
    }


_cache = {}


def run_cores(inputs, nlayers=L, trace=False):
    X = np.asarray(inputs["X"], np.float32)
    wmap = prep_weights(inputs, nlayers)

    key = ("nc", nlayers)
    if key not in _cache:
        _cache[key] = build(nlayers)
    nc = _cache[key]

    in_maps = []
    for c in range(NCORES):
        xc = X[BPC * c : BPC * (c + 1)].reshape(T, D).T  # [D, T]
        m = {"xT": np.ascontiguousarray(xc)}
        m.update(wmap)
        in_maps.append(m)

    res = run_bass_kernel_spmd(nc, in_maps, core_ids=list(range(NCORES)), trace=trace)
    out = np.zeros((B, S, D), np.float32)
    for c in range(NCORES):
        out[BPC * c : BPC * (c + 1)] = res.results[c]["out"].T.reshape(BPC, S, D)
    return out, res


def kernel(**inputs):
    out, _ = run_cores(inputs)
    return out


# revision 10
# speedup vs baseline: 1.3833x; 1.0128x over previous
"""ViT-Base encoder (12 layers, B=32, S=197, D=768, H=12, I=3072) on 8 trn2
NeuronCores, data-parallel over the batch (4 images per core).

v2: the attention block (q/k/v projections, Wo) and the LayerNorm stat
reductions run as fp8e4m3 DoubleRow matmuls (2 contraction rows per PE
cell, 2x bf16 throughput); the MLP stays bf16 (fp8 there costs ~6e-2
rel err).  Softmax normalization exploits the ones-columns trick: the
attention matmul leaves the denominator replicated on PSUM rows 64-127,
so a single [64,S] bf16 reciprocal + one multiply normalizes a head
(no PE broadcast, no per-head staging copies).  Activations feeding fp8
matmuls (xn, cat) are stored fp8; the residual stream and LN stats stay
fp32.
"""

import sys

sys.path.insert(0, "/opt/trn_rl_repo")

import contextlib

import numpy as np
import ml_dtypes

import concourse.bass as bass
import concourse.mybir as mybir
import concourse.tile as tile
from concourse.vector_clock import ScopedClock
from concourse.bass_utils import run_bass_kernel_spmd

L, D, I, H, DH = 12, 768, 3072, 12, 64
B, S = 32, 197
NCORES = 8
BPC = B // NCORES  # batches per core
T = BPC * S  # 788 tokens per core
SCALE = float(1.0 / np.sqrt(DH))
EPS = 1e-5

F32 = mybir.dt.float32
BF16 = mybir.dt.bfloat16
FP8 = mybir.dt.float8e4
AF = mybir.ActivationFunctionType
ALU = mybir.AluOpType
PM = mybir.MatmulPerfMode

KD = D // 128  # 6 contraction chunks over D
KI = I // 128  # 24 contraction chunks over I
MD = D // 128  # 6 output tiles over D
MI = I // 128  # 24 output tiles over I
KP = KD // 2  # 3 fp8 DoubleRow contraction pairs over D

NCH = [(0, 394), (394, 394)]  # PSUM-half chunks for dense matmul phases
LCH = [(0, 394), (394, 394)]  # chunks for LN/elementwise work (aligned to NCH)
TCH = [(0, 128), (128, S - 128)]  # within-batch token chunks (128+69)
VW = H * 128  # vt tile: per head [64 v-cols | 64 ones-cols]


class SplitDrainTileContext(tile.TileContext):
    """TileContext whose kernel-tail drain splits its sem waits across
    multiple SP instructions (this walrus rejects >1 wait on a Drain)."""

    def _drain_and_barrier(self, tick_clock, wait_clock):
        nc = self.nc
        drain_inst = nc.sync.drain()
        wait_clock.add_sem_waits(
            drain_inst.ins, ScopedClock({None: tick_clock.global_clock})
        )
        si = drain_inst.ins.sync_info
        waits = list(si.on_wait) if si is not None else []
        if len(waits) > 1:
            drain_inst.ins.sync_info = mybir.SyncInfo(
                on_wait=[waits[0]], on_update=list(si.on_update)
            )
            by_name = {}
            for h in self.sems.allocated().values():
                by_name[getattr(h, "name", None)] = h
            for w in waits[1:]:
                h = by_name.get(w.ant_name)
                assert h is not None, f"no handle for sem {w.ant_name}"
                nc.sync.wait_ge(h, w.wait_value)

        nc.all_engine_barrier()
        assert self.sems is not None
        popped = nc._tile_sem_poison_stack.pop()
        assert popped is self._sem_poison
        nc.clear_and_free_semaphores(list(self.sems.allocated().values()))
        nc.all_engine_barrier()


def _dedup_ldweights(nc):
    """Remove Ldweights whose weights are already resident in the PE array
    (identical signature to the previous Ldweights, nothing invalidated the
    array in between).  Carried sem waits/updates move to the next PE
    instruction; _split_multiwaits hoists any overflow afterwards."""
    removed = 0
    for fn in nc.m.functions:
        for bb in fn.blocks:
            lst = bb.instructions
            last_sig = None
            keep = []
            pending_waits = []
            pending_updates = []
            for inst in lst:
                eng = inst.engine
                if inst.opcode == "Ldweights":
                    sig = (
                        str(inst.ins[0]),
                        str(getattr(inst, "is_transpose", None)),
                        str(getattr(inst, "perf_mode", None)),
                        str(getattr(inst, "tile_position", None)),
                    )
                    if sig == last_sig:
                        si = inst.sync_info
                        if si is not None:
                            pending_waits.extend(si.on_wait)
                            pending_updates.extend(si.on_update)
                        removed += 1
                        continue
                    last_sig = sig
                elif inst.opcode == "Matmult" and str(
                    getattr(inst, "is_transpose", None)
                ) not in ("None", "False"):
                    last_sig = None  # transpose-mode clobbers the array
                if (pending_waits or pending_updates) and eng == mybir.EngineType.PE:
                    si = inst.sync_info
                    ow = list(si.on_wait) if si else []
                    ou = list(si.on_update) if si else []
                    inst.sync_info = mybir.SyncInfo(
                        on_wait=ow + pending_waits, on_update=ou + pending_updates
                    )
                    pending_waits, pending_updates = [], []
                keep.append(inst)
            assert not pending_waits and not pending_updates
            lst[:] = keep
    return removed


def _split_multiwaits(nc):
    """This walrus accepts at most 1 sem wait per instruction (2 on an
    EventSemaphore).  Tile freely packs several; hoist the excess into
    standalone EventSemaphore instructions inserted just before."""
    n = 0
    for fn in nc.m.functions:
        for bb in fn.blocks:
            lst = bb.instructions
            i = 0
            while i < len(lst):
                inst = lst[i]
                si = getattr(inst, "sync_info", None)
                if si is not None and si.on_wait:
                    cap = 2 if inst.opcode == "EventSemaphore" else 1
                    waits = list(si.on_wait)
                    if len(waits) > cap:
                        keep, extra = waits[:cap], waits[cap:]
                        new_insts = []
                        for j in range(0, len(extra), 2):
                            ev = mybir.InstEventSemaphore(
                                name=f"wsplit_{n}", ins=[], outs=[]
                            )
                            n += 1
                            ev.engine = inst.engine
                            ev.sync_info = mybir.SyncInfo(
                                on_wait=list(extra[j : j + 2]), on_update=[]
                            )
                            new_insts.append(ev)
                        inst.sync_info = mybir.SyncInfo(
                            on_wait=keep, on_update=list(si.on_update)
                        )
                        lst[i:i] = new_insts
                        i += len(new_insts)
                i += 1
    return n


def build(nlayers=L):
    nc = bass.Bass()

    xT = nc.dram_tensor("xT", [D, T], F32, kind="ExternalInput")
    Wqk_d = nc.dram_tensor("Wqk", [nlayers, 12, KD, 128, 128], FP8, kind="ExternalInput")
    Wo_d = nc.dram_tensor("Wo", [nlayers, MD, KD, 128, 128], FP8, kind="ExternalInput")
    W1_d = nc.dram_tensor("W1", [nlayers, MI, KD, 128, 128], BF16, kind="ExternalInput")
    W2_d = nc.dram_tensor("W2", [nlayers, MD, KI, 128, 128], BF16, kind="ExternalInput")
    Wv_d = nc.dram_tensor("Wv", [nlayers, D, 768], FP8, kind="ExternalInput")
    Wvaug_d = nc.dram_tensor("Wvaug", [nlayers, 1, 768], BF16, kind="ExternalInput")
    bqk_d = nc.dram_tensor("bqk", [nlayers, 2 * D], F32, kind="ExternalInput")
    bo_d = nc.dram_tensor("bo", [nlayers, D], F32, kind="ExternalInput")
    b1_d = nc.dram_tensor("b1", [nlayers, I], F32, kind="ExternalInput")
    b2_d = nc.dram_tensor("b2", [nlayers, D], F32, kind="ExternalInput")
    ident_d = nc.dram_tensor("ident", [128, 128], BF16, kind="ExternalInput")
    out_d = nc.dram_tensor("out", [D, T], F32, kind="ExternalOutput")

    with SplitDrainTileContext(nc) as tc, contextlib.ExitStack() as ctx, \
         nc.allow_low_precision(reason="fp8 attention, bf16 MLP; residual/stats fp32"):
        persist = ctx.enter_context(tc.tile_pool(name="persist", bufs=1))
        x_sb = persist.tile([128, MD, T], F32, tag="x")
        ones_row = persist.tile([1, 128], BF16, tag="ones_row")
        eps_t = persist.tile([1, 1], F32, tag="eps")
        # DoubleRow stationaries for the LN partition sums: [K=128, pair, col]
        # col 0 sums the tile, col 1 sums the squares tile.
        ones2s = persist.tile([128, 2, 64], FP8, tag="ones2s")
        ones2q = persist.tile([128, 2, 64], FP8, tag="ones2q")
        # vt tile persists so its ones-columns are memset exactly once.
        # Per head: [64 v-cols | 1 ones-col] -> token-major attention output
        # [s, 65] whose col 64 is the softmax denominator (per-partition!).
        vt_sb = persist.tile([128, 2 * BPC, H * 65], BF16, tag="vt")
        ident_b = persist.tile([128, 128], BF16, tag="ident")
        nc.sync.dma_start(out=ident_b, in_=ident_d[:, :])
        nc.vector.memset(ones_row, 1.0)
        nc.vector.memset(eps_t, EPS)
        nc.vector.memset(ones2s, 0.0)
        nc.vector.memset(ones2q, 0.0)
        nc.vector.memset(ones2s[:, :, 0:1], 1.0)
        nc.vector.memset(ones2q[:, :, 32:33], 1.0)
        for i in range(2 * BPC):
            ones_view = vt_sb[:, i, :].rearrange("p (h x) -> p h x", x=65)
            nc.gpsimd.memset(ones_view[:, :, 64:65], 1.0)

        for k in range(KD):
            nc.sync.dma_start(out=x_sb[:, k, :], in_=xT[128 * k : 128 * (k + 1), :])

        stat_pool = ctx.enter_context(tc.tile_pool(name="stats", bufs=1))
        xn8_pool = ctx.enter_context(tc.tile_pool(name="xn8", bufs=1))
        xn2_pool = ctx.enter_context(tc.tile_pool(name="xn2", bufs=1))
        qk_pool = ctx.enter_context(tc.tile_pool(name="qk", bufs=1))
        h_pool = ctx.enter_context(tc.tile_pool(name="h", bufs=1))
        bias_pool = ctx.enter_context(tc.tile_pool(name="bias", bufs=2))
        wst_pool = ctx.enter_context(tc.tile_pool(name="wst", bufs=8))
        w1st_pool = ctx.enter_context(tc.tile_pool(name="w1st", bufs=8))
        w2st_pool = ctx.enter_context(tc.tile_pool(name="w2st", bufs=4))
        wv_pool = ctx.enter_context(tc.tile_pool(name="wv", bufs=1))
        et_pool = ctx.enter_context(tc.tile_pool(name="expt", bufs=6))
        rec_pool = ctx.enter_context(tc.tile_pool(name="rec", bufs=3))
        xb_pool = ctx.enter_context(tc.tile_pool(name="xb", bufs=1))
        sq_pool = ctx.enter_context(tc.tile_pool(name="sq", bufs=1))
        lnt_pool = ctx.enter_context(tc.tile_pool(name="lnt", bufs=3))

        class LNPipe:
            """LayerNorm over features (partitions).  Stats come from an fp8
            shadow of x via DoubleRow ones-matmuls (sum into PSUM row 0,
            sum-of-squares into row 1); normalization multiplies the fp32
            residual by PE-broadcast stats."""

            def __init__(self, name, src, dst, dst_dtype):
                self.name, self.src, self.dst = name, src, dst
                self.dst_dtype = dst_dtype
                self.mu = stat_pool.tile([1, T], F32, tag="mu", name=name + "_mu")
                self.rs = stat_pool.tile([1, T], F32, tag="rs", name=name + "_rs")
                self.mu_b = stat_pool.tile([1, T], BF16, tag="mu_b", name=name + "_mub")
                self.rs_b = stat_pool.tile([1, T], BF16, tag="rs_b", name=name + "_rsb")
                self.xb = xb_pool.tile([128, KD, T], FP8, tag="xb", name=name + "_xb")
                self.sq = sq_pool.tile([128, KD, T], FP8, tag="sq", name=name + "_sq")
                self.prepped = set()

            def prep(self, ci, k):
                """fp8 shadow + squares for x[:, k, LCH[ci]] (emit as soon as
                that region is final so it overlaps the producing phase)."""
                off, sz = LCH[ci]
                cs = slice(off, off + sz)
                nc.gpsimd.tensor_copy(self.xb[:, k, cs], self.src[:, k, cs])
                nc.scalar.activation(self.sq[:, k, cs], self.xb[:, k, cs], AF.Square)
                self.prepped.add((ci, k))

            def sums(self, ci):
                off, sz = LCH[ci]
                cs = slice(off, off + sz)
                for k in range(KD):
                    if (ci, k) not in self.prepped:
                        self.prep(ci, k)
                with tc.tile_pool(
                    name=f"{self.name}_sps{ci}", bufs=1, space="PSUM"
                ) as sps:
                    sp = sps.tile([64, 394], F32, tag="sum", name=f"{self.name}_sum{ci}")
                    for p in range(KP):
                        ks = slice(2 * p, 2 * p + 2)
                        nc.tensor.matmul(
                            sp[:, :sz], ones2s, self.xb[:, ks, cs],
                            start=(p == 0), stop=False, perf_mode=PM.DoubleRow,
                            skip_group_check=True,
                        )
                        nc.tensor.matmul(
                            sp[:, :sz], ones2q, self.sq[:, ks, cs],
                            start=False, stop=(p == KP - 1), perf_mode=PM.DoubleRow,
                            skip_group_check=True,
                        )
                    nc.scalar.mul(self.mu[0:1, cs], sp[0:1, :sz], 1.0 / D)
                    # var = E[x^2] - mu^2, with E[x^2] read straight from PSUM
                    nc.vector.scalar_tensor_tensor(
                        self.rs[:, cs], self.mu[0:1, cs], -1.0, self.mu[0:1, cs],
                        ALU.mult, ALU.mult,
                    )
                    nc.vector.scalar_tensor_tensor(
                        self.rs[:, cs], sp[32:33, :sz], 1.0 / D, self.rs[:, cs],
                        ALU.mult, ALU.add,
                    )

            def finish_stats(self, ci):
                off, sz = LCH[ci]
                cs = slice(off, off + sz)
                nc.scalar.activation(
                    self.rs[:, cs], self.rs[:, cs], AF.Sqrt, bias=eps_t, scale=1.0
                )
                nc.vector.reciprocal(self.rs[:, cs], self.rs[:, cs])
                nc.vector.tensor_copy(self.mu_b[:, cs], self.mu[0:1, cs])
                nc.vector.tensor_copy(self.rs_b[:, cs], self.rs[:, cs])

            def finish_apply(self, ci):
                off, sz = LCH[ci]
                cs = slice(off, off + sz)
                with tc.tile_pool(
                    name=f"{self.name}_bps{ci}", bufs=1, space="PSUM"
                ) as bps:
                    bmu = bps.tile([128, 394], F32, tag="bmu", name=f"{self.name}_bmu{ci}")
                    brs = bps.tile([128, 394], F32, tag="brs", name=f"{self.name}_brs{ci}")
                    nc.tensor.matmul(bmu[:, :sz], ones_row, self.mu_b[:, cs])
                    nc.tensor.matmul(brs[:, :sz], ones_row, self.rs_b[:, cs])
                    for k in range(KD):
                        lnt = lnt_pool.tile(
                            [128, 394], F32, tag="lnt", name=f"{self.name}_lnt_{ci}_{k}"
                        )
                        nc.vector.tensor_sub(
                            lnt[:, :sz], self.src[:, k, cs], bmu[:, :sz]
                        )
                        nc.vector.tensor_mul(
                            self.dst[:, k, cs], lnt[:, :sz], brs[:, :sz]
                        )

        ln1 = ln2 = None
        for l in range(nlayers):
            wv = wv_pool.tile([128, KD, 768], FP8, tag="wv", name=f"wv_{l}")
            for k in range(KD):
                nc.sync.dma_start(
                    out=wv[:, k, :], in_=Wv_d[l, 128 * k : 128 * (k + 1), :]
                )
            wv_aug = wv_pool.tile([1, 768], BF16, tag="wv_aug", name=f"wva_{l}")
            nc.sync.dma_start(out=wv_aug, in_=Wvaug_d[l, :, :])
            bqk_sb = bias_pool.tile([128, 2 * MD], F32, tag="bqk", name=f"bqk_{l}")
            nc.sync.dma_start(out=bqk_sb, in_=bqk_d[l].rearrange("(m p) -> p m", p=128))

            # ---------------- LN1 -> xn (fp8) ----------------
            xn = xn8_pool.tile([128, KD, T], FP8, tag="xn", name=f"xn_{l}")
            if ln1 is None:  # first layer: sums/stats not yet emitted by W2
                ln1 = LNPipe(f"ln1_{l}", x_sb, xn, FP8)
                ln1.sums(0)
                ln1.sums(1)
                ln1.finish_stats(0)
                ln1.finish_stats(1)
            ln1.dst = xn
            ln1.finish_apply(0)
            ln1.finish_apply(1)

            # ------------- q, k projections (fp8 DoubleRow) -------------
            qk_sb = qk_pool.tile([128, 2 * MD, T], BF16, tag="qk", name=f"qk_{l}")
            with tc.tile_pool(name=f"qkps_{l}", bufs=4, space="PSUM") as qkps:
                for m in range(2 * MD):
                    wt = wst_pool.tile(
                        [128, KD, 128], FP8, tag="wst", name=f"wt_{l}_{m}"
                    )
                    nc.sync.dma_start(
                        out=wt, in_=Wqk_d[l, m].rearrange("k p c -> p k c")
                    )
                    ps = [
                        qkps.tile([128, 394], F32, tag="ps", name=f"qkps_{l}_{m}_{ci}")
                        for ci in range(2)
                    ]
                    for p in range(KP):
                        ks = slice(2 * p, 2 * p + 2)
                        for ci, (off, sz) in enumerate(NCH):
                            nc.tensor.matmul(
                                ps[ci][:, :sz],
                                wt[:, ks, :],
                                xn[:, ks, off : off + sz],
                                start=(p == 0),
                                stop=(p == KP - 1),
                                perf_mode=PM.DoubleRow,
                            )
                    for ci, (off, sz) in enumerate(NCH):
                        nc.scalar.activation(
                            qk_sb[:, m, off : off + sz],
                            ps[ci][:, :sz],
                            AF.Identity,
                            bias=bqk_sb[:, m : m + 1],
                        )
            q_sb = qk_sb[:, 0:MD, :]
            k_sb = qk_sb[:, MD : 2 * MD, :]
            ln1 = None

            # -------- vT (fp8 DoubleRow; xn stationary, wv moving) --------
            # vt layout per head: [64 v-cols | 64 ones-cols]; the ones are
            # persistent so the attention matmul yields the numerator (rows
            # 0-63) AND the replicated softmax denominator (rows 64-127).
            with tc.tile_pool(name=f"vtps_{l}", bufs=4, space="PSUM") as vtps:
                for b in range(BPC):
                    for c, (toff, tsz) in enumerate(TCH):
                        cols = S * b + toff
                        ps = [
                            vtps.tile(
                                [128, 384], F32, tag="ps", name=f"vtps_{l}_{b}_{c}_{n}"
                            )
                            for n in range(2)
                        ]
                        for k in range(KD):
                            for n in range(2):
                                nc.tensor.matmul(
                                    ps[n][:tsz, :],
                                    xn[:, k, cols : cols + tsz],
                                    wv[:, k, 384 * n : 384 * (n + 1)],
                                    start=(k == 0),
                                    stop=False,
                                    skip_group_check=True,
                                )
                        for n in range(2):
                            nc.tensor.matmul(
                                ps[n][:tsz, :],
                                ones_row[:, :tsz],
                                wv_aug[:, 384 * n : 384 * (n + 1)],
                                start=False,
                                stop=True,
                                skip_group_check=True,
                            )
                        dstv = vt_sb[:tsz, 2 * b + c, :].rearrange(
                            "p (h x) -> p h x", x=65
                        )
                        for n in range(2):
                            nc.vector.tensor_copy(
                                dstv[:, 6 * n : 6 * n + 6, 0:64],
                                ps[n][:tsz, :].rearrange("p (h x) -> p h x", x=64),
                            )

            # ---------------- attention (token-major) ----------------
            # attn output per (batch, s-chunk): [s, 12*65] split across two
            # PSUM banks of 6 heads; col 64 of each head-block is the softmax
            # denominator, landing on the token partition so one strided
            # reciprocal + per-partition tensor_scalar normalizes 6 heads.
            # The bf16 normalized tile is transposed back to feature-major
            # fp8 via PE identity-transposes.
            cat_sb = xn8_pool.tile([128, KD, T], FP8, tag="cat", name=f"cat_{l}")
            SCH = [(0, 128), (128, S - 128)]  # s-chunks within a batch
            with tc.tile_pool(name=f"scps_{l}", bufs=2, space="PSUM") as scps, \
                 tc.tile_pool(name=f"tmps_{l}", bufs=4, space="PSUM") as tmps, \
                 tc.tile_pool(name=f"tpps_{l}", bufs=2, space="PSUM") as tpps, \
                 tc.tile_pool(name=f"ctm_{l}", bufs=4) as ctm_pool, \
                 tc.tile_pool(name=f"rcp_{l}", bufs=4) as rcp_pool:

                def emit_norm(b, tm_tiles, cat_tm):
                    for sg, (soff, ssz) in enumerate(SCH):
                        for g in range(2):
                            tmt = tm_tiles[(sg, g)]
                            rcp = rcp_pool.tile(
                                [128, 6], F32, tag="rcp", name=f"rcp_{l}_{b}_{sg}_{g}"
                            )
                            den = tmt[0:ssz, :].rearrange(
                                "p (h x) -> p h x", x=65
                            )[:, :, 64]
                            nc.vector.tensor_copy(rcp[0:ssz, :], den)
                            nc.vector.reciprocal(rcp[0:ssz, :], rcp[0:ssz, :])
                            for j in range(6):
                                nc.vector.tensor_scalar(
                                    out=cat_tm[sg][0:ssz, 64 * (6 * g + j) : 64 * (6 * g + j) + 64],
                                    in0=tmt[0:ssz, 65 * j : 65 * j + 64],
                                    scalar1=rcp[0:ssz, j : j + 1],
                                    scalar2=None,
                                    op0=ALU.mult,
                                )

                def emit_transpose(b, cat_tm):
                    for sg, (soff, ssz) in enumerate(SCH):
                        for f in range(MD):
                            tp = tpps.tile(
                                [128, 128], BF16, tag="tp", name=f"tp_{l}_{b}_{sg}_{f}"
                            )
                            nc.tensor.matmul(
                                tp[:, 0:ssz],
                                cat_tm[sg][0:ssz, 128 * f : 128 * f + 128],
                                ident_b[0:ssz, 0:ssz],
                                is_transpose=True,
                            )
                            dst = cat_sb[:, f, S * b + soff : S * b + soff + ssz]
                            if f % 2 == 0:
                                nc.vector.tensor_copy(dst, tp[:, 0:ssz])
                            else:
                                nc.scalar.copy(dst, tp[:, 0:ssz])

                prev = None
                for b in range(BPC):
                    exp_tiles = {}
                    tm_tiles = {}
                    cat_tm = [
                        ctm_pool.tile(
                            [128, 768], BF16, tag="ctm", name=f"ctm_{l}_{b}_{sg}"
                        )
                        for sg in range(2)
                    ]

                    def emit_scores(h, b=b, exp_tiles=exp_tiles):
                        j, half = h // 2, h % 2
                        rows = slice(64 * half, 64 * half + 64)
                        sps_t = scps.tile(
                            [128, 2 * S], F32, tag="ps", name=f"sc_{l}_{b}_{h}"
                        )
                        for c, (toff, tsz) in enumerate(TCH):
                            cols = S * b + toff
                            nc.tensor.matmul(
                                sps_t[:tsz, S * c : S * c + S],
                                k_sb[rows, j, cols : cols + tsz],
                                q_sb[rows, j, S * b : S * (b + 1)],
                                start=(c == 0),
                                stop=True,
                                skip_group_check=True,
                            )
                        et = et_pool.tile(
                            [128, 2 * S], BF16, tag="expT", name=f"et_{l}_{b}_{h}"
                        )
                        nc.scalar.activation(et, sps_t, AF.Exp, scale=SCALE)
                        exp_tiles[h] = et

                    def emit_attn(h, b=b, exp_tiles=exp_tiles, tm_tiles=tm_tiles):
                        g, j = h // 6, h % 6
                        et = exp_tiles.pop(h)
                        for sg, (soff, ssz) in enumerate(SCH):
                            if (sg, g) not in tm_tiles:
                                tm_tiles[(sg, g)] = tmps.tile(
                                    [128, 390], F32, tag="tm",
                                    name=f"tm_{l}_{b}_{sg}_{g}",
                                )
                            tmt = tm_tiles[(sg, g)]
                            for c, (toff, tsz) in enumerate(TCH):
                                nc.tensor.matmul(
                                    tmt[0:ssz, 65 * j : 65 * j + 65],
                                    et[0:tsz, S * c + soff : S * c + soff + ssz],
                                    vt_sb[0:tsz, 2 * b + c, 65 * h : 65 * h + 65],
                                    start=(j == 0 and c == 0),
                                    stop=(c == 1),
                                    skip_group_check=True,
                                )

                    for h in range(2):
                        emit_scores(h)
                    for h in range(H):
                        if h + 2 < H:
                            emit_scores(h + 2)
                        emit_attn(h)
                    emit_norm(b, tm_tiles, cat_tm)
                    if prev is not None:
                        emit_transpose(*prev)
                    prev = (b, cat_tm)
                emit_transpose(*prev)

            # ------- Wo projection (fp8 DR) + residual --------------
            bo_sb = bias_pool.tile([128, MD], F32, tag="bo", name=f"bo_{l}")
            nc.sync.dma_start(out=bo_sb, in_=bo_d[l].rearrange("(m p) -> p m", p=128))
            xn2 = xn2_pool.tile([128, KD, T], BF16, tag="xn2", name=f"xn2_{l}")
            ln2 = LNPipe(f"ln2_{l}", x_sb, xn2, BF16)
            with tc.tile_pool(name=f"wops_{l}", bufs=4, space="PSUM") as wops:
                for m in range(MD):
                    wt = wst_pool.tile(
                        [128, KD, 128], FP8, tag="wst", name=f"wto_{l}_{m}"
                    )
                    nc.sync.dma_start(
                        out=wt, in_=Wo_d[l, m].rearrange("k p c -> p k c")
                    )
                    ps = [
                        wops.tile([128, 394], F32, tag="ps", name=f"wops_{l}_{m}_{ci}")
                        for ci in range(2)
                    ]
                    for p in range(KP):
                        ks = slice(2 * p, 2 * p + 2)
                        for ci, (off, sz) in enumerate(NCH):
                            nc.tensor.matmul(
                                ps[ci][:, :sz],
                                wt[:, ks, :],
                                cat_sb[:, ks, off : off + sz],
                                start=(p == 0),
                                stop=(p == KP - 1),
                                perf_mode=PM.DoubleRow,
                            )
                    for ci, (off, sz) in enumerate(NCH):
                        nc.vector.scalar_tensor_tensor(
                            x_sb[:, m, off : off + sz],
                            ps[ci][:, :sz],
                            bo_sb[:, m : m + 1],
                            x_sb[:, m, off : off + sz],
                            ALU.add,
                            ALU.add,
                        )
                    ln2.prep(0, m)
                    ln2.prep(1, m)
            ln2.sums(0)
            ln2.sums(1)

            # ---------------- LN2 -> xn2 (bf16) ----------------
            ln2.finish_stats(0)
            ln2.finish_stats(1)
            ln2.finish_apply(0)
            ln2.finish_apply(1)
            ln2 = None

            # ---------------- MLP (bf16) ----------------
            b1_sb = bias_pool.tile([128, MI], F32, tag="b1", name=f"b1_{l}")
            nc.sync.dma_start(out=b1_sb, in_=b1_d[l].rearrange("(m p) -> p m", p=128))
            b2_sb = bias_pool.tile([128, MD], F32, tag="b2", name=f"b2_{l}")
            nc.sync.dma_start(out=b2_sb, in_=b2_d[l].rearrange("(m p) -> p m", p=128))
            h_sb = h_pool.tile([128, KI, T], BF16, tag="h", name=f"h_{l}")
            with tc.tile_pool(name=f"w1ps_{l}", bufs=4, space="PSUM") as w1ps:
                for m in range(MI):
                    wt = w1st_pool.tile(
                        [128, KD, 128], BF16, tag="w1st", name=f"w1t_{l}_{m}"
                    )
                    nc.sync.dma_start(
                        out=wt, in_=W1_d[l, m].rearrange("k p c -> p k c")
                    )
                    ps = [
                        w1ps.tile([128, 394], F32, tag="ps", name=f"w1ps_{l}_{m}_{ci}")
                        for ci in range(2)
                    ]
                    for k in range(KD):
                        for ci, (off, sz) in enumerate(NCH):
                            nc.tensor.matmul(
                                ps[ci][:, :sz],
                                wt[:, k, :],
                                xn2[:, k, off : off + sz],
                                start=(k == 0),
                                stop=(k == KD - 1),
                            )
                    for ci, (off, sz) in enumerate(NCH):
                        nc.scalar.activation(
                            h_sb[:, m, off : off + sz],
                            ps[ci][:, :sz],
                            AF.Gelu,
                            bias=b1_sb[:, m : m + 1],
                        )
            xn_next = None
            ln1 = LNPipe(f"ln1n_{l}", x_sb, None, FP8)
            with tc.tile_pool(name=f"w2ps_{l}", bufs=4, space="PSUM") as w2ps:
                for m in range(MD):
                    w2t = w2st_pool.tile(
                        [128, KI, 128], BF16, tag="w2st", name=f"w2t_{l}_{m}"
                    )
                    nc.sync.dma_start(
                        out=w2t, in_=W2_d[l, m].rearrange("k p c -> p k c")
                    )
                    ps = [
                        w2ps.tile([128, 394], F32, tag="ps", name=f"w2ps_{l}_{m}_{ci}")
                        for ci in range(2)
                    ]
                    for k in range(KI):
                        for ci, (off, sz) in enumerate(NCH):
                            nc.tensor.matmul(
                                ps[ci][:, :sz],
                                w2t[:, k, :],
                                h_sb[:, k, off : off + sz],
                                start=(k == 0),
                                stop=(k == KI - 1),
                            )
                    for ci, (off, sz) in enumerate(NCH):
                        nc.vector.scalar_tensor_tensor(
                            x_sb[:, m, off : off + sz],
                            ps[ci][:, :sz],
                            b2_sb[:, m : m + 1],
                            x_sb[:, m, off : off + sz],
                            ALU.add,
                            ALU.add,
                        )
                    if l + 1 < nlayers:
                        ln1.prep(0, m)
                        ln1.prep(1, m)
            if l + 1 < nlayers:
                ln1.sums(0)
                ln1.sums(1)
                ln1.finish_stats(0)
                ln1.finish_stats(1)
            else:
                ln1 = None

        for k in range(KD):
            nc.sync.dma_start(out=out_d[128 * k : 128 * (k + 1), :], in_=x_sb[:, k, :])

    ndedup = _dedup_ldweights(nc)
    nsplit = _split_multiwaits(nc)
    print(f"dedup {ndedup} ldweights; split {nsplit} multi-wait instructions")
    return nc


def prep_weights(inputs, nlayers=L):
    """Fold gamma/beta/biases into effective weights, host side (numpy)."""
    f32 = np.float32
    fp8 = ml_dtypes.float8_e4m3fn
    bf16 = ml_dtypes.bfloat16
    Wq = np.asarray(inputs["Wq"], f32)
    bq = np.asarray(inputs["bq"], f32)
    Wk = np.asarray(inputs["Wk"], f32)
    bk = np.asarray(inputs["bk"], f32)
    Wv = np.asarray(inputs["Wv"], f32)
    bv = np.asarray(inputs["bv"], f32)
    Wo = np.asarray(inputs["Wo"], f32)
    bo = np.asarray(inputs["bo"], f32)
    W1 = np.asarray(inputs["W1"], f32)
    b1 = np.asarray(inputs["b1"], f32)
    W2 = np.asarray(inputs["W2"], f32)
    b2 = np.asarray(inputs["b2"], f32)
    g1 = np.asarray(inputs["g1"], f32)
    be1 = np.asarray(inputs["be1"], f32)
    g2 = np.asarray(inputs["g2"], f32)
    be2 = np.asarray(inputs["be2"], f32)

    Wqk = np.zeros((nlayers, D, 2 * D), f32)
    bqk = np.zeros((nlayers, 2 * D), f32)
    Wvd = np.zeros((nlayers, D, 768), f32)
    Wvaug = np.zeros((nlayers, 1, 768), f32)
    W1e = np.zeros((nlayers, D, I), f32)
    b1e = np.zeros((nlayers, I), f32)
    for l in range(nlayers):
        for h in range(H):
            Wqk[l, :, h * DH : (h + 1) * DH] = Wq[l, h] * g1[l][:, None]
            Wqk[l, :, D + h * DH : D + (h + 1) * DH] = Wk[l, h] * g1[l][:, None]
            bqk[l, h * DH : (h + 1) * DH] = bq[l, h] + Wq[l, h].T @ be1[l]
            bqk[l, D + h * DH : D + (h + 1) * DH] = bk[l, h] + Wk[l, h].T @ be1[l]
            Wvd[l, :, 64 * h : 64 * h + DH] = Wv[l, h] * g1[l][:, None]
            Wvaug[l, 0, 64 * h : 64 * h + DH] = bv[l, h] + Wv[l, h].T @ be1[l]
        W1e[l] = W1[l] * g2[l][:, None]
        b1e[l] = b1[l] + W1[l].T @ be2[l]

    Wqk8 = np.zeros((nlayers, 12, KD, 128, 128), fp8)
    Wo8 = np.zeros((nlayers, MD, KD, 128, 128), fp8)
    W1b = np.zeros((nlayers, MI, KD, 128, 128), bf16)
    W2b = np.zeros((nlayers, MD, KI, 128, 128), bf16)
    for l in range(nlayers):
        for m in range(12):
            Wqk8[l, m] = Wqk[l][:, 128 * m : 128 * (m + 1)].reshape(KD, 128, 128)
        for m in range(MD):
            Wo8[l, m] = Wo[l][:, 128 * m : 128 * (m + 1)].reshape(KD, 128, 128)
        for m in range(MI):
            W1b[l, m] = W1e[l][:, 128 * m : 128 * (m + 1)].reshape(KD, 128, 128)
        for m in range(MD):
            W2b[l, m] = W2[l][:, 128 * m : 128 * (m + 1)].reshape(KI, 128, 128)

    return {
        "ident": np.eye(128, dtype=bf16),
        "Wqk": Wqk8,
        "Wo": Wo8,
        "W1": W1b,
        "W2": W2b,
        "Wv": Wvd.astype(fp8),
        "Wvaug": Wvaug.astype(bf16),
        "bqk": bqk,
        "bo": np.ascontiguousarray(bo[:nlayers]),
        "b1": b1e,
        "b2": np.ascontiguousarray(b2[:nlayers]),
    }


_cache = {}


def run_cores(inputs, nlayers=L, trace=False):
    X = np.asarray(inputs["X"], np.float32)
    wmap = prep_weights(inputs, nlayers)

    key = ("nc", nlayers)
    if key not in _cache:
        _cache[key] = build(nlayers)
    nc = _cache[key]

    in_maps = []
    for c in range(NCORES):
        xc = X[BPC * c : BPC * (c + 1)].reshape(T, D).T  # [D, T]
        m = {"xT": np.ascontiguousarray(xc)}
        m.update(wmap)
        in_maps.append(m)

    res = run_bass_kernel_spmd(nc, in_maps, core_ids=list(range(NCORES)), trace=trace)
    out = np.zeros((B, S, D), np.float32)
    for c in range(NCORES):
        out[BPC * c : BPC * (c + 1)] = res.results[c]["out"].T.reshape(BPC, S, D)
    return out, res


def kernel(**inputs):
    out, _ = run_cores(inputs)
    return out


# revision 11
# speedup vs baseline: 1.4201x; 1.0266x over previous
"""ViT-Base encoder (12 layers, B=32, S=197, D=768, H=12, I=3072) on 8 trn2
NeuronCores, data-parallel over the batch (4 images per core).

v2: the attention block (q/k/v projections, Wo) and the LayerNorm stat
reductions run as fp8e4m3 DoubleRow matmuls (2 contraction rows per PE
cell, 2x bf16 throughput); the MLP stays bf16 (fp8 there costs ~6e-2
rel err).  Softmax normalization exploits the ones-columns trick: the
attention matmul leaves the denominator replicated on PSUM rows 64-127,
so a single [64,S] bf16 reciprocal + one multiply normalizes a head
(no PE broadcast, no per-head staging copies).  Activations feeding fp8
matmuls (xn, cat) are stored fp8; the residual stream and LN stats stay
fp32.
"""

import sys

sys.path.insert(0, "/opt/trn_rl_repo")

import contextlib

import numpy as np
import ml_dtypes

import concourse.bass as bass
import concourse.mybir as mybir
import concourse.tile as tile
from concourse.vector_clock import ScopedClock
from concourse.bass_utils import run_bass_kernel_spmd

L, D, I, H, DH = 12, 768, 3072, 12, 64
B, S = 32, 197
NCORES = 8
BPC = B // NCORES  # batches per core
T = BPC * S  # 788 tokens per core
SCALE = float(1.0 / np.sqrt(DH))
EPS = 1e-5

F32 = mybir.dt.float32
BF16 = mybir.dt.bfloat16
FP8 = mybir.dt.float8e4
AF = mybir.ActivationFunctionType
ALU = mybir.AluOpType
PM = mybir.MatmulPerfMode

KD = D // 128  # 6 contraction chunks over D
KI = I // 128  # 24 contraction chunks over I
MD = D // 128  # 6 output tiles over D
MI = I // 128  # 24 output tiles over I
KP = KD // 2  # 3 fp8 DoubleRow contraction pairs over D

NCH = [(0, 394), (394, 394)]  # PSUM-half chunks for dense matmul phases
LCH = [(0, 394), (394, 394)]  # chunks for LN/elementwise work (aligned to NCH)
TCH = [(0, 128), (128, S - 128)]  # within-batch token chunks (128+69)
VW = H * 128  # vt tile: per head [64 v-cols | 64 ones-cols]


class SplitDrainTileContext(tile.TileContext):
    """TileContext whose kernel-tail drain splits its sem waits across
    multiple SP instructions (this walrus rejects >1 wait on a Drain)."""

    def _drain_and_barrier(self, tick_clock, wait_clock):
        nc = self.nc
        drain_inst = nc.sync.drain()
        wait_clock.add_sem_waits(
            drain_inst.ins, ScopedClock({None: tick_clock.global_clock})
        )
        si = drain_inst.ins.sync_info
        waits = list(si.on_wait) if si is not None else []
        if len(waits) > 1:
            drain_inst.ins.sync_info = mybir.SyncInfo(
                on_wait=[waits[0]], on_update=list(si.on_update)
            )
            by_name = {}
            for h in self.sems.allocated().values():
                by_name[getattr(h, "name", None)] = h
            for w in waits[1:]:
                h = by_name.get(w.ant_name)
                assert h is not None, f"no handle for sem {w.ant_name}"
                nc.sync.wait_ge(h, w.wait_value)

        nc.all_engine_barrier()
        assert self.sems is not None
        popped = nc._tile_sem_poison_stack.pop()
        assert popped is self._sem_poison
        nc.clear_and_free_semaphores(list(self.sems.allocated().values()))
        nc.all_engine_barrier()


def _dedup_ldweights(nc):
    """Remove Ldweights whose weights are already resident in the PE array
    (identical signature to the previous Ldweights, nothing invalidated the
    array in between).  Carried sem waits/updates move to the next PE
    instruction; _split_multiwaits hoists any overflow afterwards."""
    removed = 0
    for fn in nc.m.functions:
        for bb in fn.blocks:
            lst = bb.instructions
            last_sig = None
            keep = []
            pending_waits = []
            pending_updates = []
            for inst in lst:
                eng = inst.engine
                if inst.opcode == "Ldweights":
                    sig = (
                        str(inst.ins[0]),
                        str(getattr(inst, "is_transpose", None)),
                        str(getattr(inst, "perf_mode", None)),
                        str(getattr(inst, "tile_position", None)),
                    )
                    if sig == last_sig:
                        si = inst.sync_info
                        if si is not None:
                            pending_waits.extend(si.on_wait)
                            pending_updates.extend(si.on_update)
                        removed += 1
                        continue
                    last_sig = sig
                elif inst.opcode == "Matmult" and str(
                    getattr(inst, "is_transpose", None)
                ) not in ("None", "False"):
                    last_sig = None  # transpose-mode clobbers the array
                if (pending_waits or pending_updates) and eng == mybir.EngineType.PE:
                    si = inst.sync_info
                    ow = list(si.on_wait) if si else []
                    ou = list(si.on_update) if si else []
                    inst.sync_info = mybir.SyncInfo(
                        on_wait=ow + pending_waits, on_update=ou + pending_updates
                    )
                    pending_waits, pending_updates = [], []
                keep.append(inst)
            assert not pending_waits and not pending_updates
            lst[:] = keep
    return removed


def _split_multiwaits(nc):
    """This walrus accepts at most 1 sem wait per instruction (2 on an
    EventSemaphore).  Tile freely packs several; hoist the excess into
    standalone EventSemaphore instructions inserted just before."""
    n = 0
    for fn in nc.m.functions:
        for bb in fn.blocks:
            lst = bb.instructions
            i = 0
            while i < len(lst):
                inst = lst[i]
                si = getattr(inst, "sync_info", None)
                if si is not None and si.on_wait:
                    cap = 2 if inst.opcode == "EventSemaphore" else 1
                    waits = list(si.on_wait)
                    if len(waits) > cap:
                        keep, extra = waits[:cap], waits[cap:]
                        new_insts = []
                        for j in range(0, len(extra), 2):
                            ev = mybir.InstEventSemaphore(
                                name=f"wsplit_{n}", ins=[], outs=[]
                            )
                            n += 1
                            ev.engine = inst.engine
                            ev.sync_info = mybir.SyncInfo(
                                on_wait=list(extra[j : j + 2]), on_update=[]
                            )
                            new_insts.append(ev)
                        inst.sync_info = mybir.SyncInfo(
                            on_wait=keep, on_update=list(si.on_update)
                        )
                        lst[i:i] = new_insts
                        i += len(new_insts)
                i += 1
    return n


def build(nlayers=L):
    nc = bass.Bass()

    xT = nc.dram_tensor("xT", [D, T], F32, kind="ExternalInput")
    Wqk_d = nc.dram_tensor("Wqk", [nlayers, 12, KD, 128, 128], FP8, kind="ExternalInput")
    Wo_d = nc.dram_tensor("Wo", [nlayers, MD, KD, 128, 128], FP8, kind="ExternalInput")
    W1_d = nc.dram_tensor("W1", [nlayers, MI, KD, 128, 128], BF16, kind="ExternalInput")
    W2_d = nc.dram_tensor("W2", [nlayers, MD, KI, 128, 128], BF16, kind="ExternalInput")
    Wv_d = nc.dram_tensor("Wv", [nlayers, D, 768], FP8, kind="ExternalInput")
    Wvaug_d = nc.dram_tensor("Wvaug", [nlayers, 1, 768], BF16, kind="ExternalInput")
    bqk_d = nc.dram_tensor("bqk", [nlayers, 2 * D], F32, kind="ExternalInput")
    bo_d = nc.dram_tensor("bo", [nlayers, D], F32, kind="ExternalInput")
    b1_d = nc.dram_tensor("b1", [nlayers, I], F32, kind="ExternalInput")
    b2_d = nc.dram_tensor("b2", [nlayers, D], F32, kind="ExternalInput")
    ident_d = nc.dram_tensor("ident", [128, 128], BF16, kind="ExternalInput")
    out_d = nc.dram_tensor("out", [D, T], F32, kind="ExternalOutput")

    with SplitDrainTileContext(nc) as tc, contextlib.ExitStack() as ctx, \
         nc.allow_low_precision(reason="fp8 attention, bf16 MLP; residual/stats fp32"):
        persist = ctx.enter_context(tc.tile_pool(name="persist", bufs=1))
        x_sb = persist.tile([128, MD, T], F32, tag="x")
        ones_row = persist.tile([1, 128], BF16, tag="ones_row")
        eps_t = persist.tile([1, 1], F32, tag="eps")
        # DoubleRow stationaries for the LN partition sums: [K=128, pair, col]
        # col 0 sums the tile, col 1 sums the squares tile.
        ones2s = persist.tile([128, 2, 64], FP8, tag="ones2s")
        ones2q = persist.tile([128, 2, 64], FP8, tag="ones2q")
        # vt tile persists so its ones-columns are memset exactly once.
        # Per head: [64 v-cols | 1 ones-col] -> token-major attention output
        # [s, 65] whose col 64 is the softmax denominator (per-partition!).
        vt_sb = persist.tile([128, 2 * BPC, H * 65], BF16, tag="vt")
        ident_b = persist.tile([128, 128], BF16, tag="ident")
        nc.sync.dma_start(out=ident_b, in_=ident_d[:, :])
        nc.vector.memset(ones_row, 1.0)
        nc.vector.memset(eps_t, EPS)
        nc.vector.memset(ones2s, 0.0)
        nc.vector.memset(ones2q, 0.0)
        nc.vector.memset(ones2s[:, :, 0:1], 1.0)
        nc.vector.memset(ones2q[:, :, 32:33], 1.0)
        for i in range(2 * BPC):
            ones_view = vt_sb[:, i, :].rearrange("p (h x) -> p h x", x=65)
            nc.gpsimd.memset(ones_view[:, :, 64:65], 1.0)

        for k in range(KD):
            nc.sync.dma_start(out=x_sb[:, k, :], in_=xT[128 * k : 128 * (k + 1), :])

        rsqrt_fixups = []
        stat_pool = ctx.enter_context(tc.tile_pool(name="stats", bufs=1))
        xn8_pool = ctx.enter_context(tc.tile_pool(name="xn8", bufs=1))
        xn2_pool = ctx.enter_context(tc.tile_pool(name="xn2", bufs=1))
        qk_pool = ctx.enter_context(tc.tile_pool(name="qk", bufs=1))
        h_pool = ctx.enter_context(tc.tile_pool(name="h", bufs=1))
        bias_pool = ctx.enter_context(tc.tile_pool(name="bias", bufs=2))
        wst_pool = ctx.enter_context(tc.tile_pool(name="wst", bufs=8))
        w1st_pool = ctx.enter_context(tc.tile_pool(name="w1st", bufs=8))
        w2st_pool = ctx.enter_context(tc.tile_pool(name="w2st", bufs=4))
        wv_pool = ctx.enter_context(tc.tile_pool(name="wv", bufs=1))
        et_pool = ctx.enter_context(tc.tile_pool(name="expt", bufs=6))
        rec_pool = ctx.enter_context(tc.tile_pool(name="rec", bufs=3))
        xb_pool = ctx.enter_context(tc.tile_pool(name="xb", bufs=1))
        sq_pool = ctx.enter_context(tc.tile_pool(name="sq", bufs=1))
        lnt_pool = ctx.enter_context(tc.tile_pool(name="lnt", bufs=3))

        class LNPipe:
            """LayerNorm over features (partitions).  Stats come from an fp8
            shadow of x via DoubleRow ones-matmuls (sum into PSUM row 0,
            sum-of-squares into row 1); normalization multiplies the fp32
            residual by PE-broadcast stats."""

            def __init__(self, name, src, dst, dst_dtype):
                self.name, self.src, self.dst = name, src, dst
                self.dst_dtype = dst_dtype
                self.mu = stat_pool.tile([1, T], F32, tag="mu", name=name + "_mu")
                self.rs = stat_pool.tile([1, T], F32, tag="rs", name=name + "_rs")
                self.mu_b = stat_pool.tile([1, T], BF16, tag="mu_b", name=name + "_mub")
                self.rs_b = stat_pool.tile([1, T], BF16, tag="rs_b", name=name + "_rsb")
                self.xb = xb_pool.tile([128, KD, T], FP8, tag="xb", name=name + "_xb")
                self.sq = sq_pool.tile([128, KD, T], FP8, tag="sq", name=name + "_sq")
                self.prepped = set()

            def prep(self, ci, k):
                """fp8 shadow + squares for x[:, k, LCH[ci]] (emit as soon as
                that region is final so it overlaps the producing phase)."""
                off, sz = LCH[ci]
                cs = slice(off, off + sz)
                nc.gpsimd.tensor_copy(self.xb[:, k, cs], self.src[:, k, cs])
                nc.scalar.activation(self.sq[:, k, cs], self.xb[:, k, cs], AF.Square)
                self.prepped.add((ci, k))

            def sums(self, ci):
                off, sz = LCH[ci]
                cs = slice(off, off + sz)
                for k in range(KD):
                    if (ci, k) not in self.prepped:
                        self.prep(ci, k)
                with tc.tile_pool(
                    name=f"{self.name}_sps{ci}", bufs=1, space="PSUM"
                ) as sps:
                    sp = sps.tile([64, 394], F32, tag="sum", name=f"{self.name}_sum{ci}")
                    for p in range(KP):
                        ks = slice(2 * p, 2 * p + 2)
                        nc.tensor.matmul(
                            sp[:, :sz], ones2s, self.xb[:, ks, cs],
                            start=(p == 0), stop=False, perf_mode=PM.DoubleRow,
                            skip_group_check=True,
                        )
                        nc.tensor.matmul(
                            sp[:, :sz], ones2q, self.sq[:, ks, cs],
                            start=False, stop=(p == KP - 1), perf_mode=PM.DoubleRow,
                            skip_group_check=True,
                        )
                    nc.scalar.mul(self.mu[0:1, cs], sp[0:1, :sz], 1.0 / D)
                    # var = E[x^2] - mu^2, with E[x^2] read straight from PSUM
                    nc.vector.scalar_tensor_tensor(
                        self.rs[:, cs], self.mu[0:1, cs], -1.0, self.mu[0:1, cs],
                        ALU.mult, ALU.mult,
                    )
                    nc.vector.scalar_tensor_tensor(
                        self.rs[:, cs], sp[32:33, :sz], 1.0 / D, self.rs[:, cs],
                        ALU.mult, ALU.add,
                    )

            def finish_stats(self, ci):
                off, sz = LCH[ci]
                cs = slice(off, off + sz)
                h = nc.scalar.activation(
                    self.rs_b[:, cs], self.rs[:, cs], AF.Sqrt, bias=eps_t, scale=1.0
                )
                rsqrt_fixups.append(h.ins)
                nc.scalar.copy(self.mu_b[:, cs], self.mu[0:1, cs])

            def finish_apply(self, ci):
                off, sz = LCH[ci]
                cs = slice(off, off + sz)
                with tc.tile_pool(
                    name=f"{self.name}_bps{ci}", bufs=1, space="PSUM"
                ) as bps:
                    bmu = bps.tile([128, 394], F32, tag="bmu", name=f"{self.name}_bmu{ci}")
                    brs = bps.tile([128, 394], F32, tag="brs", name=f"{self.name}_brs{ci}")
                    nc.tensor.matmul(bmu[:, :sz], ones_row, self.mu_b[:, cs])
                    nc.tensor.matmul(brs[:, :sz], ones_row, self.rs_b[:, cs])
                    for k in range(KD):
                        lnt = lnt_pool.tile(
                            [128, 394], F32, tag="lnt", name=f"{self.name}_lnt_{ci}_{k}"
                        )
                        nc.vector.tensor_sub(
                            lnt[:, :sz], self.src[:, k, cs], bmu[:, :sz]
                        )
                        nc.vector.tensor_mul(
                            self.dst[:, k, cs], lnt[:, :sz], brs[:, :sz]
                        )

        ln1 = ln2 = None
        for l in range(nlayers):
            wv = wv_pool.tile([128, KD, 768], FP8, tag="wv", name=f"wv_{l}")
            for k in range(KD):
                nc.sync.dma_start(
                    out=wv[:, k, :], in_=Wv_d[l, 128 * k : 128 * (k + 1), :]
                )
            wv_aug = wv_pool.tile([1, 768], BF16, tag="wv_aug", name=f"wva_{l}")
            nc.sync.dma_start(out=wv_aug, in_=Wvaug_d[l, :, :])
            bqk_sb = bias_pool.tile([128, 2 * MD], F32, tag="bqk", name=f"bqk_{l}")
            nc.sync.dma_start(out=bqk_sb, in_=bqk_d[l].rearrange("(m p) -> p m", p=128))

            # ---------------- LN1 -> xn (fp8) ----------------
            xn = xn8_pool.tile([128, KD, T], FP8, tag="xn", name=f"xn_{l}")
            if ln1 is None:  # first layer: sums/stats not yet emitted by W2
                ln1 = LNPipe(f"ln1_{l}", x_sb, xn, FP8)
                ln1.sums(0)
                ln1.sums(1)
                ln1.finish_stats(0)
                ln1.finish_stats(1)
            ln1.dst = xn
            ln1.finish_apply(0)
            ln1.finish_apply(1)

            # ------------- q, k projections (fp8 DoubleRow) -------------
            qk_sb = qk_pool.tile([128, 2 * MD, T], BF16, tag="qk", name=f"qk_{l}")
            with tc.tile_pool(name=f"qkps_{l}", bufs=4, space="PSUM") as qkps:
                for m in range(2 * MD):
                    wt = wst_pool.tile(
                        [128, KD, 128], FP8, tag="wst", name=f"wt_{l}_{m}"
                    )
                    nc.sync.dma_start(
                        out=wt, in_=Wqk_d[l, m].rearrange("k p c -> p k c")
                    )
                    ps = [
                        qkps.tile([128, 394], F32, tag="ps", name=f"qkps_{l}_{m}_{ci}")
                        for ci in range(2)
                    ]
                    for p in range(KP):
                        ks = slice(2 * p, 2 * p + 2)
                        for ci, (off, sz) in enumerate(NCH):
                            nc.tensor.matmul(
                                ps[ci][:, :sz],
                                wt[:, ks, :],
                                xn[:, ks, off : off + sz],
                                start=(p == 0),
                                stop=(p == KP - 1),
                                perf_mode=PM.DoubleRow,
                            )
                    for ci, (off, sz) in enumerate(NCH):
                        nc.scalar.activation(
                            qk_sb[:, m, off : off + sz],
                            ps[ci][:, :sz],
                            AF.Identity,
                            bias=bqk_sb[:, m : m + 1],
                        )
            q_sb = qk_sb[:, 0:MD, :]
            k_sb = qk_sb[:, MD : 2 * MD, :]
            ln1 = None

            # -------- vT (fp8 DoubleRow; xn stationary, wv moving) --------
            # vt layout per head: [64 v-cols | 64 ones-cols]; the ones are
            # persistent so the attention matmul yields the numerator (rows
            # 0-63) AND the replicated softmax denominator (rows 64-127).
            with tc.tile_pool(name=f"vtps_{l}", bufs=4, space="PSUM") as vtps:
                for b in range(BPC):
                    for c, (toff, tsz) in enumerate(TCH):
                        cols = S * b + toff
                        ps = [
                            vtps.tile(
                                [128, 384], F32, tag="ps", name=f"vtps_{l}_{b}_{c}_{n}"
                            )
                            for n in range(2)
                        ]
                        for k in range(KD):
                            for n in range(2):
                                nc.tensor.matmul(
                                    ps[n][:tsz, :],
                                    xn[:, k, cols : cols + tsz],
                                    wv[:, k, 384 * n : 384 * (n + 1)],
                                    start=(k == 0),
                                    stop=False,
                                    skip_group_check=True,
                                )
                        for n in range(2):
                            nc.tensor.matmul(
                                ps[n][:tsz, :],
                                ones_row[:, :tsz],
                                wv_aug[:, 384 * n : 384 * (n + 1)],
                                start=False,
                                stop=True,
                                skip_group_check=True,
                            )
                        dstv = vt_sb[:tsz, 2 * b + c, :].rearrange(
                            "p (h x) -> p h x", x=65
                        )
                        for n in range(2):
                            nc.vector.tensor_copy(
                                dstv[:, 6 * n : 6 * n + 6, 0:64],
                                ps[n][:tsz, :].rearrange("p (h x) -> p h x", x=64),
                            )

            # ---------------- attention (token-major) ----------------
            # attn output per (batch, s-chunk): [s, 12*65] split across two
            # PSUM banks of 6 heads; col 64 of each head-block is the softmax
            # denominator, landing on the token partition so one strided
            # reciprocal + per-partition tensor_scalar normalizes 6 heads.
            # The bf16 normalized tile is transposed back to feature-major
            # fp8 via PE identity-transposes.
            cat_sb = xn8_pool.tile([128, KD, T], FP8, tag="cat", name=f"cat_{l}")
            SCH = [(0, 128), (128, S - 128)]  # s-chunks within a batch
            with tc.tile_pool(name=f"scps_{l}", bufs=2, space="PSUM") as scps, \
                 tc.tile_pool(name=f"tmps_{l}", bufs=4, space="PSUM") as tmps, \
                 tc.tile_pool(name=f"tpps_{l}", bufs=2, space="PSUM") as tpps, \
                 tc.tile_pool(name=f"ctm_{l}", bufs=4) as ctm_pool, \
                 tc.tile_pool(name=f"rcp_{l}", bufs=4) as rcp_pool:

                def emit_norm(b, tm_tiles, cat_tm):
                    for sg, (soff, ssz) in enumerate(SCH):
                        for g in range(2):
                            tmt = tm_tiles[(sg, g)]
                            rcp = rcp_pool.tile(
                                [128, 6], F32, tag="rcp", name=f"rcp_{l}_{b}_{sg}_{g}"
                            )
                            den = tmt[0:ssz, :].rearrange(
                                "p (h x) -> p h x", x=65
                            )[:, :, 64]
                            nc.vector.tensor_copy(rcp[0:ssz, :], den)
                            nc.vector.reciprocal(rcp[0:ssz, :], rcp[0:ssz, :])
                            for j in range(6):
                                nc.vector.tensor_scalar(
                                    out=cat_tm[sg][0:ssz, 64 * (6 * g + j) : 64 * (6 * g + j) + 64],
                                    in0=tmt[0:ssz, 65 * j : 65 * j + 64],
                                    scalar1=rcp[0:ssz, j : j + 1],
                                    scalar2=None,
                                    op0=ALU.mult,
                                )

                def emit_transpose(b, cat_tm):
                    for sg, (soff, ssz) in enumerate(SCH):
                        for f in range(MD):
                            tp = tpps.tile(
                                [128, 128], BF16, tag="tp", name=f"tp_{l}_{b}_{sg}_{f}"
                            )
                            nc.tensor.matmul(
                                tp[:, 0:ssz],
                                cat_tm[sg][0:ssz, 128 * f : 128 * f + 128],
                                ident_b[0:ssz, 0:ssz],
                                is_transpose=True,
                            )
                            dst = cat_sb[:, f, S * b + soff : S * b + soff + ssz]
                            if f % 2 == 0:
                                nc.vector.tensor_copy(dst, tp[:, 0:ssz])
                            else:
                                nc.scalar.copy(dst, tp[:, 0:ssz])

                prev = None
                for b in range(BPC):
                    exp_tiles = {}
                    tm_tiles = {}
                    cat_tm = [
                        ctm_pool.tile(
                            [128, 768], BF16, tag="ctm", name=f"ctm_{l}_{b}_{sg}"
                        )
                        for sg in range(2)
                    ]

                    def emit_scores(h, b=b, exp_tiles=exp_tiles):
                        j, half = h // 2, h % 2
                        rows = slice(64 * half, 64 * half + 64)
                        sps_t = scps.tile(
                            [128, 2 * S], F32, tag="ps", name=f"sc_{l}_{b}_{h}"
                        )
                        for c, (toff, tsz) in enumerate(TCH):
                            cols = S * b + toff
                            nc.tensor.matmul(
                                sps_t[:tsz, S * c : S * c + S],
                                k_sb[rows, j, cols : cols + tsz],
                                q_sb[rows, j, S * b : S * (b + 1)],
                                start=(c == 0),
                                stop=True,
                                skip_group_check=True,
                            )
                        et = et_pool.tile(
                            [128, 2 * S], BF16, tag="expT", name=f"et_{l}_{b}_{h}"
                        )
                        nc.scalar.activation(et, sps_t, AF.Exp, scale=SCALE)
                        exp_tiles[h] = et

                    def emit_attn(h, b=b, exp_tiles=exp_tiles, tm_tiles=tm_tiles):
                        g, j = h // 6, h % 6
                        et = exp_tiles.pop(h)
                        for sg, (soff, ssz) in enumerate(SCH):
                            if (sg, g) not in tm_tiles:
                                tm_tiles[(sg, g)] = tmps.tile(
                                    [128, 390], F32, tag="tm",
                                    name=f"tm_{l}_{b}_{sg}_{g}",
                                )
                            tmt = tm_tiles[(sg, g)]
                            for c, (toff, tsz) in enumerate(TCH):
                                nc.tensor.matmul(
                                    tmt[0:ssz, 65 * j : 65 * j + 65],
                                    et[0:tsz, S * c + soff : S * c + soff + ssz],
                                    vt_sb[0:tsz, 2 * b + c, 65 * h : 65 * h + 65],
                                    start=(j == 0 and c == 0),
                                    stop=(c == 1),
                                    skip_group_check=True,
                                )

                    for h in range(2):
                        emit_scores(h)
                    for h in range(H):
                        if h + 2 < H:
                            emit_scores(h + 2)
                        emit_attn(h)
                    emit_norm(b, tm_tiles, cat_tm)
                    if prev is not None:
                        emit_transpose(*prev)
                    prev = (b, cat_tm)
                emit_transpose(*prev)

            # ------- Wo projection (fp8 DR) + residual --------------
            bo_sb = bias_pool.tile([128, MD], F32, tag="bo", name=f"bo_{l}")
            nc.sync.dma_start(out=bo_sb, in_=bo_d[l].rearrange("(m p) -> p m", p=128))
            xn2 = xn2_pool.tile([128, KD, T], BF16, tag="xn2", name=f"xn2_{l}")
            ln2 = LNPipe(f"ln2_{l}", x_sb, xn2, BF16)
            with tc.tile_pool(name=f"wops_{l}", bufs=4, space="PSUM") as wops:
                for m in range(MD):
                    wt = wst_pool.tile(
                        [128, KD, 128], FP8, tag="wst", name=f"wto_{l}_{m}"
                    )
                    nc.sync.dma_start(
                        out=wt, in_=Wo_d[l, m].rearrange("k p c -> p k c")
                    )
                    ps = [
                        wops.tile([128, 394], F32, tag="ps", name=f"wops_{l}_{m}_{ci}")
                        for ci in range(2)
                    ]
                    for p in range(KP):
                        ks = slice(2 * p, 2 * p + 2)
                        for ci, (off, sz) in enumerate(NCH):
                            nc.tensor.matmul(
                                ps[ci][:, :sz],
                                wt[:, ks, :],
                                cat_sb[:, ks, off : off + sz],
                                start=(p == 0),
                                stop=(p == KP - 1),
                                perf_mode=PM.DoubleRow,
                            )
                    for ci, (off, sz) in enumerate(NCH):
                        nc.vector.scalar_tensor_tensor(
                            x_sb[:, m, off : off + sz],
                            ps[ci][:, :sz],
                            bo_sb[:, m : m + 1],
                            x_sb[:, m, off : off + sz],
                            ALU.add,
                            ALU.add,
                        )
                    ln2.prep(0, m)
                    ln2.prep(1, m)
            ln2.sums(0)
            ln2.sums(1)

            # ---------------- LN2 -> xn2 (bf16) ----------------
            ln2.finish_stats(0)
            ln2.finish_stats(1)
            ln2.finish_apply(0)
            ln2.finish_apply(1)
            ln2 = None

            # ---------------- MLP (bf16) ----------------
            b1_sb = bias_pool.tile([128, MI], F32, tag="b1", name=f"b1_{l}")
            nc.sync.dma_start(out=b1_sb, in_=b1_d[l].rearrange("(m p) -> p m", p=128))
            b2_sb = bias_pool.tile([128, MD], F32, tag="b2", name=f"b2_{l}")
            nc.sync.dma_start(out=b2_sb, in_=b2_d[l].rearrange("(m p) -> p m", p=128))
            h_sb = h_pool.tile([128, KI, T], BF16, tag="h", name=f"h_{l}")
            with tc.tile_pool(name=f"w1ps_{l}", bufs=4, space="PSUM") as w1ps:
                for m in range(MI):
                    wt = w1st_pool.tile(
                        [128, KD, 128], BF16, tag="w1st", name=f"w1t_{l}_{m}"
                    )
                    nc.sync.dma_start(
                        out=wt, in_=W1_d[l, m].rearrange("k p c -> p k c")
                    )
                    ps = [
                        w1ps.tile([128, 394], F32, tag="ps", name=f"w1ps_{l}_{m}_{ci}")
                        for ci in range(2)
                    ]
                    for k in range(KD):
                        for ci, (off, sz) in enumerate(NCH):
                            nc.tensor.matmul(
                                ps[ci][:, :sz],
                                wt[:, k, :],
                                xn2[:, k, off : off + sz],
                                start=(k == 0),
                                stop=(k == KD - 1),
                            )
                    for ci, (off, sz) in enumerate(NCH):
                        nc.scalar.activation(
                            h_sb[:, m, off : off + sz],
                            ps[ci][:, :sz],
                            AF.Gelu,
                            bias=b1_sb[:, m : m + 1],
                        )
            xn_next = None
            ln1 = LNPipe(f"ln1n_{l}", x_sb, None, FP8)
            with tc.tile_pool(name=f"w2ps_{l}", bufs=4, space="PSUM") as w2ps:
                for m in range(MD):
                    w2t = w2st_pool.tile(
                        [128, KI, 128], BF16, tag="w2st", name=f"w2t_{l}_{m}"
                    )
                    nc.sync.dma_start(
                        out=w2t, in_=W2_d[l, m].rearrange("k p c -> p k c")
                    )
                    ps = [
                        w2ps.tile([128, 394], F32, tag="ps", name=f"w2ps_{l}_{m}_{ci}")
                        for ci in range(2)
                    ]
                    for k in range(KI):
                        for ci, (off, sz) in enumerate(NCH):
                            nc.tensor.matmul(
                                ps[ci][:, :sz],
                                w2t[:, k, :],
                                h_sb[:, k, off : off + sz],
                                start=(k == 0),
                                stop=(k == KI - 1),
                            )
                    for ci, (off, sz) in enumerate(NCH):
                        nc.vector.scalar_tensor_tensor(
                            x_sb[:, m, off : off + sz],
                            ps[ci][:, :sz],
                            b2_sb[:, m : m + 1],
                            x_sb[:, m, off : off + sz],
                            ALU.add,
                            ALU.add,
                        )
                    if l + 1 < nlayers:
                        ln1.prep(0, m)
                        ln1.prep(1, m)
            if l + 1 < nlayers:
                ln1.sums(0)
                ln1.sums(1)
                ln1.finish_stats(0)
                ln1.finish_stats(1)
            else:
                ln1 = None

        for k in range(KD):
            nc.sync.dma_start(out=out_d[128 * k : 128 * (k + 1), :], in_=x_sb[:, k, :])

    for inst in rsqrt_fixups:
        inst.func = mybir.ActivationFunctionType.Rsqrt
    ndedup = _dedup_ldweights(nc)
    nsplit = _split_multiwaits(nc)
    print(f"dedup {ndedup} ldweights; split {nsplit} multi-wait instructions")
    return nc


def prep_weights(inputs, nlayers=L):
    """Fold gamma/beta/biases into effective weights, host side (numpy)."""
    f32 = np.float32
    fp8 = ml_dtypes.float8_e4m3fn
    bf16 = ml_dtypes.bfloat16
    Wq = np.asarray(inputs["Wq"], f32)
    bq = np.asarray(inputs["bq"], f32)
    Wk = np.asarray(inputs["Wk"], f32)
    bk = np.asarray(inputs["bk"], f32)
    Wv = np.asarray(inputs["Wv"], f32)
    bv = np.asarray(inputs["bv"], f32)
    Wo = np.asarray(inputs["Wo"], f32)
    bo = np.asarray(inputs["bo"], f32)
    W1 = np.asarray(inputs["W1"], f32)
    b1 = np.asarray(inputs["b1"], f32)
    W2 = np.asarray(inputs["W2"], f32)
    b2 = np.asarray(inputs["b2"], f32)
    g1 = np.asarray(inputs["g1"], f32)
    be1 = np.asarray(inputs["be1"], f32)
    g2 = np.asarray(inputs["g2"], f32)
    be2 = np.asarray(inputs["be2"], f32)

    Wqk = np.zeros((nlayers, D, 2 * D), f32)
    bqk = np.zeros((nlayers, 2 * D), f32)
    Wvd = np.zeros((nlayers, D, 768), f32)
    Wvaug = np.zeros((nlayers, 1, 768), f32)
    W1e = np.zeros((nlayers, D, I), f32)
    b1e = np.zeros((nlayers, I), f32)
    for l in range(nlayers):
        for h in range(H):
            Wqk[l, :, h * DH : (h + 1) * DH] = Wq[l, h] * g1[l][:, None]
            Wqk[l, :, D + h * DH : D + (h + 1) * DH] = Wk[l, h] * g1[l][:, None]
            bqk[l, h * DH : (h + 1) * DH] = bq[l, h] + Wq[l, h].T @ be1[l]
            bqk[l, D + h * DH : D + (h + 1) * DH] = bk[l, h] + Wk[l, h].T @ be1[l]
            Wvd[l, :, 64 * h : 64 * h + DH] = Wv[l, h] * g1[l][:, None]
            Wvaug[l, 0, 64 * h : 64 * h + DH] = bv[l, h] + Wv[l, h].T @ be1[l]
        W1e[l] = W1[l] * g2[l][:, None]
        b1e[l] = b1[l] + W1[l].T @ be2[l]

    Wqk8 = np.zeros((nlayers, 12, KD, 128, 128), fp8)
    Wo8 = np.zeros((nlayers, MD, KD, 128, 128), fp8)
    W1b = np.zeros((nlayers, MI, KD, 128, 128), bf16)
    W2b = np.zeros((nlayers, MD, KI, 128, 128), bf16)
    for l in range(nlayers):
        for m in range(12):
            Wqk8[l, m] = Wqk[l][:, 128 * m : 128 * (m + 1)].reshape(KD, 128, 128)
        for m in range(MD):
            Wo8[l, m] = Wo[l][:, 128 * m : 128 * (m + 1)].reshape(KD, 128, 128)
        for m in range(MI):
            W1b[l, m] = W1e[l][:, 128 * m : 128 * (m + 1)].reshape(KD, 128, 128)
        for m in range(MD):
            W2b[l, m] = W2[l][:, 128 * m : 128 * (m + 1)].reshape(KI, 128, 128)

    return {
        "ident": np.eye(128, dtype=bf16),
        "Wqk": Wqk8,
        "Wo": Wo8,
        "W1": W1b,
        "W2": W2b,
        "Wv": Wvd.astype(fp8),
        "Wvaug": Wvaug.astype(bf16),
        "bqk": bqk,
        "bo": np.ascontiguousarray(bo[:nlayers]),
        "b1": b1e,
        "b2": np.ascontiguousarray(b2[:nlayers]),
    }


_cache = {}


def run_cores(inputs, nlayers=L, trace=False):
    X = np.asarray(inputs["X"], np.float32)
    wmap = prep_weights(inputs, nlayers)

    key = ("nc", nlayers)
    if key not in _cache:
        _cache[key] = build(nlayers)
    nc = _cache[key]

    in_maps = []
    for c in range(NCORES):
        xc = X[BPC * c : BPC * (c + 1)].reshape(T, D).T  # [D, T]
        m = {"xT": np.ascontiguousarray(xc)}
        m.update(wmap)
        in_maps.append(m)

    res = run_bass_kernel_spmd(nc, in_maps, core_ids=list(range(NCORES)), trace=trace)
    out = np.zeros((B, S, D), np.float32)
    for c in range(NCORES):
        out[BPC * c : BPC * (c + 1)] = res.results[c]["out"].T.reshape(BPC, S, D)
    return out, res


def kernel(**inputs):
    out, _ = run_cores(inputs)
    return out


# revision 13
# speedup vs baseline: 1.5571x; 1.0964x over previous
"""ViT-Base encoder (12 layers, B=32, S=197, D=768, H=12, I=3072) on 8 trn2
NeuronCores, data-parallel over the batch (4 images per core).

v2: the attention block (q/k/v projections, Wo) and the LayerNorm stat
reductions run as fp8e4m3 DoubleRow matmuls (2 contraction rows per PE
cell, 2x bf16 throughput); the MLP stays bf16 (fp8 there costs ~6e-2
rel err).  Softmax normalization exploits the ones-columns trick: the
attention matmul leaves the denominator replicated on PSUM rows 64-127,
so a single [64,S] bf16 reciprocal + one multiply normalizes a head
(no PE broadcast, no per-head staging copies).  Activations feeding fp8
matmuls (xn, cat) are stored fp8; the residual stream and LN stats stay
fp32.
"""

import sys

sys.path.insert(0, "/opt/trn_rl_repo")

import contextlib

import numpy as np
import ml_dtypes

import concourse.bass as bass
import concourse.mybir as mybir
import concourse.tile as tile
from concourse.vector_clock import ScopedClock
from concourse.bass_utils import run_bass_kernel_spmd

L, D, I, H, DH = 12, 768, 3072, 12, 64
B, S = 32, 197
NCORES = 8
BPC = B // NCORES  # batches per core
T = BPC * S  # 788 tokens per core
SCALE = float(1.0 / np.sqrt(DH))
EPS = 1e-5

F32 = mybir.dt.float32
BF16 = mybir.dt.bfloat16
FP8 = mybir.dt.float8e4
AF = mybir.ActivationFunctionType
ALU = mybir.AluOpType
PM = mybir.MatmulPerfMode

KD = D // 128  # 6 contraction chunks over D
KI = I // 128  # 24 contraction chunks over I
MD = D // 128  # 6 output tiles over D
MI = I // 128  # 24 output tiles over I
KP = KD // 2  # 3 fp8 DoubleRow contraction pairs over D

NCH = [(0, 394), (394, 394)]  # PSUM-half chunks for dense matmul phases
LCH = [(0, 394), (394, 394)]  # chunks for LN/elementwise work (aligned to NCH)
TCH = [(0, 128), (128, S - 128)]  # within-batch token chunks (128+69)
VW = H * 128  # vt tile: per head [64 v-cols | 64 ones-cols]


class SplitDrainTileContext(tile.TileContext):
    """TileContext whose kernel-tail drain splits its sem waits across
    multiple SP instructions (this walrus rejects >1 wait on a Drain)."""

    def _drain_and_barrier(self, tick_clock, wait_clock):
        nc = self.nc
        drain_inst = nc.sync.drain()
        wait_clock.add_sem_waits(
            drain_inst.ins, ScopedClock({None: tick_clock.global_clock})
        )
        si = drain_inst.ins.sync_info
        waits = list(si.on_wait) if si is not None else []
        if len(waits) > 1:
            drain_inst.ins.sync_info = mybir.SyncInfo(
                on_wait=[waits[0]], on_update=list(si.on_update)
            )
            by_name = {}
            for h in self.sems.allocated().values():
                by_name[getattr(h, "name", None)] = h
            for w in waits[1:]:
                h = by_name.get(w.ant_name)
                assert h is not None, f"no handle for sem {w.ant_name}"
                nc.sync.wait_ge(h, w.wait_value)

        nc.all_engine_barrier()
        assert self.sems is not None
        popped = nc._tile_sem_poison_stack.pop()
        assert popped is self._sem_poison
        nc.clear_and_free_semaphores(list(self.sems.allocated().values()))
        nc.all_engine_barrier()


def _dedup_ldweights(nc):
    """Remove Ldweights whose weights are already resident in the PE array
    (identical signature to the previous Ldweights, nothing invalidated the
    array in between).  Carried sem waits/updates move to the next PE
    instruction; _split_multiwaits hoists any overflow afterwards."""
    removed = 0
    for fn in nc.m.functions:
        for bb in fn.blocks:
            lst = bb.instructions
            last_sig = None
            keep = []
            pending_waits = []
            pending_updates = []
            for inst in lst:
                eng = inst.engine
                if inst.opcode == "Ldweights":
                    sig = (
                        str(inst.ins[0]),
                        str(getattr(inst, "is_transpose", None)),
                        str(getattr(inst, "perf_mode", None)),
                        str(getattr(inst, "tile_position", None)),
                    )
                    if sig == last_sig:
                        si = inst.sync_info
                        if si is not None:
                            pending_waits.extend(si.on_wait)
                            pending_updates.extend(si.on_update)
                        removed += 1
                        continue
                    last_sig = sig
                elif inst.opcode == "Matmult" and str(
                    getattr(inst, "is_transpose", None)
                ) not in ("None", "False"):
                    last_sig = None  # transpose-mode clobbers the array
                if (pending_waits or pending_updates) and eng == mybir.EngineType.PE:
                    si = inst.sync_info
                    ow = list(si.on_wait) if si else []
                    ou = list(si.on_update) if si else []
                    inst.sync_info = mybir.SyncInfo(
                        on_wait=ow + pending_waits, on_update=ou + pending_updates
                    )
                    pending_waits, pending_updates = [], []
                keep.append(inst)
            assert not pending_waits and not pending_updates
            lst[:] = keep
    return removed


def _split_multiwaits(nc):
    """This walrus accepts at most 1 sem wait per instruction (2 on an
    EventSemaphore).  Tile freely packs several; hoist the excess into
    standalone EventSemaphore instructions inserted just before."""
    n = 0
    for fn in nc.m.functions:
        for bb in fn.blocks:
            lst = bb.instructions
            i = 0
            while i < len(lst):
                inst = lst[i]
                si = getattr(inst, "sync_info", None)
                if si is not None and si.on_wait:
                    cap = 2 if inst.opcode == "EventSemaphore" else 1
                    waits = list(si.on_wait)
                    if len(waits) > cap:
                        keep, extra = waits[:cap], waits[cap:]
                        new_insts = []
                        for j in range(0, len(extra), 2):
                            ev = mybir.InstEventSemaphore(
                                name=f"wsplit_{n}", ins=[], outs=[]
                            )
                            n += 1
                            ev.engine = inst.engine
                            ev.sync_info = mybir.SyncInfo(
                                on_wait=list(extra[j : j + 2]), on_update=[]
                            )
                            new_insts.append(ev)
                        inst.sync_info = mybir.SyncInfo(
                            on_wait=keep, on_update=list(si.on_update)
                        )
                        lst[i:i] = new_insts
                        i += len(new_insts)
                i += 1
    return n


def build(nlayers=L):
    nc = bass.Bass()

    xT = nc.dram_tensor("xT", [D, T], F32, kind="ExternalInput")
    Wqk_d = nc.dram_tensor("Wqk", [nlayers, 12, KD, 128, 128], FP8, kind="ExternalInput")
    Wo_d = nc.dram_tensor("Wo", [nlayers, MD, KD, 128, 128], FP8, kind="ExternalInput")
    W1_d = nc.dram_tensor("W1", [nlayers, MI, KD, 128, 128], BF16, kind="ExternalInput")
    W2_d = nc.dram_tensor("W2", [nlayers, MD, KI, 128, 128], BF16, kind="ExternalInput")
    Wv_d = nc.dram_tensor("Wv", [nlayers, D, 768], FP8, kind="ExternalInput")
    Wvaug_d = nc.dram_tensor("Wvaug", [nlayers, 1, 768], BF16, kind="ExternalInput")
    bqk_d = nc.dram_tensor("bqk", [nlayers, 2 * D], F32, kind="ExternalInput")
    bo_d = nc.dram_tensor("bo", [nlayers, D], F32, kind="ExternalInput")
    b1_d = nc.dram_tensor("b1", [nlayers, I], F32, kind="ExternalInput")
    b2_d = nc.dram_tensor("b2", [nlayers, D], F32, kind="ExternalInput")
    ident_d = nc.dram_tensor("ident", [128, 128], BF16, kind="ExternalInput")
    out_d = nc.dram_tensor("out", [D, T], F32, kind="ExternalOutput")

    with SplitDrainTileContext(nc) as tc, contextlib.ExitStack() as ctx, \
         nc.allow_low_precision(reason="fp8 attention, bf16 MLP; residual/stats fp32"):
        persist = ctx.enter_context(tc.tile_pool(name="persist", bufs=1))
        x_sb = persist.tile([128, MD, T], F32, tag="x")
        ones_row = persist.tile([1, 128], BF16, tag="ones_row")
        eps_t = persist.tile([1, 1], F32, tag="eps")
        # DoubleRow stationaries for the LN partition sums: [K=128, pair, col]
        # col 0 sums the tile, col 1 sums the squares tile.
        ones2s = persist.tile([128, 2, 64], FP8, tag="ones2s")
        ones2q = persist.tile([128, 2, 64], FP8, tag="ones2q")
        # vt tile persists so its ones-columns are memset exactly once.
        # Per head: [64 v-cols | 1 ones-col] -> token-major attention output
        # [s, 65] whose col 64 is the softmax denominator (per-partition!).
        vt_sb = persist.tile([128, 2 * BPC, H * 65], BF16, tag="vt")
        ident_b = persist.tile([128, 128], BF16, tag="ident")
        nc.sync.dma_start(out=ident_b, in_=ident_d[:, :])
        nc.vector.memset(ones_row, 1.0)
        nc.vector.memset(eps_t, EPS)
        nc.vector.memset(ones2s, 0.0)
        nc.vector.memset(ones2q, 0.0)
        nc.vector.memset(ones2s[:, :, 0:1], 1.0)
        nc.vector.memset(ones2q[:, :, 32:33], 1.0)
        for i in range(2 * BPC):
            ones_view = vt_sb[:, i, :].rearrange("p (h x) -> p h x", x=65)
            nc.gpsimd.memset(ones_view[:, :, 64:65], 1.0)

        for k in range(KD):
            nc.sync.dma_start(out=x_sb[:, k, :], in_=xT[128 * k : 128 * (k + 1), :])

        rsqrt_fixups = []
        stat_pool = ctx.enter_context(tc.tile_pool(name="stats", bufs=1))
        xn8_pool = ctx.enter_context(tc.tile_pool(name="xn8", bufs=1))
        xn2_pool = ctx.enter_context(tc.tile_pool(name="xn2", bufs=1))
        qk_pool = ctx.enter_context(tc.tile_pool(name="qk", bufs=1))
        h_pool = ctx.enter_context(tc.tile_pool(name="h", bufs=1))
        bias_pool = ctx.enter_context(tc.tile_pool(name="bias", bufs=2))
        wst_pool = ctx.enter_context(tc.tile_pool(name="wst", bufs=8))
        w1st_pool = ctx.enter_context(tc.tile_pool(name="w1st", bufs=8))
        w2st_pool = ctx.enter_context(tc.tile_pool(name="w2st", bufs=4))
        wv_pool = ctx.enter_context(tc.tile_pool(name="wv", bufs=1))
        et_pool = ctx.enter_context(tc.tile_pool(name="expt", bufs=6))
        rec_pool = ctx.enter_context(tc.tile_pool(name="rec", bufs=3))
        xb_pool = ctx.enter_context(tc.tile_pool(name="xb", bufs=1))
        sq_pool = ctx.enter_context(tc.tile_pool(name="sq", bufs=1))
        lnt_pool = ctx.enter_context(tc.tile_pool(name="lnt", bufs=3))

        class LNPipe:
            """LayerNorm over features (partitions).  Stats come from an fp8
            shadow of x via DoubleRow ones-matmuls (sum into PSUM row 0,
            sum-of-squares into row 1); normalization multiplies the fp32
            residual by PE-broadcast stats."""

            def __init__(self, name, src, dst, dst_dtype):
                self.name, self.src, self.dst = name, src, dst
                self.dst_dtype = dst_dtype
                self.mu = stat_pool.tile([1, T], F32, tag="mu", name=name + "_mu")
                self.rs = stat_pool.tile([1, T], F32, tag="rs", name=name + "_rs")
                self.mu_b = stat_pool.tile([1, T], BF16, tag="mu_b", name=name + "_mub")
                self.rs_b = stat_pool.tile([1, T], BF16, tag="rs_b", name=name + "_rsb")
                self.xb = xb_pool.tile([128, KD, T], FP8, tag="xb", name=name + "_xb")
                self.sq = sq_pool.tile([128, KD, T], FP8, tag="sq", name=name + "_sq")
                self.prepped = set()

            def prep(self, ci, k):
                """fp8 shadow + squares for x[:, k, LCH[ci]] (emit as soon as
                that region is final so it overlaps the producing phase)."""
                off, sz = LCH[ci]
                cs = slice(off, off + sz)
                nc.gpsimd.tensor_copy(self.xb[:, k, cs], self.src[:, k, cs])
                nc.scalar.activation(self.sq[:, k, cs], self.xb[:, k, cs], AF.Square)
                self.prepped.add((ci, k))

            def sums(self, ci):
                off, sz = LCH[ci]
                cs = slice(off, off + sz)
                for k in range(KD):
                    if (ci, k) not in self.prepped:
                        self.prep(ci, k)
                with tc.tile_pool(
                    name=f"{self.name}_sps{ci}", bufs=1, space="PSUM"
                ) as sps:
                    sp = sps.tile([64, 394], F32, tag="sum", name=f"{self.name}_sum{ci}")
                    for p in range(KP):
                        ks = slice(2 * p, 2 * p + 2)
                        nc.tensor.matmul(
                            sp[:, :sz], ones2s, self.xb[:, ks, cs],
                            start=(p == 0), stop=False, perf_mode=PM.DoubleRow,
                            skip_group_check=True,
                        )
                        nc.tensor.matmul(
                            sp[:, :sz], ones2q, self.sq[:, ks, cs],
                            start=False, stop=(p == KP - 1), perf_mode=PM.DoubleRow,
                            skip_group_check=True,
                        )
                    nc.scalar.mul(self.mu[0:1, cs], sp[0:1, :sz], 1.0 / D)
                    # var = E[x^2] - mu^2, with E[x^2] read straight from PSUM
                    nc.vector.scalar_tensor_tensor(
                        self.rs[:, cs], self.mu[0:1, cs], -1.0, self.mu[0:1, cs],
                        ALU.mult, ALU.mult,
                    )
                    nc.vector.scalar_tensor_tensor(
                        self.rs[:, cs], sp[32:33, :sz], 1.0 / D, self.rs[:, cs],
                        ALU.mult, ALU.add,
                    )

            def finish_stats(self, ci):
                off, sz = LCH[ci]
                cs = slice(off, off + sz)
                h = nc.scalar.activation(
                    self.rs_b[:, cs], self.rs[:, cs], AF.Sqrt, bias=eps_t, scale=1.0
                )
                rsqrt_fixups.append(h.ins)
                nc.scalar.copy(self.mu_b[:, cs], self.mu[0:1, cs])

            def finish_apply(self, ci):
                off, sz = LCH[ci]
                cs = slice(off, off + sz)
                with tc.tile_pool(
                    name=f"{self.name}_bps{ci}", bufs=1, space="PSUM"
                ) as bps:
                    bmu = bps.tile([128, 394], F32, tag="bmu", name=f"{self.name}_bmu{ci}")
                    brs = bps.tile([128, 394], F32, tag="brs", name=f"{self.name}_brs{ci}")
                    nc.tensor.matmul(bmu[:, :sz], ones_row, self.mu_b[:, cs])
                    nc.tensor.matmul(brs[:, :sz], ones_row, self.rs_b[:, cs])
                    for k in range(KD):
                        lnt = lnt_pool.tile(
                            [128, 394], F32, tag="lnt", name=f"{self.name}_lnt_{ci}_{k}"
                        )
                        nc.vector.tensor_sub(
                            lnt[:, :sz], self.src[:, k, cs], bmu[:, :sz]
                        )
                        nc.vector.tensor_mul(
                            self.dst[:, k, cs], lnt[:, :sz], brs[:, :sz]
                        )

        ln1 = ln2 = None
        for l in range(nlayers):
            wv = wv_pool.tile([128, KD, 768], FP8, tag="wv", name=f"wv_{l}")
            for k in range(KD):
                nc.sync.dma_start(
                    out=wv[:, k, :], in_=Wv_d[l, 128 * k : 128 * (k + 1), :]
                )
            wv_aug = wv_pool.tile([1, 768], BF16, tag="wv_aug", name=f"wva_{l}")
            nc.sync.dma_start(out=wv_aug, in_=Wvaug_d[l, :, :])
            bqk_sb = bias_pool.tile([128, 2 * MD], F32, tag="bqk", name=f"bqk_{l}")
            nc.sync.dma_start(out=bqk_sb, in_=bqk_d[l].rearrange("(m p) -> p m", p=128))

            # ---------------- LN1 -> xn (fp8) ----------------
            xn = xn8_pool.tile([128, KD, T], FP8, tag="xn", name=f"xn_{l}")
            # pair-contiguous copy of xn for the v DoubleRow stationary:
            # [p, kpair, (b,c)-chunk, pair, token]; tail chunks padded to 128
            # (garbage pad tokens only pollute unread PSUM rows).
            xnv = xn8_pool.tile([128, KP, 2 * BPC, 2, 128], FP8, tag="xnv",
                                name=f"xnv_{l}")

            def emit_xnv(ci):
                for b in (range(0, 2) if ci == 0 else range(2, BPC)):
                    for c, (toff, tsz) in enumerate(TCH):
                        cols = S * b + toff
                        for p in range(KP):
                            nc.gpsimd.tensor_copy(
                                xnv[:, p, 2 * b + c, :, :tsz],
                                xn[:, 2 * p : 2 * p + 2, cols : cols + tsz],
                            )
            if ln1 is None:  # first layer: sums/stats not yet emitted by W2
                ln1 = LNPipe(f"ln1_{l}", x_sb, xn, FP8)
                ln1.sums(0)
                ln1.sums(1)
                ln1.finish_stats(0)
                ln1.finish_stats(1)
            ln1.dst = xn
            ln1.finish_apply(0)
            ln1.finish_apply(1)

            # ------------- q, k projections (fp8 DoubleRow, ci-outer) -------------
            qk_sb = qk_pool.tile([128, 2 * MD, T], BF16, tag="qk", name=f"qk_{l}")
            with tc.tile_pool(name=f"qkps_{l}", bufs=4, space="PSUM") as qkps:
                for ci, (off, sz) in enumerate(NCH):
                    for m in range(2 * MD):
                        wt = wst_pool.tile(
                            [128, KD, 128], FP8, tag="wst", name=f"wt_{l}_{m}_{ci}"
                        )
                        nc.sync.dma_start(
                            out=wt, in_=Wqk_d[l, m].rearrange("k p c -> p k c")
                        )
                        ps = qkps.tile(
                            [128, 394], F32, tag="ps", name=f"qkps_{l}_{m}_{ci}"
                        )
                        for p in range(KP):
                            ks = slice(2 * p, 2 * p + 2)
                            nc.tensor.matmul(
                                ps[:, :sz],
                                wt[:, ks, :],
                                xn[:, ks, off : off + sz],
                                start=(p == 0),
                                stop=(p == KP - 1),
                                perf_mode=PM.DoubleRow,
                            )
                        nc.scalar.activation(
                            qk_sb[:, m, off : off + sz],
                            ps[:, :sz],
                            AF.Identity,
                            bias=bqk_sb[:, m : m + 1],
                        )
                    emit_xnv(ci)
            q_sb = qk_sb[:, 0:MD, :]
            k_sb = qk_sb[:, MD : 2 * MD, :]
            ln1 = None

            # -------- vT (fp8 DoubleRow; xn stationary, wv moving) --------
            # vt layout per head: [64 v-cols | 64 ones-cols]; the ones are
            # persistent so the attention matmul yields the numerator (rows
            # 0-63) AND the replicated softmax denominator (rows 64-127).
            with tc.tile_pool(name=f"vtps_{l}", bufs=4, space="PSUM") as vtps:
                for b in range(BPC):
                    for c, (toff, tsz) in enumerate(TCH):
                        cols = S * b + toff
                        ps = [
                            vtps.tile(
                                [128, 384], F32, tag="ps", name=f"vtps_{l}_{b}_{c}_{n}"
                            )
                            for n in range(2)
                        ]
                        for p in range(KP):
                            ks = slice(2 * p, 2 * p + 2)
                            for n in range(2):
                                nc.tensor.matmul(
                                    ps[n][:, :],
                                    xnv[:, p, 2 * b + c, :, :],
                                    wv[:, ks, 384 * n : 384 * (n + 1)],
                                    start=(p == 0),
                                    stop=False,
                                    perf_mode=PM.DoubleRow,
                                    skip_group_check=True,
                                )
                        for n in range(2):
                            nc.tensor.matmul(
                                ps[n][:tsz, :],
                                ones_row[:, :tsz],
                                wv_aug[:, 384 * n : 384 * (n + 1)],
                                start=False,
                                stop=True,
                                skip_group_check=True,
                            )
                        dstv = vt_sb[:tsz, 2 * b + c, :].rearrange(
                            "p (h x) -> p h x", x=65
                        )
                        for n in range(2):
                            nc.vector.tensor_copy(
                                dstv[:, 6 * n : 6 * n + 6, 0:64],
                                ps[n][:tsz, :].rearrange("p (h x) -> p h x", x=64),
                            )

            # ---------------- attention (token-major) ----------------
            # attn output per (batch, s-chunk): [s, 12*65] split across two
            # PSUM banks of 6 heads; col 64 of each head-block is the softmax
            # denominator, landing on the token partition so one strided
            # reciprocal + per-partition tensor_scalar normalizes 6 heads.
            # The bf16 normalized tile is transposed back to feature-major
            # fp8 via PE identity-transposes.
            cat_sb = xn8_pool.tile([128, KD, T], FP8, tag="cat", name=f"cat_{l}")
            SCH = [(0, 128), (128, S - 128)]  # s-chunks within a batch
            with tc.tile_pool(name=f"scps_{l}", bufs=2, space="PSUM") as scps, \
                 tc.tile_pool(name=f"tmps_{l}", bufs=4, space="PSUM") as tmps, \
                 tc.tile_pool(name=f"tpps_{l}", bufs=2, space="PSUM") as tpps, \
                 tc.tile_pool(name=f"ctm_{l}", bufs=4) as ctm_pool, \
                 tc.tile_pool(name=f"rcp_{l}", bufs=4) as rcp_pool:

                def emit_norm(b, tm_tiles, cat_tm):
                    for sg, (soff, ssz) in enumerate(SCH):
                        for g in range(2):
                            tmt = tm_tiles[(sg, g)]
                            rcp = rcp_pool.tile(
                                [128, 6], F32, tag="rcp", name=f"rcp_{l}_{b}_{sg}_{g}"
                            )
                            den = tmt[0:ssz, :].rearrange(
                                "p (h x) -> p h x", x=65
                            )[:, :, 64]
                            nc.vector.tensor_copy(rcp[0:ssz, :], den)
                            nc.vector.reciprocal(rcp[0:ssz, :], rcp[0:ssz, :])
                            for j in range(6):
                                nc.vector.tensor_scalar(
                                    out=cat_tm[sg][0:ssz, 64 * (6 * g + j) : 64 * (6 * g + j) + 64],
                                    in0=tmt[0:ssz, 65 * j : 65 * j + 64],
                                    scalar1=rcp[0:ssz, j : j + 1],
                                    scalar2=None,
                                    op0=ALU.mult,
                                )

                def emit_transpose(b, cat_tm):
                    for sg, (soff, ssz) in enumerate(SCH):
                        for f in range(MD):
                            tp = tpps.tile(
                                [128, 128], BF16, tag="tp", name=f"tp_{l}_{b}_{sg}_{f}"
                            )
                            nc.tensor.matmul(
                                tp[:, 0:ssz],
                                cat_tm[sg][0:ssz, 128 * f : 128 * f + 128],
                                ident_b[0:ssz, 0:ssz],
                                is_transpose=True,
                            )
                            dst = cat_sb[:, f, S * b + soff : S * b + soff + ssz]
                            if f % 2 == 0:
                                nc.vector.tensor_copy(dst, tp[:, 0:ssz])
                            else:
                                nc.scalar.copy(dst, tp[:, 0:ssz])

                prev = None
                for b in range(BPC):
                    exp_tiles = {}
                    tm_tiles = {}
                    cat_tm = [
                        ctm_pool.tile(
                            [128, 768], BF16, tag="ctm", name=f"ctm_{l}_{b}_{sg}"
                        )
                        for sg in range(2)
                    ]

                    def emit_scores(h, b=b, exp_tiles=exp_tiles):
                        j, half = h // 2, h % 2
                        rows = slice(64 * half, 64 * half + 64)
                        sps_t = scps.tile(
                            [128, 2 * S], F32, tag="ps", name=f"sc_{l}_{b}_{h}"
                        )
                        for c, (toff, tsz) in enumerate(TCH):
                            cols = S * b + toff
                            nc.tensor.matmul(
                                sps_t[:tsz, S * c : S * c + S],
                                k_sb[rows, j, cols : cols + tsz],
                                q_sb[rows, j, S * b : S * (b + 1)],
                                start=(c == 0),
                                stop=True,
                                skip_group_check=True,
                            )
                        et = et_pool.tile(
                            [128, 2 * S], BF16, tag="expT", name=f"et_{l}_{b}_{h}"
                        )
                        nc.scalar.activation(et, sps_t, AF.Exp, scale=SCALE)
                        exp_tiles[h] = et

                    def emit_attn(h, b=b, exp_tiles=exp_tiles, tm_tiles=tm_tiles):
                        g, j = h // 6, h % 6
                        et = exp_tiles.pop(h)
                        for sg, (soff, ssz) in enumerate(SCH):
                            if (sg, g) not in tm_tiles:
                                tm_tiles[(sg, g)] = tmps.tile(
                                    [128, 390], F32, tag="tm",
                                    name=f"tm_{l}_{b}_{sg}_{g}",
                                )
                            tmt = tm_tiles[(sg, g)]
                            for c, (toff, tsz) in enumerate(TCH):
                                nc.tensor.matmul(
                                    tmt[0:ssz, 65 * j : 65 * j + 65],
                                    et[0:tsz, S * c + soff : S * c + soff + ssz],
                                    vt_sb[0:tsz, 2 * b + c, 65 * h : 65 * h + 65],
                                    start=(j == 0 and c == 0),
                                    stop=(c == 1),
                                    skip_group_check=True,
                                )

                    for h in range(2):
                        emit_scores(h)
                    for h in range(H):
                        if h + 2 < H:
                            emit_scores(h + 2)
                        emit_attn(h)
                    emit_norm(b, tm_tiles, cat_tm)
                    if prev is not None:
                        emit_transpose(*prev)
                    prev = (b, cat_tm)
                emit_transpose(*prev)

            # ------- Wo projection (fp8 DR) + residual --------------
            bo_sb = bias_pool.tile([128, MD], F32, tag="bo", name=f"bo_{l}")
            nc.sync.dma_start(out=bo_sb, in_=bo_d[l].rearrange("(m p) -> p m", p=128))
            xn2 = xn2_pool.tile([128, KD, T], BF16, tag="xn2", name=f"xn2_{l}")
            ln2 = LNPipe(f"ln2_{l}", x_sb, xn2, BF16)
            with tc.tile_pool(name=f"wops_{l}", bufs=4, space="PSUM") as wops:
                for ci, (off, sz) in enumerate(NCH):
                    for m in range(MD):
                        wt = wst_pool.tile(
                            [128, KD, 128], FP8, tag="wst", name=f"wto_{l}_{m}_{ci}"
                        )
                        nc.sync.dma_start(
                            out=wt, in_=Wo_d[l, m].rearrange("k p c -> p k c")
                        )
                        ps = wops.tile(
                            [128, 394], F32, tag="ps", name=f"wops_{l}_{m}_{ci}"
                        )
                        for p in range(KP):
                            ks = slice(2 * p, 2 * p + 2)
                            nc.tensor.matmul(
                                ps[:, :sz],
                                wt[:, ks, :],
                                cat_sb[:, ks, off : off + sz],
                                start=(p == 0),
                                stop=(p == KP - 1),
                                perf_mode=PM.DoubleRow,
                            )
                        nc.vector.scalar_tensor_tensor(
                            x_sb[:, m, off : off + sz],
                            ps[:, :sz],
                            bo_sb[:, m : m + 1],
                            x_sb[:, m, off : off + sz],
                            ALU.add,
                            ALU.add,
                        )
                        ln2.prep(ci, m)
                    ln2.sums(ci)
                    ln2.finish_stats(ci)
                    ln2.finish_apply(ci)

            ln2 = None

            # ---------------- MLP (bf16) ----------------
            b1_sb = bias_pool.tile([128, MI], F32, tag="b1", name=f"b1_{l}")
            nc.sync.dma_start(out=b1_sb, in_=b1_d[l].rearrange("(m p) -> p m", p=128))
            b2_sb = bias_pool.tile([128, MD], F32, tag="b2", name=f"b2_{l}")
            nc.sync.dma_start(out=b2_sb, in_=b2_d[l].rearrange("(m p) -> p m", p=128))
            h_sb = h_pool.tile([128, KI, T], BF16, tag="h", name=f"h_{l}")
            with tc.tile_pool(name=f"w1ps_{l}", bufs=4, space="PSUM") as w1ps:
                for ci, (off, sz) in enumerate(NCH):
                    for m in range(MI):
                        wt = w1st_pool.tile(
                            [128, KD, 128], BF16, tag="w1st", name=f"w1t_{l}_{m}_{ci}"
                        )
                        nc.sync.dma_start(
                            out=wt, in_=W1_d[l, m].rearrange("k p c -> p k c")
                        )
                        ps = w1ps.tile(
                            [128, 394], F32, tag="ps", name=f"w1ps_{l}_{m}_{ci}"
                        )
                        for k in range(KD):
                            nc.tensor.matmul(
                                ps[:, :sz],
                                wt[:, k, :],
                                xn2[:, k, off : off + sz],
                                start=(k == 0),
                                stop=(k == KD - 1),
                            )
                        nc.scalar.activation(
                            h_sb[:, m, off : off + sz],
                            ps[:, :sz],
                            AF.Gelu,
                            bias=b1_sb[:, m : m + 1],
                        )
            xn_next = None
            ln1 = LNPipe(f"ln1n_{l}", x_sb, None, FP8)
            with tc.tile_pool(name=f"w2ps_{l}", bufs=4, space="PSUM") as w2ps:
                for ci, (off, sz) in enumerate(NCH):
                    for m in range(MD):
                        w2t = w2st_pool.tile(
                            [128, KI, 128], BF16, tag="w2st", name=f"w2t_{l}_{m}_{ci}"
                        )
                        nc.sync.dma_start(
                            out=w2t, in_=W2_d[l, m].rearrange("k p c -> p k c")
                        )
                        ps = w2ps.tile(
                            [128, 394], F32, tag="ps", name=f"w2ps_{l}_{m}_{ci}"
                        )
                        for k in range(KI):
                            nc.tensor.matmul(
                                ps[:, :sz],
                                w2t[:, k, :],
                                h_sb[:, k, off : off + sz],
                                start=(k == 0),
                                stop=(k == KI - 1),
                            )
                        nc.vector.scalar_tensor_tensor(
                            x_sb[:, m, off : off + sz],
                            ps[:, :sz],
                            b2_sb[:, m : m + 1],
                            x_sb[:, m, off : off + sz],
                            ALU.add,
                            ALU.add,
                        )
                        if l + 1 < nlayers:
                            ln1.prep(ci, m)
                    if l + 1 < nlayers:
                        ln1.sums(ci)
                        ln1.finish_stats(ci)
            if l + 1 >= nlayers:
                ln1 = None

        for k in range(KD):
            nc.sync.dma_start(out=out_d[128 * k : 128 * (k + 1), :], in_=x_sb[:, k, :])

    for inst in rsqrt_fixups:
        inst.func = mybir.ActivationFunctionType.Rsqrt
    ndedup = _dedup_ldweights(nc)
    nsplit = _split_multiwaits(nc)
    print(f"dedup {ndedup} ldweights; split {nsplit} multi-wait instructions")
    return nc


def prep_weights(inputs, nlayers=L):
    """Fold gamma/beta/biases into effective weights, host side (numpy)."""
    f32 = np.float32
    fp8 = ml_dtypes.float8_e4m3fn
    bf16 = ml_dtypes.bfloat16
    Wq = np.asarray(inputs["Wq"], f32)
    bq = np.asarray(inputs["bq"], f32)
    Wk = np.asarray(inputs["Wk"], f32)
    bk = np.asarray(inputs["bk"], f32)
    Wv = np.asarray(inputs["Wv"], f32)
    bv = np.asarray(inputs["bv"], f32)
    Wo = np.asarray(inputs["Wo"], f32)
    bo = np.asarray(inputs["bo"], f32)
    W1 = np.asarray(inputs["W1"], f32)
    b1 = np.asarray(inputs["b1"], f32)
    W2 = np.asarray(inputs["W2"], f32)
    b2 = np.asarray(inputs["b2"], f32)
    g1 = np.asarray(inputs["g1"], f32)
    be1 = np.asarray(inputs["be1"], f32)
    g2 = np.asarray(inputs["g2"], f32)
    be2 = np.asarray(inputs["be2"], f32)

    Wqk = np.zeros((nlayers, D, 2 * D), f32)
    bqk = np.zeros((nlayers, 2 * D), f32)
    Wvd = np.zeros((nlayers, D, 768), f32)
    Wvaug = np.zeros((nlayers, 1, 768), f32)
    W1e = np.zeros((nlayers, D, I), f32)
    b1e = np.zeros((nlayers, I), f32)
    for l in range(nlayers):
        for h in range(H):
            Wqk[l, :, h * DH : (h + 1) * DH] = Wq[l, h] * g1[l][:, None]
            Wqk[l, :, D + h * DH : D + (h + 1) * DH] = Wk[l, h] * g1[l][:, None]
            bqk[l, h * DH : (h + 1) * DH] = bq[l, h] + Wq[l, h].T @ be1[l]
            bqk[l, D + h * DH : D + (h + 1) * DH] = bk[l, h] + Wk[l, h].T @ be1[l]
            Wvd[l, :, 64 * h : 64 * h + DH] = Wv[l, h] * g1[l][:, None]
            Wvaug[l, 0, 64 * h : 64 * h + DH] = bv[l, h] + Wv[l, h].T @ be1[l]
        W1e[l] = W1[l] * g2[l][:, None]
        b1e[l] = b1[l] + W1[l].T @ be2[l]

    Wqk8 = np.zeros((nlayers, 12, KD, 128, 128), fp8)
    Wo8 = np.zeros((nlayers, MD, KD, 128, 128), fp8)
    W1b = np.zeros((nlayers, MI, KD, 128, 128), bf16)
    W2b = np.zeros((nlayers, MD, KI, 128, 128), bf16)
    for l in range(nlayers):
        for m in range(12):
            Wqk8[l, m] = Wqk[l][:, 128 * m : 128 * (m + 1)].reshape(KD, 128, 128)
        for m in range(MD):
            Wo8[l, m] = Wo[l][:, 128 * m : 128 * (m + 1)].reshape(KD, 128, 128)
        for m in range(MI):
            W1b[l, m] = W1e[l][:, 128 * m : 128 * (m + 1)].reshape(KD, 128, 128)
        for m in range(MD):
            W2b[l, m] = W2[l][:, 128 * m : 128 * (m + 1)].reshape(KI, 128, 128)

    return {
        "ident": np.eye(128, dtype=bf16),
        "Wqk": Wqk8,
        "Wo": Wo8,
        "W1": W1b,
        "W2": W2b,
        "Wv": Wvd.astype(fp8),
        "Wvaug": Wvaug.astype(bf16),
        "bqk": bqk,
        "bo": np.ascontiguousarray(bo[:nlayers]),
        "b1": b1e,
        "b2": np.ascontiguousarray(b2[:nlayers]),
    }


_cache = {}


def run_cores(inputs, nlayers=L, trace=False):
    X = np.asarray(inputs["X"], np.float32)
    wmap = prep_weights(inputs, nlayers)

    key = ("nc", nlayers)
    if key not in _cache:
        _cache[key] = build(nlayers)
    nc = _cache[key]

    in_maps = []
    for c in range(NCORES):
        xc = X[BPC * c : BPC * (c + 1)].reshape(T, D).T  # [D, T]
        m = {"xT": np.ascontiguousarray(xc)}
        m.update(wmap)
        in_maps.append(m)

    res = run_bass_kernel_spmd(nc, in_maps, core_ids=list(range(NCORES)), trace=trace)
    out = np.zeros((B, S, D), np.float32)
    for c in range(NCORES):
        out[BPC * c : BPC * (c + 1)] = res.results[c]["out"].T.reshape(BPC, S, D)
    return out, res


def kernel(**inputs):
    out, _ = run_cores(inputs)
    return out


# revision 16
# speedup vs baseline: 1.5892x; 1.0206x over previous
"""ViT-Base encoder (12 layers, B=32, S=197, D=768, H=12, I=3072) on 8 trn2
NeuronCores, data-parallel over the batch (4 images per core).

v2: the attention block (q/k/v projections, Wo) and the LayerNorm stat
reductions run as fp8e4m3 DoubleRow matmuls (2 contraction rows per PE
cell, 2x bf16 throughput); the MLP stays bf16 (fp8 there costs ~6e-2
rel err).  Softmax normalization exploits the ones-columns trick: the
attention matmul leaves the denominator replicated on PSUM rows 64-127,
so a single [64,S] bf16 reciprocal + one multiply normalizes a head
(no PE broadcast, no per-head staging copies).  Activations feeding fp8
matmuls (xn, cat) are stored fp8; the residual stream and LN stats stay
fp32.
"""

import sys

sys.path.insert(0, "/opt/trn_rl_repo")

import contextlib

import numpy as np
import ml_dtypes

import concourse.bass as bass
import concourse.mybir as mybir
import concourse.tile as tile
from concourse.vector_clock import ScopedClock
from concourse.bass_utils import run_bass_kernel_spmd

L, D, I, H, DH = 12, 768, 3072, 12, 64
B, S = 32, 197
NCORES = 8
BPC = B // NCORES  # batches per core
T = BPC * S  # 788 tokens per core
SCALE = float(1.0 / np.sqrt(DH))
EPS = 1e-5

F32 = mybir.dt.float32
BF16 = mybir.dt.bfloat16
FP8 = mybir.dt.float8e4
AF = mybir.ActivationFunctionType
ALU = mybir.AluOpType
PM = mybir.MatmulPerfMode

KD = D // 128  # 6 contraction chunks over D
KI = I // 128  # 24 contraction chunks over I
MD = D // 128  # 6 output tiles over D
MI = I // 128  # 24 output tiles over I
KP = KD // 2  # 3 fp8 DoubleRow contraction pairs over D

NCH = [(0, 394), (394, 394)]  # PSUM-half chunks for dense matmul phases
LCH = [(0, 394), (394, 394)]  # chunks for LN/elementwise work (aligned to NCH)
TCH = [(0, 128), (128, S - 128)]  # within-batch token chunks (128+69)
VW = H * 128  # vt tile: per head [64 v-cols | 64 ones-cols]


class SplitDrainTileContext(tile.TileContext):
    """TileContext whose kernel-tail drain splits its sem waits across
    multiple SP instructions (this walrus rejects >1 wait on a Drain)."""

    def _drain_and_barrier(self, tick_clock, wait_clock):
        nc = self.nc
        drain_inst = nc.sync.drain()
        wait_clock.add_sem_waits(
            drain_inst.ins, ScopedClock({None: tick_clock.global_clock})
        )
        si = drain_inst.ins.sync_info
        waits = list(si.on_wait) if si is not None else []
        if len(waits) > 1:
            drain_inst.ins.sync_info = mybir.SyncInfo(
                on_wait=[waits[0]], on_update=list(si.on_update)
            )
            by_name = {}
            for h in self.sems.allocated().values():
                by_name[getattr(h, "name", None)] = h
            for w in waits[1:]:
                h = by_name.get(w.ant_name)
                assert h is not None, f"no handle for sem {w.ant_name}"
                nc.sync.wait_ge(h, w.wait_value)

        nc.all_engine_barrier()
        assert self.sems is not None
        popped = nc._tile_sem_poison_stack.pop()
        assert popped is self._sem_poison
        nc.clear_and_free_semaphores(list(self.sems.allocated().values()))
        nc.all_engine_barrier()


def _dedup_ldweights(nc):
    """Remove Ldweights whose weights are already resident in the PE array
    (identical signature to the previous Ldweights, nothing invalidated the
    array in between).  Carried sem waits/updates move to the next PE
    instruction; _split_multiwaits hoists any overflow afterwards."""
    removed = 0
    for fn in nc.m.functions:
        for bb in fn.blocks:
            lst = bb.instructions
            last_sig = None
            keep = []
            pending_waits = []
            pending_updates = []
            for inst in lst:
                eng = inst.engine
                if inst.opcode == "Ldweights":
                    sig = (
                        str(inst.ins[0]),
                        str(getattr(inst, "is_transpose", None)),
                        str(getattr(inst, "perf_mode", None)),
                        str(getattr(inst, "tile_position", None)),
                    )
                    if sig == last_sig:
                        si = inst.sync_info
                        if si is not None:
                            pending_waits.extend(si.on_wait)
                            pending_updates.extend(si.on_update)
                        removed += 1
                        continue
                    last_sig = sig
                elif inst.opcode == "Matmult" and str(
                    getattr(inst, "is_transpose", None)
                ) not in ("None", "False"):
                    last_sig = None  # transpose-mode clobbers the array
                if (pending_waits or pending_updates) and eng == mybir.EngineType.PE:
                    si = inst.sync_info
                    ow = list(si.on_wait) if si else []
                    ou = list(si.on_update) if si else []
                    inst.sync_info = mybir.SyncInfo(
                        on_wait=ow + pending_waits, on_update=ou + pending_updates
                    )
                    pending_waits, pending_updates = [], []
                keep.append(inst)
            assert not pending_waits and not pending_updates
            lst[:] = keep
    return removed


def _split_multiwaits(nc):
    """This walrus accepts at most 1 sem wait per instruction (2 on an
    EventSemaphore).  Tile freely packs several; hoist the excess into
    standalone EventSemaphore instructions inserted just before."""
    n = 0
    for fn in nc.m.functions:
        for bb in fn.blocks:
            lst = bb.instructions
            i = 0
            while i < len(lst):
                inst = lst[i]
                si = getattr(inst, "sync_info", None)
                if si is not None and si.on_wait:
                    cap = 2 if inst.opcode == "EventSemaphore" else 1
                    waits = list(si.on_wait)
                    if len(waits) > cap:
                        keep, extra = waits[:cap], waits[cap:]
                        new_insts = []
                        for j in range(0, len(extra), 2):
                            ev = mybir.InstEventSemaphore(
                                name=f"wsplit_{n}", ins=[], outs=[]
                            )
                            n += 1
                            ev.engine = inst.engine
                            ev.sync_info = mybir.SyncInfo(
                                on_wait=list(extra[j : j + 2]), on_update=[]
                            )
                            new_insts.append(ev)
                        inst.sync_info = mybir.SyncInfo(
                            on_wait=keep, on_update=list(si.on_update)
                        )
                        lst[i:i] = new_insts
                        i += len(new_insts)
                i += 1
    return n


def build(nlayers=L):
    nc = bass.Bass()

    xT = nc.dram_tensor("xT", [D, T], F32, kind="ExternalInput")
    Wqk_d = nc.dram_tensor("Wqk", [nlayers, 12, KD, 128, 128], FP8, kind="ExternalInput")
    Wo_d = nc.dram_tensor("Wo", [nlayers, MD, KD, 128, 128], FP8, kind="ExternalInput")
    W1_d = nc.dram_tensor("W1", [nlayers, MI, KD, 128, 128], BF16, kind="ExternalInput")
    W2_d = nc.dram_tensor("W2", [nlayers, MD, KI, 128, 128], BF16, kind="ExternalInput")
    Wv_d = nc.dram_tensor("Wv", [nlayers, D, 768], FP8, kind="ExternalInput")
    Wvaug_d = nc.dram_tensor("Wvaug", [nlayers, 1, 768], BF16, kind="ExternalInput")
    bqk_d = nc.dram_tensor("bqk", [nlayers, 2 * D], F32, kind="ExternalInput")
    bo_d = nc.dram_tensor("bo", [nlayers, D], F32, kind="ExternalInput")
    b1_d = nc.dram_tensor("b1", [nlayers, I], F32, kind="ExternalInput")
    b2_d = nc.dram_tensor("b2", [nlayers, D], F32, kind="ExternalInput")
    ident_d = nc.dram_tensor("ident", [128, 128], BF16, kind="ExternalInput")
    out_d = nc.dram_tensor("out", [D, T], F32, kind="ExternalOutput")

    with SplitDrainTileContext(nc) as tc, contextlib.ExitStack() as ctx, \
         nc.allow_low_precision(reason="fp8 attention, bf16 MLP; residual/stats fp32"):
        persist = ctx.enter_context(tc.tile_pool(name="persist", bufs=1))
        x_sb = persist.tile([128, MD, T], F32, tag="x")
        ones_row = persist.tile([1, 128], BF16, tag="ones_row")
        eps_t = persist.tile([1, 1], F32, tag="eps")
        # DoubleRow stationaries for the LN partition sums: [K=128, pair, col]
        # col 0 sums the tile, col 1 sums the squares tile.
        ones2s = persist.tile([128, 2, 64], FP8, tag="ones2s")
        ones2q = persist.tile([128, 2, 64], FP8, tag="ones2q")
        # vt tile persists so its ones-columns are memset exactly once.
        # Per head: [64 v-cols | 1 ones-col] -> token-major attention output
        # [s, 65] whose col 64 is the softmax denominator (per-partition!).
        vt_sb = persist.tile([128, 2 * BPC, H * 65], BF16, tag="vt")
        ident_b = persist.tile([128, 128], BF16, tag="ident")
        nc.sync.dma_start(out=ident_b, in_=ident_d[:, :])
        nc.vector.memset(ones_row, 1.0)
        nc.vector.memset(eps_t, EPS)
        nc.vector.memset(ones2s, 0.0)
        nc.vector.memset(ones2q, 0.0)
        nc.vector.memset(ones2s[:, :, 0:1], 1.0)
        nc.vector.memset(ones2q[:, :, 32:33], 1.0)
        for i in range(2 * BPC):
            ones_view = vt_sb[:, i, :].rearrange("p (h x) -> p h x", x=65)
            nc.gpsimd.memset(ones_view[:, :, 64:65], 1.0)

        for k in range(KD):
            nc.sync.dma_start(out=x_sb[:, k, :], in_=xT[128 * k : 128 * (k + 1), :])

        rsqrt_fixups = []
        stat_pool = ctx.enter_context(tc.tile_pool(name="stats", bufs=1))
        xn8_pool = ctx.enter_context(tc.tile_pool(name="xn8", bufs=1))
        xn2_pool = ctx.enter_context(tc.tile_pool(name="xn2", bufs=1))
        qk_pool = ctx.enter_context(tc.tile_pool(name="qk", bufs=1))
        h_pool = ctx.enter_context(tc.tile_pool(name="h", bufs=1))
        bias_pool = ctx.enter_context(tc.tile_pool(name="bias", bufs=2))
        wst_pool = ctx.enter_context(tc.tile_pool(name="wst", bufs=8))
        w1st_pool = ctx.enter_context(tc.tile_pool(name="w1st", bufs=8))
        w2st_pool = ctx.enter_context(tc.tile_pool(name="w2st", bufs=4))
        wv_pool = ctx.enter_context(tc.tile_pool(name="wv", bufs=1))
        et_pool = ctx.enter_context(tc.tile_pool(name="expt", bufs=6))
        rec_pool = ctx.enter_context(tc.tile_pool(name="rec", bufs=3))
        xb_pool = ctx.enter_context(tc.tile_pool(name="xb", bufs=1))
        sq_pool = ctx.enter_context(tc.tile_pool(name="sq", bufs=1))
        lnt_pool = ctx.enter_context(tc.tile_pool(name="lnt", bufs=3))

        class LNPipe:
            """LayerNorm over features (partitions).  Stats come from an fp8
            shadow of x via DoubleRow ones-matmuls (sum into PSUM row 0,
            sum-of-squares into row 1); normalization multiplies the fp32
            residual by PE-broadcast stats."""

            def __init__(self, name, src, dst, dst_dtype):
                self.name, self.src, self.dst = name, src, dst
                self.dst_dtype = dst_dtype
                self.mu = stat_pool.tile([1, T], F32, tag="mu", name=name + "_mu")
                self.rs = stat_pool.tile([1, T], F32, tag="rs", name=name + "_rs")
                self.mu_b = stat_pool.tile([1, T], BF16, tag="mu_b", name=name + "_mub")
                self.rs_b = stat_pool.tile([1, T], BF16, tag="rs_b", name=name + "_rsb")
                self.xb = [
                    xb_pool.tile([128, 2, T], FP8, tag=f"xb{p}", name=f"{name}_xb{p}")
                    for p in range(KP)
                ]
                self.sq = [
                    sq_pool.tile([128, 2, T], FP8, tag=f"sq{p}", name=f"{name}_sq{p}")
                    for p in range(KP)
                ]
                self.prepped = set()

            def prep(self, ci, k):
                """fp8 shadow + squares for x[:, k, LCH[ci]] (emit as soon as
                that region is final so it overlaps the producing phase).  The
                last block's cast runs on DVE to keep the LN-sums chain off
                the slow gpsimd queue."""
                off, sz = LCH[ci]
                cs = slice(off, off + sz)
                xbk = self.xb[k // 2][:, k % 2, cs]
                eng = nc.vector if k == KD - 1 else nc.gpsimd
                eng.tensor_copy(xbk, self.src[:, k, cs])
                nc.scalar.activation(self.sq[k // 2][:, k % 2, cs], xbk, AF.Square)
                self.prepped.add((ci, k))

            def sums(self, ci):
                off, sz = LCH[ci]
                cs = slice(off, off + sz)
                for k in range(KD):
                    if (ci, k) not in self.prepped:
                        self.prep(ci, k)
                with tc.tile_pool(
                    name=f"{self.name}_sps{ci}", bufs=1, space="PSUM"
                ) as sps:
                    sp = sps.tile([64, 394], F32, tag="sum", name=f"{self.name}_sum{ci}")
                    for p in range(KP):
                        nc.tensor.matmul(
                            sp[:, :sz], ones2s, self.xb[p][:, :, cs],
                            start=(p == 0), stop=False, perf_mode=PM.DoubleRow,
                            skip_group_check=True,
                        )
                        nc.tensor.matmul(
                            sp[:, :sz], ones2q, self.sq[p][:, :, cs],
                            start=False, stop=(p == KP - 1), perf_mode=PM.DoubleRow,
                            skip_group_check=True,
                        )
                    nc.scalar.mul(self.mu[0:1, cs], sp[0:1, :sz], 1.0 / D)
                    # var = E[x^2] - mu^2, with E[x^2] read straight from PSUM
                    nc.vector.scalar_tensor_tensor(
                        self.rs[:, cs], self.mu[0:1, cs], -1.0, self.mu[0:1, cs],
                        ALU.mult, ALU.mult,
                    )
                    nc.vector.scalar_tensor_tensor(
                        self.rs[:, cs], sp[32:33, :sz], 1.0 / D, self.rs[:, cs],
                        ALU.mult, ALU.add,
                    )

            def finish_stats(self, ci):
                off, sz = LCH[ci]
                cs = slice(off, off + sz)
                h = nc.scalar.activation(
                    self.rs_b[:, cs], self.rs[:, cs], AF.Sqrt, bias=eps_t, scale=1.0
                )
                rsqrt_fixups.append(h.ins)
                nc.scalar.copy(self.mu_b[:, cs], self.mu[0:1, cs])

            def finish_apply(self, ci):
                off, sz = LCH[ci]
                cs = slice(off, off + sz)
                with tc.tile_pool(
                    name=f"{self.name}_bps{ci}", bufs=1, space="PSUM"
                ) as bps:
                    bmu = bps.tile([128, 394], F32, tag="bmu", name=f"{self.name}_bmu{ci}")
                    brs = bps.tile([128, 394], F32, tag="brs", name=f"{self.name}_brs{ci}")
                    nc.tensor.matmul(bmu[:, :sz], ones_row, self.mu_b[:, cs])
                    nc.tensor.matmul(brs[:, :sz], ones_row, self.rs_b[:, cs])
                    for k in range(KD):
                        lnt = lnt_pool.tile(
                            [128, 394], F32, tag="lnt", name=f"{self.name}_lnt_{ci}_{k}"
                        )
                        nc.vector.tensor_sub(
                            lnt[:, :sz], self.src[:, k, cs], bmu[:, :sz]
                        )
                        nc.vector.tensor_mul(
                            self.dst(k)[:, cs], lnt[:, :sz], brs[:, :sz]
                        )

        ln1 = ln2 = None
        for l in range(nlayers):
            wv = wv_pool.tile([128, KD, 768], FP8, tag="wv", name=f"wv_{l}")
            for k in range(KD):
                nc.sync.dma_start(
                    out=wv[:, k, :], in_=Wv_d[l, 128 * k : 128 * (k + 1), :]
                )
            wv_aug = wv_pool.tile([1, 768], BF16, tag="wv_aug", name=f"wva_{l}")
            nc.sync.dma_start(out=wv_aug, in_=Wvaug_d[l, :, :])
            bqk_sb = bias_pool.tile([128, 2 * MD], F32, tag="bqk", name=f"bqk_{l}")
            nc.sync.dma_start(out=bqk_sb, in_=bqk_d[l].rearrange("(m p) -> p m", p=128))

            # ---------------- LN1 -> xn (fp8, split per k-pair) ----------------
            xnp = [
                xn8_pool.tile([128, 2, T], FP8, tag=f"xn{p}", name=f"xn_{l}_{p}")
                for p in range(KP)
            ]
            # pair-contiguous copy of xn for the v DoubleRow stationary:
            # [p, kpair, (b,c)-chunk, pair, token]; tail chunks padded to 128
            # (garbage pad tokens only pollute unread PSUM rows).
            xnv = xn8_pool.tile([128, KP, 2 * BPC, 2, 128], FP8, tag="xnv",
                                name=f"xnv_{l}")

            def emit_xnv(ci):
                for b in (range(0, 2) if ci == 0 else range(2, BPC)):
                    for c, (toff, tsz) in enumerate(TCH):
                        cols = S * b + toff
                        for p in range(KP):
                            nc.gpsimd.tensor_copy(
                                xnv[:, p, 2 * b + c, :, :tsz],
                                xnp[p][:, :, cols : cols + tsz],
                            )
            if ln1 is None:  # first layer: sums/stats not yet emitted by W2
                ln1 = LNPipe(f"ln1_{l}", x_sb, None, FP8)
                ln1.sums(0)
                ln1.sums(1)
                ln1.finish_stats(0)
                ln1.finish_stats(1)
            ln1.dst = lambda k: xnp[k // 2][:, k % 2, :]
            ln1.finish_apply(0)
            ln1.finish_apply(1)

            # ------------- q, k projections (fp8 DoubleRow, ci-outer) -------------
            qk_sb = qk_pool.tile([128, 2 * MD, T], BF16, tag="qk", name=f"qk_{l}")
            with tc.tile_pool(name=f"qkps_{l}", bufs=4, space="PSUM") as qkps:
                for ci, (off, sz) in enumerate(NCH):
                    for m in range(2 * MD):
                        wt = wst_pool.tile(
                            [128, KD, 128], FP8, tag="wst", name=f"wt_{l}_{m}_{ci}"
                        )
                        nc.sync.dma_start(
                            out=wt, in_=Wqk_d[l, m].rearrange("k p c -> p k c")
                        )
                        ps = qkps.tile(
                            [128, 394], F32, tag="ps", name=f"qkps_{l}_{m}_{ci}"
                        )
                        for p in range(KP):
                            nc.tensor.matmul(
                                ps[:, :sz],
                                wt[:, 2 * p : 2 * p + 2, :],
                                xnp[p][:, :, off : off + sz],
                                start=(p == 0),
                                stop=(p == KP - 1),
                                perf_mode=PM.DoubleRow,
                            )
                        nc.scalar.activation(
                            qk_sb[:, m, off : off + sz],
                            ps[:, :sz],
                            AF.Identity,
                            bias=bqk_sb[:, m : m + 1],
                        )
                    emit_xnv(ci)
            q_sb = qk_sb[:, 0:MD, :]
            k_sb = qk_sb[:, MD : 2 * MD, :]
            ln1 = None

            # -------- vT (fp8 DoubleRow; xn stationary, wv moving) --------
            # vt layout per head: [64 v-cols | 64 ones-cols]; the ones are
            # persistent so the attention matmul yields the numerator (rows
            # 0-63) AND the replicated softmax denominator (rows 64-127).
            with tc.tile_pool(name=f"vtps_{l}", bufs=4, space="PSUM") as vtps:
                for b in range(BPC):
                    for c, (toff, tsz) in enumerate(TCH):
                        cols = S * b + toff
                        ps = [
                            vtps.tile(
                                [128, 384], F32, tag="ps", name=f"vtps_{l}_{b}_{c}_{n}"
                            )
                            for n in range(2)
                        ]
                        for p in range(KP):
                            ks = slice(2 * p, 2 * p + 2)
                            for n in range(2):
                                nc.tensor.matmul(
                                    ps[n][:, :],
                                    xnv[:, p, 2 * b + c, :, :],
                                    wv[:, ks, 384 * n : 384 * (n + 1)],
                                    start=(p == 0),
                                    stop=False,
                                    perf_mode=PM.DoubleRow,
                                    skip_group_check=True,
                                )
                        for n in range(2):
                            nc.tensor.matmul(
                                ps[n][:tsz, :],
                                ones_row[:, :tsz],
                                wv_aug[:, 384 * n : 384 * (n + 1)],
                                start=False,
                                stop=True,
                                skip_group_check=True,
                            )
                        dstv = vt_sb[:tsz, 2 * b + c, :].rearrange(
                            "p (h x) -> p h x", x=65
                        )
                        for n in range(2):
                            nc.vector.tensor_copy(
                                dstv[:, 6 * n : 6 * n + 6, 0:64],
                                ps[n][:tsz, :].rearrange("p (h x) -> p h x", x=64),
                            )

            # ---------------- attention (token-major) ----------------
            # attn output per (batch, s-chunk): [s, 12*65] split across two
            # PSUM banks of 6 heads; col 64 of each head-block is the softmax
            # denominator, landing on the token partition so one strided
            # reciprocal + per-partition tensor_scalar normalizes 6 heads.
            # The bf16 normalized tile is transposed back to feature-major
            # fp8 via PE identity-transposes.
            cat_sb = xn8_pool.tile([128, KD, T], FP8, tag="cat", name=f"cat_{l}")
            SCH = [(0, 128), (128, S - 128)]  # s-chunks within a batch
            with tc.tile_pool(name=f"scps_{l}", bufs=2, space="PSUM") as scps, \
                 tc.tile_pool(name=f"tmps_{l}", bufs=4, space="PSUM") as tmps, \
                 tc.tile_pool(name=f"tpps_{l}", bufs=2, space="PSUM") as tpps, \
                 tc.tile_pool(name=f"ctm_{l}", bufs=4) as ctm_pool, \
                 tc.tile_pool(name=f"rcp_{l}", bufs=4) as rcp_pool:

                def emit_norm(b, tm_tiles, cat_tm):
                    for sg, (soff, ssz) in enumerate(SCH):
                        for g in range(2):
                            tmt = tm_tiles[(sg, g)]
                            rcp = rcp_pool.tile(
                                [128, 6], F32, tag="rcp", name=f"rcp_{l}_{b}_{sg}_{g}"
                            )
                            den = tmt[0:ssz, :].rearrange(
                                "p (h x) -> p h x", x=65
                            )[:, :, 64]
                            nc.vector.tensor_copy(rcp[0:ssz, :], den)
                            nc.vector.reciprocal(rcp[0:ssz, :], rcp[0:ssz, :])
                            for j in range(6):
                                nc.vector.tensor_scalar(
                                    out=cat_tm[sg][0:ssz, 64 * (6 * g + j) : 64 * (6 * g + j) + 64],
                                    in0=tmt[0:ssz, 65 * j : 65 * j + 64],
                                    scalar1=rcp[0:ssz, j : j + 1],
                                    scalar2=None,
                                    op0=ALU.mult,
                                )

                def emit_transpose(b, cat_tm):
                    for sg, (soff, ssz) in enumerate(SCH):
                        for f in range(MD):
                            tp = tpps.tile(
                                [128, 128], BF16, tag="tp", name=f"tp_{l}_{b}_{sg}_{f}"
                            )
                            nc.tensor.matmul(
                                tp[:, 0:ssz],
                                cat_tm[sg][0:ssz, 128 * f : 128 * f + 128],
                                ident_b[0:ssz, 0:ssz],
                                is_transpose=True,
                            )
                            dst = cat_sb[:, f, S * b + soff : S * b + soff + ssz]
                            if f % 2 == 0:
                                nc.vector.tensor_copy(dst, tp[:, 0:ssz])
                            else:
                                nc.scalar.copy(dst, tp[:, 0:ssz])

                prev = None
                for b in range(BPC):
                    exp_tiles = {}
                    tm_tiles = {}
                    cat_tm = [
                        ctm_pool.tile(
                            [128, 768], BF16, tag="ctm", name=f"ctm_{l}_{b}_{sg}"
                        )
                        for sg in range(2)
                    ]

                    def emit_scores(h, b=b, exp_tiles=exp_tiles):
                        j, half = h // 2, h % 2
                        rows = slice(64 * half, 64 * half + 64)
                        sps_t = scps.tile(
                            [128, 2 * S], F32, tag="ps", name=f"sc_{l}_{b}_{h}"
                        )
                        for c, (toff, tsz) in enumerate(TCH):
                            cols = S * b + toff
                            nc.tensor.matmul(
                                sps_t[:tsz, S * c : S * c + S],
                                k_sb[rows, j, cols : cols + tsz],
                                q_sb[rows, j, S * b : S * (b + 1)],
                                start=(c == 0),
                                stop=True,
                                skip_group_check=True,
                            )
                        et = et_pool.tile(
                            [128, 2 * S], BF16, tag="expT", name=f"et_{l}_{b}_{h}"
                        )
                        nc.scalar.activation(et, sps_t, AF.Exp, scale=SCALE)
                        exp_tiles[h] = et

                    def emit_attn(h, b=b, exp_tiles=exp_tiles, tm_tiles=tm_tiles):
                        g, j = h // 6, h % 6
                        et = exp_tiles.pop(h)
                        for sg, (soff, ssz) in enumerate(SCH):
                            if (sg, g) not in tm_tiles:
                                tm_tiles[(sg, g)] = tmps.tile(
                                    [128, 390], F32, tag="tm",
                                    name=f"tm_{l}_{b}_{sg}_{g}",
                                )
                            tmt = tm_tiles[(sg, g)]
                            for c, (toff, tsz) in enumerate(TCH):
                                nc.tensor.matmul(
                                    tmt[0:ssz, 65 * j : 65 * j + 65],
                                    et[0:tsz, S * c + soff : S * c + soff + ssz],
                                    vt_sb[0:tsz, 2 * b + c, 65 * h : 65 * h + 65],
                                    start=(j == 0 and c == 0),
                                    stop=(c == 1),
                                    skip_group_check=True,
                                )

                    for h in range(2):
                        emit_scores(h)
                    for h in range(H):
                        if h + 2 < H:
                            emit_scores(h + 2)
                        emit_attn(h)
                    emit_norm(b, tm_tiles, cat_tm)
                    if prev is not None:
                        emit_transpose(*prev)
                    prev = (b, cat_tm)
                emit_transpose(*prev)

            # ------- Wo projection (fp8 DR) + residual --------------
            bo_sb = bias_pool.tile([128, MD], F32, tag="bo", name=f"bo_{l}")
            nc.sync.dma_start(out=bo_sb, in_=bo_d[l].rearrange("(m p) -> p m", p=128))
            xn2p = [
                xn2_pool.tile([128, 2, T], BF16, tag=f"xn2{p}", name=f"xn2_{l}_{p}")
                for p in range(KP)
            ]
            ln2 = LNPipe(f"ln2_{l}", x_sb, None, BF16)
            ln2.dst = lambda k: xn2p[k // 2][:, k % 2, :]
            with tc.tile_pool(name=f"wops_{l}", bufs=4, space="PSUM") as wops:
                for ci, (off, sz) in enumerate(NCH):
                    for m in range(MD):
                        wt = wst_pool.tile(
                            [128, KD, 128], FP8, tag="wst", name=f"wto_{l}_{m}_{ci}"
                        )
                        nc.sync.dma_start(
                            out=wt, in_=Wo_d[l, m].rearrange("k p c -> p k c")
                        )
                        ps = wops.tile(
                            [128, 394], F32, tag="ps", name=f"wops_{l}_{m}_{ci}"
                        )
                        for p in range(KP):
                            ks = slice(2 * p, 2 * p + 2)
                            nc.tensor.matmul(
                                ps[:, :sz],
                                wt[:, ks, :],
                                cat_sb[:, ks, off : off + sz],
                                start=(p == 0),
                                stop=(p == KP - 1),
                                perf_mode=PM.DoubleRow,
                            )
                        nc.vector.scalar_tensor_tensor(
                            x_sb[:, m, off : off + sz],
                            ps[:, :sz],
                            bo_sb[:, m : m + 1],
                            x_sb[:, m, off : off + sz],
                            ALU.add,
                            ALU.add,
                        )
                        ln2.prep(ci, m)
                    ln2.sums(ci)
                    ln2.finish_stats(ci)
                    ln2.finish_apply(ci)

            ln2 = None

            # ---------------- MLP (bf16) ----------------
            b1_sb = bias_pool.tile([128, MI], F32, tag="b1", name=f"b1_{l}")
            nc.sync.dma_start(out=b1_sb, in_=b1_d[l].rearrange("(m p) -> p m", p=128))
            b2_sb = bias_pool.tile([128, MD], F32, tag="b2", name=f"b2_{l}")
            nc.sync.dma_start(out=b2_sb, in_=b2_d[l].rearrange("(m p) -> p m", p=128))
            h_sb = h_pool.tile([128, KI, T], BF16, tag="h", name=f"h_{l}")
            with tc.tile_pool(name=f"w1ps_{l}", bufs=4, space="PSUM") as w1ps:
                for ci, (off, sz) in enumerate(NCH):
                    for m in range(MI):
                        wt = w1st_pool.tile(
                            [128, KD, 128], BF16, tag="w1st", name=f"w1t_{l}_{m}_{ci}"
                        )
                        nc.sync.dma_start(
                            out=wt, in_=W1_d[l, m].rearrange("k p c -> p k c")
                        )
                        ps = w1ps.tile(
                            [128, 394], F32, tag="ps", name=f"w1ps_{l}_{m}_{ci}"
                        )
                        for k in range(KD):
                            nc.tensor.matmul(
                                ps[:, :sz],
                                wt[:, k, :],
                                xn2p[k // 2][:, k % 2, off : off + sz],
                                start=(k == 0),
                                stop=(k == KD - 1),
                            )
                        nc.scalar.activation(
                            h_sb[:, m, off : off + sz],
                            ps[:, :sz],
                            AF.Gelu,
                            bias=b1_sb[:, m : m + 1],
                        )
            xn_next = None
            ln1 = LNPipe(f"ln1n_{l}", x_sb, None, FP8)
            with tc.tile_pool(name=f"w2ps_{l}", bufs=4, space="PSUM") as w2ps:
                for ci, (off, sz) in enumerate(NCH):
                    for m in range(MD):
                        w2t = w2st_pool.tile(
                            [128, KI, 128], BF16, tag="w2st", name=f"w2t_{l}_{m}_{ci}"
                        )
                        nc.sync.dma_start(
                            out=w2t, in_=W2_d[l, m].rearrange("k p c -> p k c")
                        )
                        ps = w2ps.tile(
                            [128, 394], F32, tag="ps", name=f"w2ps_{l}_{m}_{ci}"
                        )
                        for k in range(KI):
                            nc.tensor.matmul(
                                ps[:, :sz],
                                w2t[:, k, :],
                                h_sb[:, k, off : off + sz],
                                start=(k == 0),
                                stop=(k == KI - 1),
                            )
                        nc.vector.scalar_tensor_tensor(
                            x_sb[:, m, off : off + sz],
                            ps[:, :sz],
                            b2_sb[:, m : m + 1],
                            x_sb[:, m, off : off + sz],
                            ALU.add,
                            ALU.add,
                        )
                        if l + 1 < nlayers:
                            ln1.prep(ci, m)
                    if l + 1 < nlayers:
                        ln1.sums(ci)
                        ln1.finish_stats(ci)
            if l + 1 >= nlayers:
                ln1 = None

        for k in range(KD):
            nc.sync.dma_start(out=out_d[128 * k : 128 * (k + 1), :], in_=x_sb[:, k, :])

    for inst in rsqrt_fixups:
        inst.func = mybir.ActivationFunctionType.Rsqrt
    ndedup = _dedup_ldweights(nc)
    nsplit = _split_multiwaits(nc)
    print(f"dedup {ndedup} ldweights; split {nsplit} multi-wait instructions")
    return nc


def prep_weights(inputs, nlayers=L):
    """Fold gamma/beta/biases into effective weights, host side (numpy)."""
    f32 = np.float32
    fp8 = ml_dtypes.float8_e4m3fn
    bf16 = ml_dtypes.bfloat16
    Wq = np.asarray(inputs["Wq"], f32)
    bq = np.asarray(inputs["bq"], f32)
    Wk = np.asarray(inputs["Wk"], f32)
    bk = np.asarray(inputs["bk"], f32)
    Wv = np.asarray(inputs["Wv"], f32)
    bv = np.asarray(inputs["bv"], f32)
    Wo = np.asarray(inputs["Wo"], f32)
    bo = np.asarray(inputs["bo"], f32)
    W1 = np.asarray(inputs["W1"], f32)
    b1 = np.asarray(inputs["b1"], f32)
    W2 = np.asarray(inputs["W2"], f32)
    b2 = np.asarray(inputs["b2"], f32)
    g1 = np.asarray(inputs["g1"], f32)
    be1 = np.asarray(inputs["be1"], f32)
    g2 = np.asarray(inputs["g2"], f32)
    be2 = np.asarray(inputs["be2"], f32)

    Wqk = np.zeros((nlayers, D, 2 * D), f32)
    bqk = np.zeros((nlayers, 2 * D), f32)
    Wvd = np.zeros((nlayers, D, 768), f32)
    Wvaug = np.zeros((nlayers, 1, 768), f32)
    W1e = np.zeros((nlayers, D, I), f32)
    b1e = np.zeros((nlayers, I), f32)
    for l in range(nlayers):
        for h in range(H):
            Wqk[l, :, h * DH : (h + 1) * DH] = Wq[l, h] * g1[l][:, None]
            Wqk[l, :, D + h * DH : D + (h + 1) * DH] = Wk[l, h] * g1[l][:, None]
            bqk[l, h * DH : (h + 1) * DH] = bq[l, h] + Wq[l, h].T @ be1[l]
            bqk[l, D + h * DH : D + (h + 1) * DH] = bk[l, h] + Wk[l, h].T @ be1[l]
            Wvd[l, :, 64 * h : 64 * h + DH] = Wv[l, h] * g1[l][:, None]
            Wvaug[l, 0, 64 * h : 64 * h + DH] = bv[l, h] + Wv[l, h].T @ be1[l]
        W1e[l] = W1[l] * g2[l][:, None]
        b1e[l] = b1[l] + W1[l].T @ be2[l]

    Wqk8 = np.zeros((nlayers, 12, KD, 128, 128), fp8)
    Wo8 = np.zeros((nlayers, MD, KD, 128, 128), fp8)
    W1b = np.zeros((nlayers, MI, KD, 128, 128), bf16)
    W2b = np.zeros((nlayers, MD, KI, 128, 128), bf16)
    for l in range(nlayers):
        for m in range(12):
            Wqk8[l, m] = Wqk[l][:, 128 * m : 128 * (m + 1)].reshape(KD, 128, 128)
        for m in range(MD):
            Wo8[l, m] = Wo[l][:, 128 * m : 128 * (m + 1)].reshape(KD, 128, 128)
        for m in range(MI):
            W1b[l, m] = W1e[l][:, 128 * m : 128 * (m + 1)].reshape(KD, 128, 128)
        for m in range(MD):
            W2b[l, m] = W2[l][:, 128 * m : 128 * (m + 1)].reshape(KI, 128, 128)

    return {
        "ident": np.eye(128, dtype=bf16),
        "Wqk": Wqk8,
        "Wo": Wo8,
        "W1": W1b,
        "W2": W2b,
        "Wv": Wvd.astype(fp8),
        "Wvaug": Wvaug.astype(bf16),
        "bqk": bqk,
        "bo": np.ascontiguousarray(bo[:nlayers]),
        "b1": b1e,
        "b2": np.ascontiguousarray(b2[:nlayers]),
    }


_cache = {}


def run_cores(inputs, nlayers=L, trace=False):
    X = np.asarray(inputs["X"], np.float32)
    wmap = prep_weights(inputs, nlayers)

    key = ("nc", nlayers)
    if key not in _cache:
        _cache[key] = build(nlayers)
    nc = _cache[key]

    in_maps = []
    for c in range(NCORES):
        xc = X[BPC * c : BPC * (c + 1)].reshape(T, D).T  # [D, T]
        m = {"xT": np.ascontiguousarray(xc)}
        m.update(wmap)
        in_maps.append(m)

    res = run_bass_kernel_spmd(nc, in_maps, core_ids=list(range(NCORES)), trace=trace)
    out = np.zeros((B, S, D), np.float32)
    for c in range(NCORES):
        out[BPC * c : BPC * (c + 1)] = res.results[c]["out"].T.reshape(BPC, S, D)
    return out, res


def kernel(**inputs):
    out, _ = run_cores(inputs)
    return out


# revision 17
# speedup vs baseline: 1.6355x; 1.0291x over previous
"""ViT-Base encoder (12 layers, B=32, S=197, D=768, H=12, I=3072) on 8 trn2
NeuronCores, data-parallel over the batch (4 images per core).

v2: the attention block (q/k/v projections, Wo) and the LayerNorm stat
reductions run as fp8e4m3 DoubleRow matmuls (2 contraction rows per PE
cell, 2x bf16 throughput); the MLP stays bf16 (fp8 there costs ~6e-2
rel err).  Softmax normalization exploits the ones-columns trick: the
attention matmul leaves the denominator replicated on PSUM rows 64-127,
so a single [64,S] bf16 reciprocal + one multiply normalizes a head
(no PE broadcast, no per-head staging copies).  Activations feeding fp8
matmuls (xn, cat) are stored fp8; the residual stream and LN stats stay
fp32.
"""

import sys

sys.path.insert(0, "/opt/trn_rl_repo")

import contextlib

import numpy as np
import ml_dtypes

import concourse.bass as bass
import concourse.mybir as mybir
import concourse.tile as tile
from concourse.vector_clock import ScopedClock
from concourse.bass_utils import run_bass_kernel_spmd

L, D, I, H, DH = 12, 768, 3072, 12, 64
B, S = 32, 197
NCORES = 8
BPC = B // NCORES  # batches per core
T = BPC * S  # 788 tokens per core
SCALE = float(1.0 / np.sqrt(DH))
EPS = 1e-5

F32 = mybir.dt.float32
BF16 = mybir.dt.bfloat16
FP8 = mybir.dt.float8e4
AF = mybir.ActivationFunctionType
ALU = mybir.AluOpType
PM = mybir.MatmulPerfMode

KD = D // 128  # 6 contraction chunks over D
KI = I // 128  # 24 contraction chunks over I
MD = D // 128  # 6 output tiles over D
MI = I // 128  # 24 output tiles over I
KP = KD // 2  # 3 fp8 DoubleRow contraction pairs over D

NCH = [(0, 394), (394, 394)]  # PSUM-half chunks for dense matmul phases
LCH = [(0, 394), (394, 394)]  # chunks for LN/elementwise work (aligned to NCH)
TCH = [(0, 128), (128, S - 128)]  # within-batch token chunks (128+69)
VW = H * 128  # vt tile: per head [64 v-cols | 64 ones-cols]


class SplitDrainTileContext(tile.TileContext):
    """TileContext whose kernel-tail drain splits its sem waits across
    multiple SP instructions (this walrus rejects >1 wait on a Drain)."""

    def _drain_and_barrier(self, tick_clock, wait_clock):
        nc = self.nc
        drain_inst = nc.sync.drain()
        wait_clock.add_sem_waits(
            drain_inst.ins, ScopedClock({None: tick_clock.global_clock})
        )
        si = drain_inst.ins.sync_info
        waits = list(si.on_wait) if si is not None else []
        if len(waits) > 1:
            drain_inst.ins.sync_info = mybir.SyncInfo(
                on_wait=[waits[0]], on_update=list(si.on_update)
            )
            by_name = {}
            for h in self.sems.allocated().values():
                by_name[getattr(h, "name", None)] = h
            for w in waits[1:]:
                h = by_name.get(w.ant_name)
                assert h is not None, f"no handle for sem {w.ant_name}"
                nc.sync.wait_ge(h, w.wait_value)

        nc.all_engine_barrier()
        assert self.sems is not None
        popped = nc._tile_sem_poison_stack.pop()
        assert popped is self._sem_poison
        nc.clear_and_free_semaphores(list(self.sems.allocated().values()))
        nc.all_engine_barrier()


def _dedup_ldweights(nc):
    """Remove Ldweights whose weights are already resident in the PE array
    (identical signature to the previous Ldweights, nothing invalidated the
    array in between).  Carried sem waits/updates move to the next PE
    instruction; _split_multiwaits hoists any overflow afterwards."""
    removed = 0
    for fn in nc.m.functions:
        for bb in fn.blocks:
            lst = bb.instructions
            last_sig = None
            keep = []
            pending_waits = []
            pending_updates = []
            for inst in lst:
                eng = inst.engine
                if inst.opcode == "Ldweights":
                    sig = (
                        str(inst.ins[0]),
                        str(getattr(inst, "is_transpose", None)),
                        str(getattr(inst, "perf_mode", None)),
                        str(getattr(inst, "tile_position", None)),
                    )
                    if sig == last_sig:
                        si = inst.sync_info
                        if si is not None:
                            pending_waits.extend(si.on_wait)
                            pending_updates.extend(si.on_update)
                        removed += 1
                        continue
                    last_sig = sig
                elif inst.opcode == "Matmult" and str(
                    getattr(inst, "is_transpose", None)
                ) not in ("None", "False"):
                    last_sig = None  # transpose-mode clobbers the array
                if (pending_waits or pending_updates) and eng == mybir.EngineType.PE:
                    si = inst.sync_info
                    ow = list(si.on_wait) if si else []
                    ou = list(si.on_update) if si else []
                    inst.sync_info = mybir.SyncInfo(
                        on_wait=ow + pending_waits, on_update=ou + pending_updates
                    )
                    pending_waits, pending_updates = [], []
                keep.append(inst)
            assert not pending_waits and not pending_updates
            lst[:] = keep
    return removed


def _split_multiwaits(nc):
    """This walrus accepts at most 1 sem wait per instruction (2 on an
    EventSemaphore).  Tile freely packs several; hoist the excess into
    standalone EventSemaphore instructions inserted just before."""
    n = 0
    for fn in nc.m.functions:
        for bb in fn.blocks:
            lst = bb.instructions
            i = 0
            while i < len(lst):
                inst = lst[i]
                si = getattr(inst, "sync_info", None)
                if si is not None and si.on_wait:
                    cap = 2 if inst.opcode == "EventSemaphore" else 1
                    waits = list(si.on_wait)
                    if len(waits) > cap:
                        keep, extra = waits[:cap], waits[cap:]
                        new_insts = []
                        for j in range(0, len(extra), 2):
                            ev = mybir.InstEventSemaphore(
                                name=f"wsplit_{n}", ins=[], outs=[]
                            )
                            n += 1
                            ev.engine = inst.engine
                            ev.sync_info = mybir.SyncInfo(
                                on_wait=list(extra[j : j + 2]), on_update=[]
                            )
                            new_insts.append(ev)
                        inst.sync_info = mybir.SyncInfo(
                            on_wait=keep, on_update=list(si.on_update)
                        )
                        lst[i:i] = new_insts
                        i += len(new_insts)
                i += 1
    return n


def build(nlayers=L):
    nc = bass.Bass()

    xT = nc.dram_tensor("xT", [D, T], F32, kind="ExternalInput")
    Wqk_d = nc.dram_tensor("Wqk", [nlayers, 12, KD, 128, 128], FP8, kind="ExternalInput")
    Wo_d = nc.dram_tensor("Wo", [nlayers, MD, KD, 128, 128], FP8, kind="ExternalInput")
    W1_d = nc.dram_tensor("W1", [nlayers, MI, KD, 128, 128], BF16, kind="ExternalInput")
    W2_d = nc.dram_tensor("W2", [nlayers, MD, KI, 128, 128], BF16, kind="ExternalInput")
    Wv_d = nc.dram_tensor("Wv", [nlayers, D, 768], FP8, kind="ExternalInput")
    bqk_d = nc.dram_tensor("bqk", [nlayers, 2 * D], F32, kind="ExternalInput")
    bo_d = nc.dram_tensor("bo", [nlayers, D], F32, kind="ExternalInput")
    b1_d = nc.dram_tensor("b1", [nlayers, I], F32, kind="ExternalInput")
    b2_d = nc.dram_tensor("b2", [nlayers, D], F32, kind="ExternalInput")
    ident_d = nc.dram_tensor("ident", [128, 128], BF16, kind="ExternalInput")
    out_d = nc.dram_tensor("out", [D, T], F32, kind="ExternalOutput")

    with SplitDrainTileContext(nc) as tc, contextlib.ExitStack() as ctx, \
         nc.allow_low_precision(reason="fp8 attention, bf16 MLP; residual/stats fp32"):
        persist = ctx.enter_context(tc.tile_pool(name="persist", bufs=1))
        x_sb = persist.tile([128, MD, T], F32, tag="x")
        ones_row = persist.tile([1, 128], BF16, tag="ones_row")
        eps_t = persist.tile([1, 1], F32, tag="eps")
        # DoubleRow stationaries for the LN partition sums: [K=128, pair, col]
        # col 0 sums the tile, col 1 sums the squares tile.
        ones2s = persist.tile([128, 2, 64], FP8, tag="ones2s")
        ones2q = persist.tile([128, 2, 64], FP8, tag="ones2q")
        # vt tile persists so its ones-columns are memset exactly once.
        # Per head: [64 v-cols | 1 ones-col] -> token-major attention output
        # [s, 65] whose col 64 is the softmax denominator (per-partition!).
        vt_sb = persist.tile([128, 2 * BPC, H * 65], BF16, tag="vt")
        ident_b = persist.tile([128, 128], BF16, tag="ident")
        nc.sync.dma_start(out=ident_b, in_=ident_d[:, :])
        nc.vector.memset(ones_row, 1.0)
        nc.vector.memset(eps_t, EPS)
        nc.vector.memset(ones2s, 0.0)
        nc.vector.memset(ones2q, 0.0)
        nc.vector.memset(ones2s[:, :, 0:1], 1.0)
        nc.vector.memset(ones2q[:, :, 32:33], 1.0)
        for i in range(2 * BPC):
            ones_view = vt_sb[:, i, :].rearrange("p (h x) -> p h x", x=65)
            nc.gpsimd.memset(ones_view[:, :, 64:65], 1.0)

        for k in range(KD):
            nc.sync.dma_start(out=x_sb[:, k, :], in_=xT[128 * k : 128 * (k + 1), :])

        rsqrt_fixups = []
        stat_pool = ctx.enter_context(tc.tile_pool(name="stats", bufs=1))
        xn8_pool = ctx.enter_context(tc.tile_pool(name="xn8", bufs=1))
        xn2_pool = ctx.enter_context(tc.tile_pool(name="xn2", bufs=1))
        qk_pool = ctx.enter_context(tc.tile_pool(name="qk", bufs=1))
        h_pool = ctx.enter_context(tc.tile_pool(name="h", bufs=1))
        bias_pool = ctx.enter_context(tc.tile_pool(name="bias", bufs=2))
        wst_pool = ctx.enter_context(tc.tile_pool(name="wst", bufs=8))
        w1st_pool = ctx.enter_context(tc.tile_pool(name="w1st", bufs=8))
        w2st_pool = ctx.enter_context(tc.tile_pool(name="w2st", bufs=4))
        wv_pool = ctx.enter_context(tc.tile_pool(name="wv", bufs=1))
        et_pool = ctx.enter_context(tc.tile_pool(name="expt", bufs=6))
        rec_pool = ctx.enter_context(tc.tile_pool(name="rec", bufs=3))
        xb_pool = ctx.enter_context(tc.tile_pool(name="xb", bufs=1))
        sq_pool = ctx.enter_context(tc.tile_pool(name="sq", bufs=1))
        lnt_pool = ctx.enter_context(tc.tile_pool(name="lnt", bufs=3))

        class LNPipe:
            """LayerNorm over features (partitions).  Stats come from an fp8
            shadow of x via DoubleRow ones-matmuls (sum into PSUM row 0,
            sum-of-squares into row 1); normalization multiplies the fp32
            residual by PE-broadcast stats."""

            def __init__(self, name, src, dst, dst_dtype):
                self.name, self.src, self.dst = name, src, dst
                self.dst_dtype = dst_dtype
                self.mu = stat_pool.tile([1, T], F32, tag="mu", name=name + "_mu")
                self.rs = stat_pool.tile([1, T], F32, tag="rs", name=name + "_rs")
                self.mu_b = stat_pool.tile([1, T], BF16, tag="mu_b", name=name + "_mub")
                self.rs_b = stat_pool.tile([1, T], BF16, tag="rs_b", name=name + "_rsb")
                self.xb = [
                    xb_pool.tile([128, 2, T], FP8, tag=f"xb{p}", name=f"{name}_xb{p}")
                    for p in range(KP)
                ]
                self.sq = [
                    sq_pool.tile([128, 2, T], FP8, tag=f"sq{p}", name=f"{name}_sq{p}")
                    for p in range(KP)
                ]
                self.prepped = set()

            def prep(self, ci, k):
                """fp8 shadow + squares for x[:, k, LCH[ci]] (emit as soon as
                that region is final so it overlaps the producing phase).  The
                last block's cast runs on DVE to keep the LN-sums chain off
                the slow gpsimd queue."""
                off, sz = LCH[ci]
                cs = slice(off, off + sz)
                xbk = self.xb[k // 2][:, k % 2, cs]
                eng = nc.vector if k == KD - 1 else nc.gpsimd
                eng.tensor_copy(xbk, self.src[:, k, cs])
                nc.scalar.activation(self.sq[k // 2][:, k % 2, cs], xbk, AF.Square)
                self.prepped.add((ci, k))

            def sums(self, ci):
                off, sz = LCH[ci]
                cs = slice(off, off + sz)
                for k in range(KD):
                    if (ci, k) not in self.prepped:
                        self.prep(ci, k)
                with tc.tile_pool(
                    name=f"{self.name}_sps{ci}", bufs=1, space="PSUM"
                ) as sps:
                    sp = sps.tile([64, 394], F32, tag="sum", name=f"{self.name}_sum{ci}")
                    for p in range(KP):
                        nc.tensor.matmul(
                            sp[:, :sz], ones2s, self.xb[p][:, :, cs],
                            start=(p == 0), stop=False, perf_mode=PM.DoubleRow,
                            skip_group_check=True,
                        )
                        nc.tensor.matmul(
                            sp[:, :sz], ones2q, self.sq[p][:, :, cs],
                            start=False, stop=(p == KP - 1), perf_mode=PM.DoubleRow,
                            skip_group_check=True,
                        )
                    nc.scalar.mul(self.mu[0:1, cs], sp[0:1, :sz], 1.0 / D)
                    # var = E[x^2] - mu^2, with E[x^2] read straight from PSUM
                    nc.vector.scalar_tensor_tensor(
                        self.rs[:, cs], self.mu[0:1, cs], -1.0, self.mu[0:1, cs],
                        ALU.mult, ALU.mult,
                    )
                    nc.vector.scalar_tensor_tensor(
                        self.rs[:, cs], sp[32:33, :sz], 1.0 / D, self.rs[:, cs],
                        ALU.mult, ALU.add,
                    )

            def finish_stats(self, ci):
                off, sz = LCH[ci]
                cs = slice(off, off + sz)
                h = nc.scalar.activation(
                    self.rs_b[:, cs], self.rs[:, cs], AF.Sqrt, bias=eps_t, scale=1.0
                )
                rsqrt_fixups.append(h.ins)
                nc.scalar.copy(self.mu_b[:, cs], self.mu[0:1, cs])

            def finish_apply(self, ci):
                off, sz = LCH[ci]
                cs = slice(off, off + sz)
                with tc.tile_pool(
                    name=f"{self.name}_bps{ci}", bufs=1, space="PSUM"
                ) as bps:
                    bmu = bps.tile([128, 394], F32, tag="bmu", name=f"{self.name}_bmu{ci}")
                    brs = bps.tile([128, 394], F32, tag="brs", name=f"{self.name}_brs{ci}")
                    nc.tensor.matmul(bmu[:, :sz], ones_row, self.mu_b[:, cs])
                    nc.tensor.matmul(brs[:, :sz], ones_row, self.rs_b[:, cs])
                    for k in range(KD):
                        lnt = lnt_pool.tile(
                            [128, 394], F32, tag="lnt", name=f"{self.name}_lnt_{ci}_{k}"
                        )
                        nc.vector.tensor_sub(
                            lnt[:, :sz], self.src[:, k, cs], bmu[:, :sz]
                        )
                        nc.vector.tensor_mul(
                            self.dst(k)[:, cs], lnt[:, :sz], brs[:, :sz]
                        )

        ln1 = ln2 = None
        for l in range(nlayers):
            wv = wv_pool.tile([128, KD, 768], FP8, tag="wv", name=f"wv_{l}")
            for k in range(KD):
                nc.sync.dma_start(
                    out=wv[:, k, :], in_=Wv_d[l, 128 * k : 128 * (k + 1), :]
                )
            bqk_sb = bias_pool.tile([128, 2 * MD], F32, tag="bqk", name=f"bqk_{l}")
            nc.sync.dma_start(out=bqk_sb, in_=bqk_d[l].rearrange("(m p) -> p m", p=128))

            # ---------------- LN1 -> xn (fp8, split per k-pair) ----------------
            xnp = [
                xn8_pool.tile([128, 2, T], FP8, tag=f"xn{p}", name=f"xn_{l}_{p}")
                for p in range(KP)
            ]
            # pair-contiguous copy of xn for the v DoubleRow stationary:
            # [p, kpair, (b,c)-chunk, pair, token]; tail chunks padded to 128
            # (garbage pad tokens only pollute unread PSUM rows).
            xnv = xn8_pool.tile([128, KP, 2 * BPC, 2, 128], FP8, tag="xnv",
                                name=f"xnv_{l}")

            def emit_xnv(ci):
                for b in (range(0, 2) if ci == 0 else range(2, BPC)):
                    for c, (toff, tsz) in enumerate(TCH):
                        cols = S * b + toff
                        for p in range(KP):
                            nc.gpsimd.tensor_copy(
                                xnv[:, p, 2 * b + c, :, :tsz],
                                xnp[p][:, :, cols : cols + tsz],
                            )
            if ln1 is None:  # first layer: sums/stats not yet emitted by W2
                ln1 = LNPipe(f"ln1_{l}", x_sb, None, FP8)
                ln1.sums(0)
                ln1.sums(1)
                ln1.finish_stats(0)
                ln1.finish_stats(1)
            ln1.dst = lambda k: xnp[k // 2][:, k % 2, :]
            ln1.finish_apply(0)
            ln1.finish_apply(1)

            # ------------- q, k projections (fp8 DoubleRow, ci-outer) -------------
            qk_sb = qk_pool.tile([128, 2 * MD, T], BF16, tag="qk", name=f"qk_{l}")
            with tc.tile_pool(name=f"qkps_{l}", bufs=4, space="PSUM") as qkps:
                for ci, (off, sz) in enumerate(NCH):
                    for m in range(2 * MD):
                        wt = wst_pool.tile(
                            [128, KD, 128], FP8, tag="wst", name=f"wt_{l}_{m}_{ci}"
                        )
                        nc.sync.dma_start(
                            out=wt, in_=Wqk_d[l, m].rearrange("k p c -> p k c")
                        )
                        ps = qkps.tile(
                            [128, 394], F32, tag="ps", name=f"qkps_{l}_{m}_{ci}"
                        )
                        for p in range(KP):
                            nc.tensor.matmul(
                                ps[:, :sz],
                                wt[:, 2 * p : 2 * p + 2, :],
                                xnp[p][:, :, off : off + sz],
                                start=(p == 0),
                                stop=(p == KP - 1),
                                perf_mode=PM.DoubleRow,
                            )
                        nc.scalar.activation(
                            qk_sb[:, m, off : off + sz],
                            ps[:, :sz],
                            AF.Identity,
                            bias=bqk_sb[:, m : m + 1],
                        )
                    emit_xnv(ci)
            q_sb = qk_sb[:, 0:MD, :]
            k_sb = qk_sb[:, MD : 2 * MD, :]
            ln1 = None

            # -------- vT (fp8 DoubleRow; xn stationary, wv moving) --------
            # vt layout per head: [64 v-cols | 64 ones-cols]; the ones are
            # persistent so the attention matmul yields the numerator (rows
            # 0-63) AND the replicated softmax denominator (rows 64-127).
            with tc.tile_pool(name=f"vtps_{l}", bufs=4, space="PSUM") as vtps:
                for b in range(BPC):
                    for c, (toff, tsz) in enumerate(TCH):
                        cols = S * b + toff
                        ps = [
                            vtps.tile(
                                [128, 384], F32, tag="ps", name=f"vtps_{l}_{b}_{c}_{n}"
                            )
                            for n in range(2)
                        ]
                        for p in range(KP):
                            ks = slice(2 * p, 2 * p + 2)
                            for n in range(2):
                                nc.tensor.matmul(
                                    ps[n][:, :],
                                    xnv[:, p, 2 * b + c, :, :],
                                    wv[:, ks, 384 * n : 384 * (n + 1)],
                                    start=(p == 0),
                                    stop=(p == KP - 1),
                                    perf_mode=PM.DoubleRow,
                                    skip_group_check=True,
                                )
                        dstv = vt_sb[:tsz, 2 * b + c, :].rearrange(
                            "p (h x) -> p h x", x=65
                        )
                        for n in range(2):
                            nc.vector.tensor_copy(
                                dstv[:, 6 * n : 6 * n + 6, 0:64],
                                ps[n][:tsz, :].rearrange("p (h x) -> p h x", x=64),
                            )

            # ---------------- attention (token-major) ----------------
            # attn output per (batch, s-chunk): [s, 12*65] split across two
            # PSUM banks of 6 heads; col 64 of each head-block is the softmax
            # denominator, landing on the token partition so one strided
            # reciprocal + per-partition tensor_scalar normalizes 6 heads.
            # The bf16 normalized tile is transposed back to feature-major
            # fp8 via PE identity-transposes.
            cat_sb = xn8_pool.tile([128, KD, T], FP8, tag="cat", name=f"cat_{l}")
            SCH = [(0, 128), (128, S - 128)]  # s-chunks within a batch
            with tc.tile_pool(name=f"scps_{l}", bufs=2, space="PSUM") as scps, \
                 tc.tile_pool(name=f"tmps_{l}", bufs=4, space="PSUM") as tmps, \
                 tc.tile_pool(name=f"tpps_{l}", bufs=2, space="PSUM") as tpps, \
                 tc.tile_pool(name=f"ctm_{l}", bufs=4) as ctm_pool, \
                 tc.tile_pool(name=f"rcp_{l}", bufs=4) as rcp_pool:

                def emit_norm(b, tm_tiles, cat_tm):
                    for sg, (soff, ssz) in enumerate(SCH):
                        for g in range(2):
                            tmt = tm_tiles[(sg, g)]
                            rcp = rcp_pool.tile(
                                [128, 6], F32, tag="rcp", name=f"rcp_{l}_{b}_{sg}_{g}"
                            )
                            den = tmt[0:ssz, :].rearrange(
                                "p (h x) -> p h x", x=65
                            )[:, :, 64]
                            nc.vector.tensor_copy(rcp[0:ssz, :], den)
                            nc.vector.reciprocal(rcp[0:ssz, :], rcp[0:ssz, :])
                            for j in range(6):
                                nc.vector.tensor_scalar(
                                    out=cat_tm[sg][0:ssz, 64 * (6 * g + j) : 64 * (6 * g + j) + 64],
                                    in0=tmt[0:ssz, 65 * j : 65 * j + 64],
                                    scalar1=rcp[0:ssz, j : j + 1],
                                    scalar2=None,
                                    op0=ALU.mult,
                                )

                def emit_transpose(b, cat_tm):
                    for sg, (soff, ssz) in enumerate(SCH):
                        for f in range(MD):
                            tp = tpps.tile(
                                [128, 128], BF16, tag="tp", name=f"tp_{l}_{b}_{sg}_{f}"
                            )
                            nc.tensor.matmul(
                                tp[:, 0:ssz],
                                cat_tm[sg][0:ssz, 128 * f : 128 * f + 128],
                                ident_b[0:ssz, 0:ssz],
                                is_transpose=True,
                            )
                            dst = cat_sb[:, f, S * b + soff : S * b + soff + ssz]
                            if f % 2 == 0:
                                nc.vector.tensor_copy(dst, tp[:, 0:ssz])
                            else:
                                nc.scalar.copy(dst, tp[:, 0:ssz])

                prev = None
                for b in range(BPC):
                    exp_tiles = {}
                    tm_tiles = {}
                    cat_tm = [
                        ctm_pool.tile(
                            [128, 768], BF16, tag="ctm", name=f"ctm_{l}_{b}_{sg}"
                        )
                        for sg in range(2)
                    ]

                    def emit_scores(h, b=b, exp_tiles=exp_tiles):
                        j, half = h // 2, h % 2
                        rows = slice(64 * half, 64 * half + 64)
                        sps_t = scps.tile(
                            [128, 2 * S], F32, tag="ps", name=f"sc_{l}_{b}_{h}"
                        )
                        for c, (toff, tsz) in enumerate(TCH):
                            cols = S * b + toff
                            nc.tensor.matmul(
                                sps_t[:tsz, S * c : S * c + S],
                                k_sb[rows, j, cols : cols + tsz],
                                q_sb[rows, j, S * b : S * (b + 1)],
                                start=(c == 0),
                                stop=True,
                                skip_group_check=True,
                            )
                        et = et_pool.tile(
                            [128, 2 * S], BF16, tag="expT", name=f"et_{l}_{b}_{h}"
                        )
                        nc.scalar.activation(et, sps_t, AF.Exp, scale=SCALE)
                        exp_tiles[h] = et

                    def emit_attn(h, b=b, exp_tiles=exp_tiles, tm_tiles=tm_tiles):
                        g, j = h // 6, h % 6
                        et = exp_tiles.pop(h)
                        for sg, (soff, ssz) in enumerate(SCH):
                            if (sg, g) not in tm_tiles:
                                tm_tiles[(sg, g)] = tmps.tile(
                                    [128, 390], F32, tag="tm",
                                    name=f"tm_{l}_{b}_{sg}_{g}",
                                )
                            tmt = tm_tiles[(sg, g)]
                            for c, (toff, tsz) in enumerate(TCH):
                                nc.tensor.matmul(
                                    tmt[0:ssz, 65 * j : 65 * j + 65],
                                    et[0:tsz, S * c + soff : S * c + soff + ssz],
                                    vt_sb[0:tsz, 2 * b + c, 65 * h : 65 * h + 65],
                                    start=(j == 0 and c == 0),
                                    stop=(c == 1),
                                    skip_group_check=True,
                                )

                    for h in range(2):
                        emit_scores(h)
                    for h in range(H):
                        if h + 2 < H:
                            emit_scores(h + 2)
                        emit_attn(h)
                    emit_norm(b, tm_tiles, cat_tm)
                    if prev is not None:
                        emit_transpose(*prev)
                    prev = (b, cat_tm)
                emit_transpose(*prev)

            # ------- Wo projection (fp8 DR) + residual --------------
            bo_sb = bias_pool.tile([128, MD], F32, tag="bo", name=f"bo_{l}")
            nc.sync.dma_start(out=bo_sb, in_=bo_d[l].rearrange("(m p) -> p m", p=128))
            xn2p = [
                xn2_pool.tile([128, 2, T], BF16, tag=f"xn2{p}", name=f"xn2_{l}_{p}")
                for p in range(KP)
            ]
            ln2 = LNPipe(f"ln2_{l}", x_sb, None, BF16)
            ln2.dst = lambda k: xn2p[k // 2][:, k % 2, :]
            with tc.tile_pool(name=f"wops_{l}", bufs=4, space="PSUM") as wops:
                for ci, (off, sz) in enumerate(NCH):
                    for m in range(MD):
                        wt = wst_pool.tile(
                            [128, KD, 128], FP8, tag="wst", name=f"wto_{l}_{m}_{ci}"
                        )
                        nc.sync.dma_start(
                            out=wt, in_=Wo_d[l, m].rearrange("k p c -> p k c")
                        )
                        ps = wops.tile(
                            [128, 394], F32, tag="ps", name=f"wops_{l}_{m}_{ci}"
                        )
                        for p in range(KP):
                            ks = slice(2 * p, 2 * p + 2)
                            nc.tensor.matmul(
                                ps[:, :sz],
                                wt[:, ks, :],
                                cat_sb[:, ks, off : off + sz],
                                start=(p == 0),
                                stop=(p == KP - 1),
                                perf_mode=PM.DoubleRow,
                            )
                        nc.vector.scalar_tensor_tensor(
                            x_sb[:, m, off : off + sz],
                            ps[:, :sz],
                            bo_sb[:, m : m + 1],
                            x_sb[:, m, off : off + sz],
                            ALU.add,
                            ALU.add,
                        )
                        ln2.prep(ci, m)
                    ln2.sums(ci)
                    ln2.finish_stats(ci)
                    ln2.finish_apply(ci)

            ln2 = None

            # ---------------- MLP (bf16) ----------------
            b1_sb = bias_pool.tile([128, MI], F32, tag="b1", name=f"b1_{l}")
            nc.sync.dma_start(out=b1_sb, in_=b1_d[l].rearrange("(m p) -> p m", p=128))
            b2_sb = bias_pool.tile([128, MD], F32, tag="b2", name=f"b2_{l}")
            nc.sync.dma_start(out=b2_sb, in_=b2_d[l].rearrange("(m p) -> p m", p=128))
            h_sb = h_pool.tile([128, KI, T], BF16, tag="h", name=f"h_{l}")
            with tc.tile_pool(name=f"w1ps_{l}", bufs=4, space="PSUM") as w1ps:
                for ci, (off, sz) in enumerate(NCH):
                    for m in range(MI):
                        wt = w1st_pool.tile(
                            [128, KD, 128], BF16, tag="w1st", name=f"w1t_{l}_{m}_{ci}"
                        )
                        nc.sync.dma_start(
                            out=wt, in_=W1_d[l, m].rearrange("k p c -> p k c")
                        )
                        ps = w1ps.tile(
                            [128, 394], F32, tag="ps", name=f"w1ps_{l}_{m}_{ci}"
                        )
                        for k in range(KD):
                            nc.tensor.matmul(
                                ps[:, :sz],
                                wt[:, k, :],
                                xn2p[k // 2][:, k % 2, off : off + sz],
                                start=(k == 0),
                                stop=(k == KD - 1),
                            )
                        nc.scalar.activation(
                            h_sb[:, m, off : off + sz],
                            ps[:, :sz],
                            AF.Gelu,
                            bias=b1_sb[:, m : m + 1],
                        )
            xn_next = None
            ln1 = LNPipe(f"ln1n_{l}", x_sb, None, FP8)
            with tc.tile_pool(name=f"w2ps_{l}", bufs=4, space="PSUM") as w2ps:
                for ci, (off, sz) in enumerate(NCH):
                    for m in range(MD):
                        w2t = w2st_pool.tile(
                            [128, KI, 128], BF16, tag="w2st", name=f"w2t_{l}_{m}_{ci}"
                        )
                        nc.sync.dma_start(
                            out=w2t, in_=W2_d[l, m].rearrange("k p c -> p k c")
                        )
                        ps = w2ps.tile(
                            [128, 394], F32, tag="ps", name=f"w2ps_{l}_{m}_{ci}"
                        )
                        for k in range(KI):
                            nc.tensor.matmul(
                                ps[:, :sz],
                                w2t[:, k, :],
                                h_sb[:, k, off : off + sz],
                                start=(k == 0),
                                stop=(k == KI - 1),
                            )
                        nc.vector.scalar_tensor_tensor(
                            x_sb[:, m, off : off + sz],
                            ps[:, :sz],
                            b2_sb[:, m : m + 1],
                            x_sb[:, m, off : off + sz],
                            ALU.add,
                            ALU.add,
                        )
                        if l + 1 < nlayers:
                            ln1.prep(ci, m)
                    if l + 1 < nlayers:
                        ln1.sums(ci)
                        ln1.finish_stats(ci)
            if l + 1 >= nlayers:
                ln1 = None

        for k in range(KD):
            nc.sync.dma_start(out=out_d[128 * k : 128 * (k + 1), :], in_=x_sb[:, k, :])

    for inst in rsqrt_fixups:
        inst.func = mybir.ActivationFunctionType.Rsqrt
    ndedup = _dedup_ldweights(nc)
    nsplit = _split_multiwaits(nc)
    print(f"dedup {ndedup} ldweights; split {nsplit} multi-wait instructions")
    return nc


def prep_weights(inputs, nlayers=L):
    """Fold gamma/beta/biases into effective weights, host side (numpy)."""
    f32 = np.float32
    fp8 = ml_dtypes.float8_e4m3fn
    bf16 = ml_dtypes.bfloat16
    Wq = np.asarray(inputs["Wq"], f32)
    bq = np.asarray(inputs["bq"], f32)
    Wk = np.asarray(inputs["Wk"], f32)
    bk = np.asarray(inputs["bk"], f32)
    Wv = np.asarray(inputs["Wv"], f32)
    bv = np.asarray(inputs["bv"], f32)
    Wo = np.asarray(inputs["Wo"], f32)
    bo = np.asarray(inputs["bo"], f32)
    W1 = np.asarray(inputs["W1"], f32)
    b1 = np.asarray(inputs["b1"], f32)
    W2 = np.asarray(inputs["W2"], f32)
    b2 = np.asarray(inputs["b2"], f32)
    g1 = np.asarray(inputs["g1"], f32)
    be1 = np.asarray(inputs["be1"], f32)
    g2 = np.asarray(inputs["g2"], f32)
    be2 = np.asarray(inputs["be2"], f32)

    Wqk = np.zeros((nlayers, D, 2 * D), f32)
    bqk = np.zeros((nlayers, 2 * D), f32)
    Wvd = np.zeros((nlayers, D, 768), f32)
    bveff = np.zeros((nlayers, 768), f32)
    W1e = np.zeros((nlayers, D, I), f32)
    b1e = np.zeros((nlayers, I), f32)
    for l in range(nlayers):
        for h in range(H):
            Wqk[l, :, h * DH : (h + 1) * DH] = Wq[l, h] * g1[l][:, None]
            Wqk[l, :, D + h * DH : D + (h + 1) * DH] = Wk[l, h] * g1[l][:, None]
            bqk[l, h * DH : (h + 1) * DH] = bq[l, h] + Wq[l, h].T @ be1[l]
            bqk[l, D + h * DH : D + (h + 1) * DH] = bk[l, h] + Wk[l, h].T @ be1[l]
            Wvd[l, :, 64 * h : 64 * h + DH] = Wv[l, h] * g1[l][:, None]
            bveff[l, 64 * h : 64 * h + DH] = bv[l, h] + Wv[l, h].T @ be1[l]
        W1e[l] = W1[l] * g2[l][:, None]
        b1e[l] = b1[l] + W1[l].T @ be2[l]

    Wqk8 = np.zeros((nlayers, 12, KD, 128, 128), fp8)
    Wo8 = np.zeros((nlayers, MD, KD, 128, 128), fp8)
    W1b = np.zeros((nlayers, MI, KD, 128, 128), bf16)
    W2b = np.zeros((nlayers, MD, KI, 128, 128), bf16)
    for l in range(nlayers):
        for m in range(12):
            Wqk8[l, m] = Wqk[l][:, 128 * m : 128 * (m + 1)].reshape(KD, 128, 128)
        for m in range(MD):
            Wo8[l, m] = Wo[l][:, 128 * m : 128 * (m + 1)].reshape(KD, 128, 128)
        for m in range(MI):
            W1b[l, m] = W1e[l][:, 128 * m : 128 * (m + 1)].reshape(KD, 128, 128)
        for m in range(MD):
            W2b[l, m] = W2[l][:, 128 * m : 128 * (m + 1)].reshape(KI, 128, 128)

    return {
        "ident": np.eye(128, dtype=bf16),
        "Wqk": Wqk8,
        "Wo": Wo8,
        "W1": W1b,
        "W2": W2b,
        "Wv": Wvd.astype(fp8),
        "bqk": bqk,
        "bo": np.ascontiguousarray(
            bo[:nlayers]
            + np.einsum("lde,ld->le", Wo[:nlayers], bveff)
        ),
        "b1": b1e,
        "b2": np.ascontiguousarray(b2[:nlayers]),
    }


_cache = {}


def run_cores(inputs, nlayers=L, trace=False):
    X = np.asarray(inputs["X"], np.float32)
    wmap = prep_weights(inputs, nlayers)

    key = ("nc", nlayers)
    if key not in _cache:
        _cache[key] = build(nlayers)
    nc = _cache[key]

    in_maps = []
    for c in range(NCORES):
        xc = X[BPC * c : BPC * (c + 1)].reshape(T, D).T  # [D, T]
        m = {"xT": np.ascontiguousarray(xc)}
        m.update(wmap)
        in_maps.append(m)

    res = run_bass_kernel_spmd(nc, in_maps, core_ids=list(range(NCORES)), trace=trace)
    out = np.zeros((B, S, D), np.float32)
    for c in range(NCORES):
        out[BPC * c : BPC * (c + 1)] = res.results[c]["out"].T.reshape(BPC, S, D)
    return out, res


def kernel(**inputs):
    out, _ = run_cores(inputs)
    return out
